# revision 1
# baseline (speedup 1.0000x reference)
"""Trainium2 Bass kernel v2 for Box3dEncoder (nn_Box3dEncoder_75453985456565).

Same contract as v1. Redesign highlights:
  - K-form stage 2: per edge, the clipped-x integral is evaluated through
    K(u) = relu(u)^2 at 9 telescoped i-offsets; iedge_i = D[i] - D[i+1]
    with D = K(ghi) - K(glo). 6 elementwise ops per (h, i-half) instead of 11.
  - j-prep (t-window lo/hi per edge/row) is computed on the host and shipped
    in the per-core consts DMA, removing it from the device critical path.
  - Two input DMAs (consts on the Pool queue, weights on SP) instead of four.
  - A dummy warm-up matmul pins the PE p-state ramp start near t=0 so all
    real fp32 matmuls run at the full 2.4 GHz rate.
  - Stage 3 work is spread: max on DVE, onehot-compare on Pool, PSUM->SBUF
    copies on Act, selection via PE transpose+matmul, outputs on Pool's DMA
    queue.
"""
import numpy as np

B, N, K = 4, 16, 8
CUBE = (64, 64, 8)
LOW = (-32, -32, -4)
NCORES = 8
NI = CUBE[0] // NCORES          # 8 i-rows per core
NJ = CUBE[1]                    # 64
NCELL = NI * NJ                 # 512 cells per core
NCHUNK = NCELL // 128           # 4
V = CUBE[0] * CUBE[1] * CUBE[2]

OFF_LOHI = 0                    # [h][ep][j]  2*2*64 = 256
OFF_XM = 256                    # [h][i]      2*9 = 18
OFF_DX = 274                    # [h]         2
OFF_HV = 276                    # halfvol     1
CWP = 280

RW_W = 512                      # rw [h][256]
W4_OFF = 512                    # w4 96
ID_OFF = 608                    # ident 128
RWW_W = 736


def _host_prep_v2(corners3d, neck_voxel_sizes):
    f32 = np.float32
    c = np.asarray(corners3d, f32)
    vs = np.asarray(neck_voxel_sizes, f32)[0]
    vox_vol = f32(vs[0]) * f32(vs[1]) * f32(vs[2])
    halfvol = f32(0.5) * vox_vol

    poly = c[:, :, :4, :2]                     # [B,N,4,2]
    nxt = np.roll(poly, -1, axis=2)
    xa, ya = poly[..., 0], poly[..., 1]        # [B,N,4]
    xb, yb = nxt[..., 0], nxt[..., 1]
    dx, dy = xb - xa, yb - ya
    vert = np.abs(dx) < f32(1e-4)
    if bool(vert.any()):
        return None                            # caller falls back to v1
    invdy = (f32(1) / dy).astype(f32)
    w1 = (dy / dx * f32(0.5)).astype(f32)      # [B,N,4]

    zb0 = c[:, :, :, 2].min(axis=2)
    zb1 = c[:, :, :, 2].max(axis=2)
    quad_area = f32(0.5) * np.abs((xa * yb - xb * ya).sum(axis=2, dtype=f32))
    box_vol = (quad_area * (zb1 - zb0)).astype(f32)
    C = (vox_vol + box_vol + f32(1e-9)).astype(f32)
    invC = (f32(1) / C).astype(f32)

    kk = np.arange(K, dtype=f32) + LOW[2]
    z0 = kk * vs[2]
    z1 = (kk + 1) * vs[2]
    zov = np.maximum(np.minimum(z1[None, :, None], zb1[:, None, :])
                     - np.maximum(z0[None, :, None], zb0[:, None, :]),
                     f32(0)).astype(f32)       # [B,K,N]
    eps = (f32(15) - np.arange(N, dtype=f32)) * f32(2.0 ** -20)
    zrho = (zov * invC[:, None, :] * (1 + eps)[None, None, :]).astype(f32)
    C_w4 = (C / (1 + eps)[None, :]).astype(f32)

    d0 = c[:, :, 0, :2] - c[:, :, 3, :2]
    h0 = np.sqrt(d0[..., 0] ** 2 + d0[..., 1] ** 2)
    hs = np.where(h0 == 0, f32(1), h0)
    sin = np.where(h0 > 0, d0[..., 1] / hs, f32(0)).astype(f32)
    cos = np.where(h0 > 0, d0[..., 0] / hs, f32(1)).astype(f32)

    # host j-prep: t-window [lo,hi] of y(t) in [y0_j, y0_j+vs] per (b,n,e,j)
    jj = np.arange(NJ, dtype=f32) + LOW[1]
    y0j = (jj * vs[1]).astype(f32)
    ty0 = ((y0j[None, None, None, :] - ya[..., None]) * invdy[..., None]).astype(f32)
    ty1 = (ty0 + (vs[1] * invdy)[..., None]).astype(f32)
    # note: reference computes ty1 from (y0+vs-ya)*invdy; keep that exact form
    ty1 = ((y0j[None, None, None, :] + vs[1] - ya[..., None])
           * invdy[..., None]).astype(f32)
    u0 = np.clip(ty0, f32(0), f32(1))
    u1 = np.clip(ty1, f32(0), f32(1))
    lo = np.minimum(u0, u1).astype(f32)        # [B,N,4,64]
    hi = np.maximum(u0, u1).astype(f32)

    def colpack(a):                            # [B,N,4] -> [2h][128p]
        return np.ascontiguousarray(a.reshape(2, 2, N, 4).reshape(2, 128))

    # consts per core; dx is folded into lo/hi on the host so the device
    # G op is a plain tensor_tensor (legal on GpSimd)
    lohi = np.stack([lo, hi], axis=3) * dx[..., None, None]   # [B,N,4,2,64]
    lohi_p = (lohi.reshape(2, 2, N, 4, 2, NJ)  # (h, b_lo, n, e, ep, j)
              .transpose(1, 2, 3, 0, 4, 5)     # (b_lo, n, e, h, ep, j)
              .reshape(128, 256))
    xa_p = colpack(xa)
    dx_p = colpack(dx)

    consts = []
    for m in range(NCORES):
        cc = np.zeros((128, CWP), f32)
        cc[:, OFF_LOHI:OFF_LOHI + 256] = lohi_p
        ii = np.arange(NI + 1, dtype=f32) + (m * NI + LOW[0])
        x0i = (ii * vs[0]).astype(f32)         # [9]
        for h in range(2):
            cc[:, OFF_XM + h * 9:OFF_XM + (h + 1) * 9] = \
                x0i[None, :] - xa_p[h][:, None]
            cc[:, OFF_DX + h] = dx_p[h]
        cc[:, OFF_HV] = halfvol
        consts.append(np.ascontiguousarray(cc))

    # rw [128, 2, 256]: zrho (w1 folded) at (b_lo, k, n) per partition
    w1p = colpack(w1)
    rw = np.zeros((128, 2, 2, K, N), f32)
    for h in range(2):
        for p in range(128):
            b_lo, n = p // 64, (p % 64) // 4
            rw[p, h, b_lo, :, n] = zrho[2 * h + b_lo, :, n] * w1p[h, p]
    rw = rw.reshape(128, 512)

    # w4 [128, 96]: selection weights, p = (k, n)
    w4 = np.zeros((128, B, K, 3), f32)
    for p in range(128):
        kq, n = p // N, p % N
        w4[p, :, kq, 0] = C_w4[:, n]
        w4[p, :, kq, 1] = sin[:, n]
        w4[p, :, kq, 2] = cos[:, n]
    w4 = w4.reshape(128, B * K * 3)

    ident = np.eye(128, dtype=f32)
    rww = np.ascontiguousarray(np.concatenate([rw, w4, ident], axis=1))
    return consts, rww


def _build_v2(chain_mode='own', s3_mode='grouped', assign=None):
    import concourse.bass as bass
    import concourse.tile as tile
    from concourse import bacc, mybir

    f32 = mybir.dt.float32
    ALU = mybir.AluOpType
    ACT = mybir.ActivationFunctionType
    X = mybir.AxisListType.X

    nc = bacc.Bacc("TRN2", target_bir_lowering=False, debug=False,
                   num_devices=NCORES)
    d_consts = nc.dram_tensor("consts", [128, CWP], f32, kind="ExternalInput")
    d_rww = nc.dram_tensor("rww", [128, RWW_W], f32, kind="ExternalInput")
    d_out = nc.dram_tensor("out", [B, NCELL * K, 2], f32, kind="ExternalOutput")

    with tile.TileContext(nc) as tc:
        with (
            tc.tile_pool(name="const", bufs=1) as cpool,
            tc.tile_pool(name="const2", bufs=1) as cpool2,
            tc.tile_pool(name="s2", bufs=4) as wpool,
            tc.tile_pool(name="s3", bufs=1) as tpool,
            tc.tile_pool(name="psr", bufs=1, space=bass.MemorySpace.PSUM) as prho,
            tc.tile_pool(name="psoh", bufs=2, space=bass.MemorySpace.PSUM) as poh,
            tc.tile_pool(name="pssel", bufs=1, space=bass.MemorySpace.PSUM) as psel,
        ):
            # --- input DMAs: consts on Pool queue, weights on SP queue ---
            tco = cpool.tile([128, CWP], f32, tag="consts")
            nc.sync.dma_start(tco[:], d_consts[:])
            trww = cpool2.tile([128, RWW_W], f32, tag="rww")
            nc.sync.dma_start(trww[:], d_rww[:])

            # --- PE warm-up: pin the p-state ramp start near t=0 ---
            zz = cpool.tile([128, 8], f32, tag="zz")
            nc.vector.memset(zz[:], 0.0)

            trw = trww[:, 0:RW_W].rearrange("p (h q) -> p h q", q=256)
            ident = trww[:, ID_OFF:ID_OFF + 128]
            hv_col = tco[:, OFF_HV:OFF_HV + 1]

            rho_ps = []
            for cch in range(NCHUNK):
                rp = prho.tile([128, 2 * K * N * 2], f32, tag=f"rho{cch}",
                               name=f"rho{cch}")
                rho_ps.append(rp)
            # PE warm-up target: scratch corner of rho bank 0, overwritten
            # later by the real start=True matmul
            nc.tensor.matmul(rho_ps[0][:8, 0:8], zz[:], zz[:],
                             start=True, stop=True)

            # per-iter engine assignment: (G, relu, sq, D, ie)
            # engines: V=vector/DVE, P=gpsimd/Pool, A=scalar/Act
            # (G, relu, sq, D, ie); relu=None -> K fused as (G max 0) * G
            ASSIGN = assign or [
                ('V', 'A', 'A', 'V', 'P'),
                ('P', None, 'V', 'V', 'P'),
                ('V', 'A', 'A', 'V', 'P'),
                ('P', None, 'V', 'V', 'P'),
            ]

            def eng(c):
                return {'V': nc.vector, 'P': nc.gpsimd, 'A': nc.scalar}[c]

            def stage2(half, h):
                # G[p, ep, i, j] = dx*lohi[ep,j] - x0mxa[i], i in the half's
                # 5-wide telescoped window
                lohi = tco[:, OFF_LOHI + h * 128:OFF_LOHI + (h + 1) * 128] \
                    .rearrange("p (ep j) -> p ep j", j=NJ)
                xm = tco[:, OFF_XM + h * 9 + half * 4:
                         OFF_XM + h * 9 + half * 4 + 5]
                xm_e = xm[:, :, None].broadcast_to([128, 5, NJ])

                it = half * 2 + h
                eG, eR, eK, eD, eI = ASSIGN[it]
                G = wpool.tile([128, 2, 5, NJ], f32, tag="G", name="G")
                for ep in range(2):
                    lohi_e = lohi[:, ep, :][:, None, :].broadcast_to(
                        [128, 5, NJ])
                    eng(eG).tensor_tensor(G[:, ep], lohi_e, xm_e,
                                          ALU.subtract)
                Ksq = wpool.tile([128, 2, 5, NJ], f32, tag="Ksq", name="Ksq")
                Kf = Ksq[:].rearrange("p a b c -> p (a b c)")
                Gf = G[:].rearrange("p a b c -> p (a b c)")
                if eR is None:
                    # K = relu(G)^2 = max(G,0)*G in one STT op
                    eng(eK).scalar_tensor_tensor(Kf, Gf, zz[:, 0:1],
                                                 Gf, ALU.max, ALU.mult)
                else:
                    # per-endpoint relu+square pipelines the Act chain
                    # behind each G endpoint; relu='T' runs as a DVE
                    # tensor_scalar (2x_2p fast mode), halving Act's chain
                    R = wpool.tile([128, 2, 5, NJ], f32, tag="R", name="R")
                    for ep in range(2):
                        if eR == 'H' and ep == 0:
                            # endpoint 0 fused on DVE, endpoint 1 on Act
                            nc.vector.scalar_tensor_tensor(
                                Ksq[:, ep], G[:, ep], zz[:, 0:1], G[:, ep],
                                ALU.max, ALU.mult)
                            continue
                        if eR == 'T':
                            nc.vector.tensor_scalar(R[:, ep], G[:, ep],
                                                    0.0, None, ALU.max)
                        else:
                            nc.scalar.activation(R[:, ep], G[:, ep], ACT.Relu)
                        if eK == 'A':
                            nc.scalar.activation(Ksq[:, ep], R[:, ep],
                                                 ACT.Square)
                        else:
                            eng(eK).tensor_tensor(Ksq[:, ep], R[:, ep],
                                                  R[:, ep], ALU.mult)
                D = wpool.tile([128, 5, NJ], f32, tag="D", name="D")
                eng(eD).tensor_tensor(D[:], Ksq[:, 1], Ksq[:, 0],
                                      ALU.subtract)
                ie = wpool.tile([128, 4, NJ], f32, tag="ie", name="ie")
                eng(eI).tensor_tensor(ie[:], D[:, 0:4, :], D[:, 1:5, :],
                                      ALU.subtract)
                return G, Ksq, D, ie

            tiles = {}
            for half in range(2):
                for h in range(2):
                    tiles[(half, h)] = stage2(half, h)
            ies = {k: v[3] for k, v in tiles.items()}
            # p-state ramp chain: tiny matmuls dep'd on successively later
            # stage-2 tiles keep PE.SEQ occupied so the real matmuls are
            # dispatched >3us after PE's first barrier and run at full rate.
            # Writing into both rho banks (WAW) keeps the scheduler from
            # hoisting any real matmul ahead of the chain.
            K01 = tiles[(0, 1)][1][:, 0, 0, 0:1]
            D01 = tiles[(0, 1)][2][:, 0, 0:1]
            ie00 = tiles[(0, 0)][3]
            ie01 = tiles[(0, 1)][3]
            ie10 = tiles[(1, 0)][3]
            if chain_mode == 'own':
                chain = [(K01, 0), (D01, 1), (ie00[:, 0, 0:1], 2),
                         (ie01[:, 0, 0:1], 3), (ie00[:, 1, 0:1], 0),
                         (ie01[:, 1, 0:1], 1)]
            elif chain_mode == 'late':
                chain = [(K01, 0), (D01, 1), (ie01[:, 0, 0:1], 2),
                         (ie10[:, 0, 0:1], 3), (ie10[:, 1, 0:1], 0),
                         (ie10[:, 2, 0:1], 1), (ie10[:, 3, 0:1], 2),
                         (ie01[:, 1, 0:1], 3)]
            elif chain_mode == 'rev':
                K10 = tiles[(1, 0)][1][:, 0, 0, 0:1]
                K11 = tiles[(1, 1)][1][:, 0, 0, 0:1]
                ie11 = tiles[(1, 1)][3]
                chain = [(K10, 0), (K11, 1), (ie10[:, 0, 0:1], 2),
                         (ie11[:, 0, 0:1], 3), (ie00[:, 0, 0:1], 0),
                         (ie00[:, 1, 0:1], 1), (ie00[:, 2, 0:1], 2),
                         (ie00[:, 3, 0:1], 3)]
            elif chain_mode == 'mid':
                D11 = tiles[(1, 1)][2]
                chain = [(K01, 0), (D01, 1), (ie01[:, 0, 0:1], 2),
                         (D11[:, 0, 0:1], 3), (D11[:, 1, 0:1], 0),
                         (D11[:, 2, 0:1], 1), (D11[:, 3, 0:1], 2),
                         (ie01[:, 1, 0:1], 3)]
            else:
                chain = []
            for ct, bank in chain:
                nc.tensor.matmul(rho_ps[bank][:1, 0:1], zz[:, 0:1], ct,
                                 start=True, stop=True)
            HORD = (1, 0) if s3_mode == 'pairs-rev' else (0, 1)
            for half in HORD:
                for cc_l in range(2):
                    cch = half * 2 + cc_l
                    for h in range(2):
                        nc.tensor.matmul(
                            rho_ps[cch][:, h * 256:(h + 1) * 256],
                            ies[(half, h)][:, cc_l * 2:cc_l * 2 + 2, :]
                            .rearrange("p i j -> p (i j)"),
                            trw[:, h, :], start=True, stop=True)

            # ---- stage 3 ----
            maxr, oneh, ohss, sels = {}, {}, {}, {}

            def s3_max(cch):
                rho3 = rho_ps[cch][:].rearrange("p (g n) -> p g n", n=N)
                maxrho = tpool.tile([128, B * K], f32, tag=f"maxrho{cch}",
                                    name=f"maxrho{cch}")
                nc.vector.reduce_max(maxrho[:], rho3, axis=X)
                maxr[cch] = maxrho

            def s3_eq(cch):
                rho3 = rho_ps[cch][:].rearrange("p (g n) -> p g n", n=N)
                mx_b = maxr[cch][:][:, :, None].broadcast_to([128, B * K, N])
                onehot = tpool.tile([128, B * K * N], f32, tag=f"onehot{cch}",
                                    name=f"onehot{cch}")
                oh3 = onehot[:].rearrange("p (g n) -> p g n", n=N)
                nc.vector.tensor_tensor(oh3, rho3, mx_b, ALU.is_equal)
                oneh[cch] = onehot

            def s3_tr(cch):
                oh_t = poh.tile([128, 4 * 128], f32, tag="oht", name="oht")
                for b in range(B):
                    nc.tensor.transpose(oh_t[:, b * 128:(b + 1) * 128],
                                        oneh[cch][:, b * 128:(b + 1) * 128],
                                        ident)
                ohs = tpool.tile([128, 4 * 128], f32, tag=f"ohs{cch}",
                                 name=f"ohs{cch}")
                nc.scalar.copy(ohs[:], oh_t[:])
                ohss[cch] = ohs

            def s3_sel(cch):
                sel = psel.tile([128, B * K * 3], f32, tag=f"sel{cch % 2}",
                                name=f"sel{cch}")
                for b in range(B):
                    nc.tensor.matmul(
                        sel[:, b * K * 3:(b + 1) * K * 3],
                        ohss[cch][:, b * 128:(b + 1) * 128],
                        trww[:, W4_OFF + b * K * 3:W4_OFF + (b + 1) * K * 3],
                        start=True, stop=True)
                sels[cch] = sel

            def s3_out(cch):
                selg = sels[cch][:].rearrange("p (g q) -> p g q", q=3)
                inter2 = tpool.tile([128, B * K], f32, tag=f"inter2{cch}",
                                    name=f"inter2{cch}")
                nc.vector.tensor_tensor(inter2[:], maxr[cch][:],
                                        selg[:, :, 0], ALU.mult)
                # outt = (inter2 > halfvol) * sel_sincos, fused in one STT
                outt = tpool.tile([128, B * K, 2], f32, tag=f"outt{cch}",
                                  name=f"outt{cch}")
                i2b = inter2[:][:, :, None].broadcast_to([128, B * K, 2])
                nc.vector.scalar_tensor_tensor(outt[:], i2b, hv_col,
                                               selg[:, :, 1:3], ALU.is_gt,
                                               ALU.mult)
                nc.sync.dma_start(
                    d_out[:, cch * 128 * K:(cch + 1) * 128 * K, :]
                    .rearrange("b (p k) e -> p b k e", k=K),
                    outt[:].rearrange("p (b k) e -> p b k e", k=K))

            stages = [s3_max, s3_eq, s3_tr, s3_sel, s3_out]
            if s3_mode == 'grouped':
                for st in stages:
                    for cch in range(NCHUNK):
                        st(cch)
            elif s3_mode == 'pairs':
                for cch in range(NCHUNK):
                    s3_max(cch)
                    s3_eq(cch)
                for st in (s3_tr, s3_sel, s3_out):
                    for cch in range(NCHUNK):
                        st(cch)
            elif s3_mode == 'pairs-rev':
                for cch in (2, 3, 0, 1):
                    s3_max(cch)
                    s3_eq(cch)
                for st in (s3_tr, s3_sel, s3_out):
                    for cch in (2, 3, 0, 1):
                        st(cch)
            else:
                for cch in range(NCHUNK):
                    for st in stages:
                        st(cch)

    nc.compile()
    return nc



BIG = 1024.0


def _host_prep_v1(corners3d, neck_voxel_sizes):
    c = np.asarray(corners3d, np.float32)
    vs = np.asarray(neck_voxel_sizes, np.float32)[0]
    vox_vol = np.float32(vs[0]) * np.float32(vs[1]) * np.float32(vs[2])

    poly = c[:, :, :4, :2]                     # [B,N,4,2]
    nxt = np.roll(poly, -1, axis=2)
    xa, ya = poly[..., 0], poly[..., 1]        # [B,N,4]
    xb, yb = nxt[..., 0], nxt[..., 1]
    dx, dy = xb - xa, yb - ya
    with np.errstate(divide='ignore'):
        inv_dx = np.where(np.abs(dx) < 1e-12, np.float32(0),
                          np.float32(1) / np.where(dx == 0, np.float32(1), dx))
        inv_dy = np.where(np.abs(dy) < 1e-12, np.float32(0),
                          np.float32(1) / np.where(dy == 0, np.float32(1), dy))

    zb0 = c[:, :, :, 2].min(axis=2)
    zb1 = c[:, :, :, 2].max(axis=2)
    quad_area = 0.5 * np.abs((xa * yb - xb * ya).sum(axis=2))
    box_vol = quad_area * (zb1 - zb0)
    C = (vox_vol + box_vol + np.float32(1e-9)).astype(np.float32)   # [B,N]
    invC = (np.float32(1) / C).astype(np.float32)

    kk = np.arange(K, dtype=np.float32) + LOW[2]
    z0 = kk * vs[2]
    z1 = (kk + 1) * vs[2]
    zov = np.maximum(np.minimum(z1[None, :, None], zb1[:, None, :])
                     - np.maximum(z0[None, :, None], zb0[:, None, :]),
                     np.float32(0))                                  # [B,K,N]
    # tie-break epsilon: rho_n scaled by (1+eps_n), eps decreasing in n, so a
    # single reduce_max + is_equal yields the first-occurrence argmax; the C
    # selection weight is divided by (1+eps_n) to compensate exactly.
    eps = (np.float32(15) - np.arange(N, dtype=np.float32)) * np.float32(2.0 ** -20)
    zrho = (zov * invC[:, None, :] * (1 + eps)[None, None, :]).astype(np.float32)
    C_w4 = (C / (1 + eps)[None, :]).astype(np.float32)

    d = c[:, :, 0, :2] - c[:, :, 3, :2]
    h = np.sqrt(d[..., 0] ** 2 + d[..., 1] ** 2)
    hs = np.where(h == 0, np.float32(1), h)
    sin = np.where(h > 0, d[..., 1] / hs, np.float32(0)).astype(np.float32)
    cos = np.where(h > 0, d[..., 0] / hs, np.float32(1)).astype(np.float32)

    # --- stage-2 per-partition columns, p = b_lo*64 + n*4 + e, per h-iter ---
    def colpack(a):    # [B,N,4] -> [2h][128]
        return a.reshape(2, 2, N, 4).reshape(2, 128)
    DX_EPS = np.float32(1e-4)
    vert = np.abs(dx) < DX_EPS
    w1 = np.where(vert, np.float32(0), dy * inv_dx * np.float32(0.5))
    w2 = np.where(vert, dy, np.float32(0))
    cols = np.zeros((2, 128, 8), np.float32)
    cols[:, :, 0] = colpack(inv_dy)
    cols[:, :, 1] = colpack(-ya * inv_dy)
    cols[:, :, 2] = colpack((vs[1] - ya) * inv_dy)
    cols[:, :, 3] = colpack(dx)
    cols[:, :, 4] = colpack(xa)
    cols[:, :, 5] = colpack(w1)
    cols[:, :, 6] = colpack(w2)
    cols = np.ascontiguousarray(cols.transpose(1, 0, 2))       # [128,2,8]

    # edge-reduction weights with zrho folded in:
    # rw2[p=(b_lo,n,e), h, (b_lo',k,n')] = (b_lo'==b_lo & n'==n) * dy/2 * zrho[b,k,n]
    novert = not bool(vert.any())
    w1p = colpack(w1)                          # [2,128]
    rw = np.zeros((128, 2, 2, K, N), np.float32)
    for h in range(2):
        for p in range(128):
            b_lo, n = p // 64, (p % 64) // 4
            scale = w1p[h, p] if novert else np.float32(1)
            rw[p, h, b_lo, :, n] = zrho[2 * h + b_lo, :, n] * scale
    rw = np.ascontiguousarray(rw.reshape(128, 2, 2 * K * N))   # [128,2,256]

    # selection matmul weights: w4[b][(k,n), (k',q)] = (k==k') * w_q[b,n]
    # q in {C, sin, cos}
    w4 = np.zeros((128, B, K, 3), np.float32)
    for p in range(128):
        kq, n = p // N, p % N
        w4[p, :, kq, 0] = C_w4[:, n]
        w4[p, :, kq, 1] = sin[:, n]
        w4[p, :, kq, 2] = cos[:, n]
    w4 = np.ascontiguousarray(w4.reshape(128, B * K * 3))      # [128,96]

    # --- cells-major broadcast constants (partition-replicated by host) ---
    kbig = BIG - np.arange(N, dtype=np.float32)                # [16]
    jj = np.arange(NJ, dtype=np.float32) + LOW[1]
    y0 = (jj * vs[1]).astype(np.float32)                       # [64]
    halfvol = np.float32(0.5) * vox_vol

    consts = []
    for m in range(NCORES):
        ii = np.arange(NI, dtype=np.float32) + (m * NI + LOW[0])
        x0 = (ii * vs[0]).astype(np.float32)                   # [8]
        row = np.concatenate([kbig, y0, x0,
                              [halfvol, vs[0], -vs[0], 2 * vs[0]]]).astype(np.float32)
        cc = np.concatenate(
            [np.broadcast_to(row, (128, row.size)), cols.reshape(128, 16)],
            axis=1).astype(np.float32)
        consts.append(np.ascontiguousarray(cc))
    ident = np.eye(128, dtype=np.float32)
    return rw, w4, ident, consts, novert


def _build_v1(stages='all', fuse_w1=False):
    import concourse.bass as bass
    import concourse.tile as tile
    from concourse import bacc, mybir

    f32 = mybir.dt.float32
    ALU = mybir.AluOpType
    ACT = mybir.ActivationFunctionType

    CW = 16 + 64 + 8 + 4 + 16
    OFF_KBIG, OFF_Y0, OFF_X0, OFF_MISC, OFF_COLS = 0, 16, 80, 88, 92

    nc = bacc.Bacc("TRN2", target_bir_lowering=False, debug=False,
                   num_devices=NCORES)
    d_consts = nc.dram_tensor("consts", [128, CW], f32, kind="ExternalInput")
    d_rw = nc.dram_tensor("rw", [128, 2, 256], f32, kind="ExternalInput")
    d_w4 = nc.dram_tensor("w4", [128, B * K * 3], f32, kind="ExternalInput")
    d_ident = nc.dram_tensor("ident", [128, 128], f32, kind="ExternalInput")
    d_out = nc.dram_tensor("out", [B, NCELL * K, 2], f32, kind="ExternalOutput")

    with tile.TileContext(nc) as tc:
        with (
            tc.tile_pool(name="const", bufs=1) as cpool,
            tc.tile_pool(name="small", bufs=4) as spool,
            tc.tile_pool(name="work", bufs=6) as wpool,
            tc.tile_pool(name="edge", bufs=4) as epool,
            tc.tile_pool(name="st3", bufs=6) as tpool,
            tc.tile_pool(name="outp", bufs=4) as opool,
            tc.tile_pool(name="psum", bufs=1, space=bass.MemorySpace.PSUM) as ppool,
            tc.tile_pool(name="psum2", bufs=2, space=bass.MemorySpace.PSUM) as ppool2,
        ):
            tco = cpool.tile([128, CW], f32, tag="consts")
            nc.sync.dma_start(tco[:], d_consts[:])
            trw = cpool.tile([128, 2, 256], f32, tag="rw")
            nc.scalar.dma_start(trw[:], d_rw[:])
            tw4 = cpool.tile([128, B * K * 3], f32, tag="w4")
            nc.sync.dma_start(tw4[:], d_w4[:])
            ident = cpool.tile([128, 128], f32, tag="ident")
            nc.sync.dma_start(ident[:], d_ident[:])

            kbig_bc = tco[:, OFF_KBIG:OFF_KBIG + 16]
            y0_bc = tco[:, OFF_Y0:OFF_Y0 + 64]
            x0_bc = tco[:, OFF_X0:OFF_X0 + 8]
            halfvol_col = tco[:, OFF_MISC:OFF_MISC + 1]
            vs0_col = tco[:, OFF_MISC + 1:OFF_MISC + 2]
            nvs0_col = tco[:, OFF_MISC + 2:OFF_MISC + 3]
            vs0x2_col = tco[:, OFF_MISC + 3:OFF_MISC + 4]

            def bj(ap):   # [128,64] j-tile -> broadcast over i: [128,8,64]
                return ap[:, None, :].broadcast_to([128, NI, NJ])

            def bi(ap):   # [128,8] i-tile -> broadcast over j: [128,8,64]
                return ap[:, :, None].broadcast_to([128, NI, NJ])

            rho_ps = []
            for c in range(NCHUNK):
                rp = ppool.tile([128, B * K * N], f32, tag=f"rho{c}")
                rho_ps.append(rp)
            for h in range(2 if stages != 'none' else 0):
                col = lambda q: tco[:, OFF_COLS + h * 8 + q:OFF_COLS + h * 8 + q + 1]
                # j-only quantities [128, 64]
                ty0 = spool.tile([128, NJ], f32, tag="ty0")
                nc.scalar.activation(ty0[:], y0_bc, ACT.Identity,
                                     bias=col(1), scale=col(0))
                ty1 = spool.tile([128, NJ], f32, tag="ty1")
                nc.scalar.activation(ty1[:], y0_bc, ACT.Identity,
                                     bias=col(2), scale=col(0))
                u0 = spool.tile([128, NJ], f32, tag="u0")
                nc.vector.tensor_scalar(u0[:], ty0[:], 0.0, 1.0, ALU.max, ALU.min)
                u1 = spool.tile([128, NJ], f32, tag="u1")
                nc.vector.tensor_scalar(u1[:], ty1[:], 0.0, 1.0, ALU.max, ALU.min)
                lo = spool.tile([128, NJ], f32, tag="lo")
                nc.vector.tensor_tensor(lo[:], u0[:], u1[:], ALU.min)
                hi = spool.tile([128, NJ], f32, tag="hi")
                nc.vector.tensor_tensor(hi[:], u0[:], u1[:], ALU.max)
                hilo = spool.tile([128, NJ], f32, tag="hilo")
                nc.vector.tensor_tensor(hilo[:], hi[:], lo[:], ALU.subtract)
                # i-only quantities [128, 8]
                x0mxa = spool.tile([128, NI], f32, tag="x0mxa")
                nc.vector.tensor_single_scalar(x0mxa[:], x0_bc, col(4),
                                               ALU.subtract)
                # vertical-edge fallback: Fv = clamp(xa - x0, 0, vs0), * w2
                fvw = spool.tile([128, NI], f32, tag="fvw")
                nc.vector.tensor_scalar(fvw[:], x0mxa[:], -1.0, 0.0,
                                        ALU.mult, ALU.max)
                nc.vector.tensor_single_scalar(fvw[:], fvw[:], vs0_col, ALU.min)
                nc.vector.tensor_single_scalar(fvw[:], fvw[:], col(6), ALU.mult)

                NIH = NI // 2
                for half in range(2):
                    isl = slice(half * NIH, (half + 1) * NIH)

                    def full(tag):
                        t = wpool.tile([128, NIH, NJ], f32, tag=tag)
                        return t

                    def bjh(ap):
                        return ap[:, None, :].broadcast_to([128, NIH, NJ])

                    def bih(ap):
                        return ap[:, isl, None].broadcast_to([128, NIH, NJ])

                    # g at t=lo and t=hi:  g = dx*t - (x0 - xa)
                    glo = full("glo")
                    nc.vector.scalar_tensor_tensor(glo[:], bjh(lo[:]), col(3),
                                                   bih(x0mxa[:]), ALU.mult,
                                                   ALU.subtract)
                    ghi = full("ghi")
                    nc.vector.scalar_tensor_tensor(ghi[:], bjh(hi[:]), col(3),
                                                   bih(x0mxa[:]), ALU.mult,
                                                   ALU.subtract)
                    # H(u) = 0.5*clamp(u,0,c)^2 + c*relu(u-c); w1 carries 0.5
                    clo = full("clo")
                    nc.vector.tensor_scalar(clo[:], glo[:], 0.0, vs0_col,
                                            ALU.max, ALU.min)
                    chi = full("chi")
                    nc.gpsimd.tensor_scalar(chi[:], ghi[:], 0.0, vs0_col,
                                            ALU.max, ALU.min)
                    sqlo = full("sqlo")
                    nc.scalar.activation(sqlo[:], clo[:], ACT.Square)
                    sqhi = full("sqhi")
                    nc.scalar.activation(sqhi[:], chi[:], ACT.Square)
                    rlo = full("rlo")
                    nc.scalar.activation(rlo[:], glo[:], ACT.Relu,
                                         bias=nvs0_col)
                    rhi = full("rhi")
                    nc.scalar.activation(rhi[:], ghi[:], ACT.Relu,
                                         bias=nvs0_col)
                    e1 = full("e1")
                    nc.vector.tensor_tensor(e1[:], sqhi[:], sqlo[:],
                                            ALU.subtract)
                    e2 = full("e2")
                    nc.gpsimd.tensor_tensor(e2[:], rhi[:], rlo[:],
                                            ALU.subtract)
                    iedge = epool.tile([128, NCELL // 2], f32, tag="iedge")
                    if fuse_w1:
                        # w1 folded into rw: iedge = 2c*e2 + e1 directly
                        nc.vector.scalar_tensor_tensor(
                            iedge[:].rearrange("p (i j) -> p i j", j=NJ),
                            e2[:], vs0x2_col, e1[:], ALU.mult, ALU.add)
                    else:
                        s = full("s")
                        nc.vector.scalar_tensor_tensor(s[:], e2[:], vs0x2_col,
                                                       e1[:], ALU.mult, ALU.add)
                        t2w = full("t2w")
                        nc.gpsimd.tensor_tensor(t2w[:], bih(fvw[:]), bjh(hilo[:]),
                                                ALU.mult)
                        nc.vector.scalar_tensor_tensor(
                            iedge[:].rearrange("p (i j) -> p i j", j=NJ),
                            s[:], col(5), t2w[:], ALU.mult, ALU.add)

                    for cc in range(2):
                        cch = half * 2 + cc
                        nc.tensor.matmul(
                            rho_ps[cch][:, h * 256:(h + 1) * 256],
                            iedge[:, cc * 128:(cc + 1) * 128],
                            trw[:, h, :], start=True, stop=True)

            # ---- stage 3, cells-major, per 128-cell chunk ----
            for cch in range(NCHUNK if stages == 'all' else 0):
                rho3 = rho_ps[cch][:].rearrange("p (g n) -> p g n", n=N)
                maxrho = tpool.tile([128, B * K], f32, tag="maxrho")
                nc.vector.reduce_max(maxrho[:], rho3, axis=mybir.AxisListType.X)
                mx_bc = maxrho[:][:, :, None].broadcast_to([128, B * K, N])
                onehot = tpool.tile([128, B * K * N], f32, tag="onehot")
                nc.vector.tensor_tensor(
                    onehot[:].rearrange("p (g n) -> p g n", n=N), rho3, mx_bc,
                    ALU.is_equal)

                # selection sums via PE: transpose onehot per b, then matmul
                # against w4 -> SEL[cell, (k, {C,sin,cos})]
                oh_t = ppool2.tile([128, 4 * 128], f32, tag="oht")
                sel_ps = ppool2.tile([128, B * K * 3], f32, tag="selps")
                for b in range(B):
                    nc.tensor.transpose(
                        oh_t[:, b * 128:(b + 1) * 128],
                        onehot[:, b * 128:(b + 1) * 128], ident[:])
                ohs = tpool.tile([128, 4 * 128], f32, tag="ohs")
                nc.scalar.copy(ohs[:], oh_t[:])
                for b in range(B):
                    nc.tensor.matmul(
                        sel_ps[:, b * K * 3:(b + 1) * K * 3],
                        ohs[:, b * 128:(b + 1) * 128],
                        tw4[:, b * K * 3:(b + 1) * K * 3],
                        start=True, stop=True)

                sel3 = sel_ps[:].rearrange("p (b k q) -> p b k q", k=K, q=3)
                mx3 = maxrho[:].rearrange("p (b k) -> p b k", k=K)
                intersel = tpool.tile([128, B * K], f32, tag="intersel")
                nc.vector.tensor_tensor(
                    intersel[:].rearrange("p (b k) -> p b k", k=K),
                    mx3, sel3[:, :, :, 0], ALU.mult)
                mask = tpool.tile([128, B * K], f32, tag="mask")
                nc.vector.tensor_single_scalar(mask[:], intersel[:],
                                               halfvol_col, ALU.is_gt)
                outt = opool.tile([128, B, K, 2], f32, tag="outt")
                nc.vector.tensor_tensor(
                    outt[:, :, :, 0], sel3[:, :, :, 1],
                    mask[:].rearrange("p (b k) -> p b k", k=K), ALU.mult)
                nc.vector.tensor_tensor(
                    outt[:, :, :, 1], sel3[:, :, :, 2],
                    mask[:].rearrange("p (b k) -> p b k", k=K), ALU.mult)
                dma_eng = nc.sync
                dma_eng.dma_start(
                    d_out[:, cch * 128 * K:(cch + 1) * 128 * K, :]
                         .rearrange("b (p k) e -> p b k e", k=K),
                    outt[:])

    if stages != 'all':
        with tile.TileContext(nc) as tc2:
            with tc2.tile_pool(name="fin", bufs=1) as fpool:
                z = fpool.tile([128, 64], f32, tag="z")
                nc.gpsimd.memset(z[:], 0.0)
                nc.gpsimd.dma_start(
                    d_out[:, 0:1024, :].rearrange("b (p k) e -> p b k e", k=K),
                    z[:].rearrange("p (b k e) -> p b k e", k=K, e=2))
    nc.compile()
    return nc




_COMPILED = None


def kernel(corners3d, neck_voxel_sizes):
    global _COMPILED
    from concourse.bass_utils import run_bass_kernel_spmd

    prep = _host_prep_v2(corners3d, neck_voxel_sizes)
    if prep is not None:
        consts, rww = prep
        if _COMPILED is None or _COMPILED[0] != 'v2':
            _COMPILED = ('v2', _build_v2(chain_mode='late', s3_mode='pairs'))
        nc = _COMPILED[1]
        in_maps = [{"consts": consts[m], "rww": rww} for m in range(NCORES)]
    else:
        # near-vertical box edges: fall back to the v1 kernel
        rw, w4, ident, consts1, novert = _host_prep_v1(corners3d,
                                                       neck_voxel_sizes)
        if _COMPILED is None or _COMPILED[0] != ('v1', novert):
            _COMPILED = (('v1', novert), _build_v1(fuse_w1=novert))
        nc = _COMPILED[1]
        in_maps = [{"consts": consts1[m], "rw": rw, "w4": w4,
                    "ident": ident} for m in range(NCORES)]
    res = run_bass_kernel_spmd(nc, in_maps, list(range(NCORES)))
    out = np.zeros((B, V, 2), np.float32)
    for m in range(NCORES):
        blk = res.results[m]["out"]
        out[:, m * NCELL * K:(m + 1) * NCELL * K, :] = blk
    return out.reshape(B * V, 2)



# revision 10
# speedup vs baseline: 1.0890x; 1.0890x over previous
"""Trainium2 Bass kernel v3 for Box3dEncoder (nn_Box3dEncoder_75453985456565).

v3 redesign vs v2:
  - j-telescoped stage 2: per edge the row-clip boundaries satisfy
    ty1(j) == ty0(j+1), so K(u)=relu(u)^2 is evaluated once per shared row
    boundary (65 of them) instead of per (lo,hi) pair: G/K volume halves.
    Direction sign s=sign(dy) is folded into the rw matmul weights.
  - ie is folded into the PE: both D' = K_i - K_{i+1} and its negation are
    computed (operand-swapped subtract), and rho accumulates two f32r
    matmuls (j+1-shifted D' and -D') per (chunk, h) in PSUM.
  - float32r matmuls at 1 cycle/row (>=256 cols) with a dep-staggered PE
    warm-up chain.
  - stage 3 rebalanced: reduce on DVE, is_equal/io mostly on Pool (c3 on
    DVE), PSUM->SBUF copies on Act (c3 on DVE), out DMAs on SP/Act queues.
"""
import numpy as np

B, N, K = 4, 16, 8
CUBE = (64, 64, 8)
LOW = (-32, -32, -4)
NCORES = 8
NI = CUBE[0] // NCORES          # 8 i-rows per core
NJ = CUBE[1]                    # 64
NB = NJ + 1                     # 65 row boundaries
NCELL = NI * NJ                 # 512 cells per core
NCHUNK = NCELL // 128           # 4
V = CUBE[0] * CUBE[1] * CUBE[2]

# v3 consts layout [128, CW3]: ub_h0 [65] | ub_h1 [65] | xm_h0 [9] | xm_h1 [9] | hv
OFF_UB = 0
OFF_XM3 = 130
OFF_HV3 = 148
CW3 = 149

RW_W = 512
W4_OFF = 512
ID_OFF = 608
RWW_W = 736


def _colpack128(a):              # [B,N,4] -> [2h][128p], p = b_lo*64+n*4+e
    return np.ascontiguousarray(a.reshape(2, 2, N, 4).reshape(2, 128))


def _host_prep_v3(corners3d, neck_voxel_sizes):
    f32 = np.float32
    c = np.asarray(corners3d, f32)
    vs = np.asarray(neck_voxel_sizes, f32)[0]
    vox_vol = f32(vs[0]) * f32(vs[1]) * f32(vs[2])
    halfvol = f32(0.5) * vox_vol

    poly = c[:, :, :4, :2]
    nxt = np.roll(poly, -1, axis=2)
    xa, ya = poly[..., 0], poly[..., 1]
    xb, yb = nxt[..., 0], nxt[..., 1]
    dx, dy = xb - xa, yb - ya
    if bool((np.abs(dx) < 1e-4).any()) or bool((np.abs(dy) < 1e-6).any()):
        return None
    invdy = (f32(1) / dy).astype(f32)
    s = np.sign(dy).astype(f32)
    w1s = (dy / dx * f32(0.5) * s).astype(f32)

    zb0 = c[:, :, :, 2].min(axis=2)
    zb1 = c[:, :, :, 2].max(axis=2)
    quad_area = f32(0.5) * np.abs((xa * yb - xb * ya).sum(axis=2, dtype=f32))
    box_vol = (quad_area * (zb1 - zb0)).astype(f32)
    C = (vox_vol + box_vol + f32(1e-9)).astype(f32)
    invC = (f32(1) / C).astype(f32)

    kk = np.arange(K, dtype=f32) + LOW[2]
    z0 = kk * vs[2]
    z1 = (kk + 1) * vs[2]
    zov = np.maximum(np.minimum(z1[None, :, None], zb1[:, None, :])
                     - np.maximum(z0[None, :, None], zb0[:, None, :]),
                     f32(0)).astype(f32)       # [B,K,N]
    # first-occurrence-argmax tie-break epsilons (see v2)
    eps = (f32(15) - np.arange(N, dtype=f32)) * f32(2.0 ** -20)
    zrho = (zov * invC[:, None, :] * (1 + eps)[None, None, :]).astype(f32)
    C_w4 = (C / (1 + eps)[None, :]).astype(f32)

    d0 = c[:, :, 0, :2] - c[:, :, 3, :2]
    h0 = np.sqrt(d0[..., 0] ** 2 + d0[..., 1] ** 2)
    hs = np.where(h0 == 0, f32(1), h0)
    sin = np.where(h0 > 0, d0[..., 1] / hs, f32(0)).astype(f32)
    cos = np.where(h0 > 0, d0[..., 0] / hs, f32(1)).astype(f32)

    jb = np.arange(NB, dtype=f32) + LOW[1]
    ybnd = (jb * vs[1]).astype(f32)
    t = ((ybnd[None, None, None, :] - ya[..., None])
         * invdy[..., None]).astype(f32)       # [B,N,4,65]
    u = np.clip(t, f32(0), f32(1))
    ub = (u * dx[..., None]).astype(f32)
    ub_p = (ub.reshape(2, 2, N, 4, NB)
            .transpose(1, 2, 3, 0, 4)
            .reshape(128, 2, NB))
    xa_p = _colpack128(xa)

    consts = []
    for m in range(NCORES):
        cc = np.zeros((128, CW3), f32)
        cc[:, OFF_UB:OFF_UB + NB] = ub_p[:, 0, :]
        cc[:, OFF_UB + NB:OFF_UB + 2 * NB] = ub_p[:, 1, :]
        ii = np.arange(NI + 1, dtype=f32) + (m * NI + LOW[0])
        x0i = (ii * vs[0]).astype(f32)
        for h in range(2):
            cc[:, OFF_XM3 + h * 9:OFF_XM3 + (h + 1) * 9] = \
                x0i[None, :] - xa_p[h][:, None]
        cc[:, OFF_HV3] = halfvol
        consts.append(np.ascontiguousarray(cc))

    w1p = _colpack128(w1s)
    rw = np.zeros((128, 2, 2, K, N), f32)
    for h in range(2):
        for p in range(128):
            b_lo, n = p // 64, (p % 64) // 4
            rw[p, h, b_lo, :, n] = zrho[2 * h + b_lo, :, n] * w1p[h, p]
    rw = rw.reshape(128, 512)

    w4 = np.zeros((128, B, K, 3), f32)
    for p in range(128):
        kq, n = p // N, p % N
        w4[p, :, kq, 0] = C_w4[:, n]
        w4[p, :, kq, 1] = sin[:, n]
        w4[p, :, kq, 2] = cos[:, n]
    w4 = w4.reshape(128, B * K * 3)

    ident = np.eye(128, dtype=f32)
    rww = np.ascontiguousarray(np.concatenate([rw, w4, ident], axis=1))
    return consts, rww


def _build_v3(mm='f32r', cfg=None):
    import concourse.bass as bass
    import concourse.tile as tile
    from concourse import bacc, mybir

    # gpsimd cannot touch PSUM, so stage 3 (reduce/eq/io read PSUM) is
    # DVE-only unless rho is first copied to SBUF (rc='A'), which lets Pool
    # run is_equal; Pool otherwise carries stage 2, Act the K relu+square
    # pairs and the PSUM->SBUF copies.
    cfg = cfg or {}
    S2 = cfg.get('s2') or [('V', 'V', 'V', 'V'), ('P', 'A2', 'P', 'P'),
                           ('P', 'A2', 'P', 'P'), ('P', 'A2', 'P', 'P')]
    RED = cfg.get('red', 'VVVV')
    EQ = cfg.get('eq', 'VVVV')
    IO = cfg.get('io', 'VVVV')
    CPY = cfg.get('cpy', 'AAAA')
    OQ = cfg.get('oq', 'SASS')
    RC = cfg.get('rc', '....')

    f32 = mybir.dt.float32
    f32r = mybir.dt.float32r
    wdt = f32r if mm == 'f32r' else f32
    ALU = mybir.AluOpType
    X = mybir.AxisListType.X

    nc = bacc.Bacc("TRN2", target_bir_lowering=False, debug=False,
                   num_devices=NCORES)
    d_consts = nc.dram_tensor("consts", [128, CW3], f32, kind="ExternalInput")
    d_rww = nc.dram_tensor("rww", [128, RWW_W], wdt, kind="ExternalInput")
    d_out = nc.dram_tensor("out", [B, NCELL * K, 2], f32, kind="ExternalOutput")

    with tile.TileContext(nc) as tc:
        with (
            tc.tile_pool(name="const", bufs=1) as cpool,
            tc.tile_pool(name="const2", bufs=1) as cpool2,
            tc.tile_pool(name="s2", bufs=4) as wpool,
            tc.tile_pool(name="s3", bufs=1) as tpool,
            tc.tile_pool(name="psr", bufs=1, space=bass.MemorySpace.PSUM) as prho,
            tc.tile_pool(name="psoh", bufs=2, space=bass.MemorySpace.PSUM) as poh,
            tc.tile_pool(name="pssel", bufs=1, space=bass.MemorySpace.PSUM) as psel,
        ):
            tco = cpool.tile([128, CW3], f32, tag="consts")
            nc.sync.dma_start(tco[:], d_consts[:])
            trww = cpool2.tile([128, RWW_W], wdt, tag="rww")
            nc.sync.dma_start(trww[:], d_rww[:])

            zz = cpool.tile([128, 8], f32, tag="zz")
            nc.vector.memset(zz[:], 0.0)

            trw = trww[:, 0:RW_W].rearrange("p (h q) -> p h q", q=256)
            ident = trww[:, ID_OFF:ID_OFF + 128].bitcast(f32)
            w4ap = trww[:, W4_OFF:W4_OFF + B * K * 3].bitcast(f32)
            hv_col = tco[:, OFF_HV3:OFF_HV3 + 1]

            rho_ps = []
            for cch in range(NCHUNK):
                rho_ps.append(prho.tile([128, 512], f32, tag=f"rho{cch}",
                                        name=f"rho{cch}"))

            def eng(e):
                return {'V': nc.vector, 'P': nc.gpsimd, 'A': nc.scalar}[e]

            nc.tensor.matmul(rho_ps[0][:8, 0:8], zz[:], zz[:],
                             start=True, stop=True)

            def stage2(half, h):
                ubh = tco[:, OFF_UB + h * NB:OFF_UB + (h + 1) * NB]
                xm5 = tco[:, OFF_XM3 + h * 9 + half * 4:
                          OFF_XM3 + h * 9 + half * 4 + 5]
                eG, eK, eD, eN = S2[half * 2 + h]
                G = wpool.tile([128, 5, NB], f32, tag="G", name="G")
                ub_b = ubh[:, None, :].broadcast_to([128, 5, NB])
                xm_b = xm5[:, :, None].broadcast_to([128, 5, NB])
                eng(eG).tensor_tensor(G[:], ub_b, xm_b, ALU.subtract)
                Ksq = wpool.tile([128, 5, NB], f32, tag="Ksq", name="Ksq")
                if eK == 'A2':
                    ACT = mybir.ActivationFunctionType
                    R = wpool.tile([128, 5, NB], f32, tag="R", name="R")
                    nc.scalar.activation(R[:].rearrange("p a b -> p (a b)"),
                                         G[:].rearrange("p a b -> p (a b)"),
                                         ACT.Relu)
                    nc.scalar.activation(Ksq[:].rearrange("p a b -> p (a b)"),
                                         R[:].rearrange("p a b -> p (a b)"),
                                         ACT.Square)
                else:
                    eng(eK).scalar_tensor_tensor(
                        Ksq[:].rearrange("p a b -> p (a b)"),
                        G[:].rearrange("p a b -> p (a b)"), zz[:, 0:1],
                        G[:].rearrange("p a b -> p (a b)"), ALU.max, ALU.mult)
                if mm == 'f32r':
                    # D' at the high boundaries (jb=1..64) and -D' at the low
                    # boundaries (jb=0..63), both contiguous [4, 64] so the
                    # matmul weight slices stay walrus-legal
                    Dp = wpool.tile([128, 4, NJ], wdt, tag="Dp", name="Dp")
                    eng(eD).tensor_tensor(Dp[:], Ksq[:, 0:4, 1:NB],
                                          Ksq[:, 1:5, 1:NB], ALU.subtract)
                    Dn = wpool.tile([128, 4, NJ], wdt, tag="Dn", name="Dn")
                    eng(eN).tensor_tensor(Dn[:], Ksq[:, 1:5, 0:NJ],
                                          Ksq[:, 0:4, 0:NJ], ALU.subtract)
                    return G, Ksq, Dp, Dn
                Dp = wpool.tile([128, 4, NB], f32, tag="Dp", name="Dp")
                eng(eD).tensor_tensor(Dp[:], Ksq[:, 0:4, :], Ksq[:, 1:5, :],
                                      ALU.subtract)
                ie = wpool.tile([128, 4, NJ], f32, tag="ie", name="ie")
                eng(eN).tensor_tensor(ie[:], Dp[:, :, 1:NB], Dp[:, :, 0:NJ],
                                      ALU.subtract)
                return G, Ksq, Dp, ie

            tiles = {}
            for half in range(2):
                for h in range(2):
                    tiles[(half, h)] = stage2(half, h)

            t00, t01 = tiles[(0, 0)], tiles[(0, 1)]
            t10, t11 = tiles[(1, 0)], tiles[(1, 1)]
            chain = [(t00[0][:, 0, 0:1], 0), (t01[0][:, 0, 0:1], 1),
                     (t00[1][:, 0, 0:1], 2), (t01[1][:, 0, 0:1], 3),
                     (t10[1][:, 0, 0:1], 0), (t00[1][:, 0, 1:2], 1),
                     (t11[1][:, 0, 0:1], 2), (t10[1][:, 0, 1:2], 3)]
            for ct, bank in chain:
                nc.tensor.matmul(rho_ps[bank][:1, 0:1], zz[:, 0:1], ct,
                                 start=True, stop=True)

            for half in range(2):
                for cc_l in range(2):
                    cch = half * 2 + cc_l
                    for h in range(2):
                        dst = rho_ps[cch][:, h * 256:(h + 1) * 256]
                        if mm == 'f32r':
                            _, _, Dp, Dn = tiles[(half, h)]
                            mva = Dp[:, cc_l * 2:cc_l * 2 + 2, :]
                            mvb = Dn[:, cc_l * 2:cc_l * 2 + 2, :]
                            nc.tensor.matmul(dst, mva, trw[:, h, :],
                                             start=True, stop=False)
                            nc.tensor.matmul(dst, mvb, trw[:, h, :],
                                             start=False, stop=True)
                        else:
                            ie = tiles[(half, h)][3]
                            nc.tensor.matmul(
                                dst,
                                ie[:, cc_l * 2:cc_l * 2 + 2, :]
                                .rearrange("p i j -> p (i j)"),
                                trw[:, h, :], start=True, stop=True)

            # ---- stage 3 ----
            maxr, oneh, ohss, sels = {}, {}, {}, {}

            rho_sb = {}

            def s3_max(cch):
                if RC[cch] == 'A':
                    rs = tpool.tile([128, 512], f32, tag=f"rhosb{cch}",
                                    name=f"rhosb{cch}")
                    nc.scalar.copy(rs[:], rho_ps[cch][:])
                    rho_sb[cch] = rs
                src = rho_sb.get(cch, rho_ps[cch])
                rho3 = src[:].rearrange("p (g n) -> p g n", n=N)
                maxrho = tpool.tile([128, B * K], f32, tag=f"maxrho{cch}",
                                    name=f"maxrho{cch}")
                eng(RED[cch]).reduce_max(maxrho[:], rho3, axis=X)
                maxr[cch] = maxrho

            def s3_eq(cch):
                src = rho_sb.get(cch, rho_ps[cch])
                rho3 = src[:].rearrange("p (g n) -> p g n", n=N)
                mx_b = maxr[cch][:][:, :, None].broadcast_to([128, B * K, N])
                onehot = tpool.tile([128, B * K * N], f32, tag=f"onehot{cch}",
                                    name=f"onehot{cch}")
                oh3 = onehot[:].rearrange("p (g n) -> p g n", n=N)
                eng(EQ[cch]).tensor_tensor(oh3, rho3, mx_b, ALU.is_equal)
                oneh[cch] = onehot

            def s3_tr(cch):
                oh_t = poh.tile([128, 4 * 128], f32, tag="oht", name="oht")
                for b in range(B):
                    nc.tensor.transpose(oh_t[:, b * 128:(b + 1) * 128],
                                        oneh[cch][:, b * 128:(b + 1) * 128],
                                        ident)
                ohs = tpool.tile([128, 4 * 128], f32, tag=f"ohs{cch}",
                                 name=f"ohs{cch}")
                if CPY[cch] == 'V':
                    nc.vector.tensor_scalar(ohs[:], oh_t[:], 0.0, None,
                                            ALU.add)
                else:
                    nc.scalar.copy(ohs[:], oh_t[:])
                ohss[cch] = ohs

            def s3_sel(cch):
                sel = psel.tile([128, B * K * 3], f32, tag=f"sel{cch % 2}",
                                name=f"sel{cch}")
                for b in range(B):
                    nc.tensor.matmul(
                        sel[:, b * K * 3:(b + 1) * K * 3],
                        ohss[cch][:, b * 128:(b + 1) * 128],
                        w4ap[:, b * K * 3:(b + 1) * K * 3],
                        start=True, stop=True)
                sels[cch] = sel

            def s3_out(cch):
                selg = sels[cch][:].rearrange("p (g q) -> p g q", q=3)
                inter2 = tpool.tile([128, B * K], f32, tag=f"inter2{cch}",
                                    name=f"inter2{cch}")
                e = eng(IO[cch])
                e.tensor_tensor(inter2[:], maxr[cch][:], selg[:, :, 0],
                                ALU.mult)
                outt = tpool.tile([128, B * K, 2], f32, tag=f"outt{cch}",
                                  name=f"outt{cch}")
                i2b = inter2[:][:, :, None].broadcast_to([128, B * K, 2])
                e.scalar_tensor_tensor(outt[:], i2b, hv_col,
                                       selg[:, :, 1:3], ALU.is_gt, ALU.mult)
                q = {'S': nc.sync, 'A': nc.scalar}[OQ[cch]]
                q.dma_start(
                    d_out[:, cch * 128 * K:(cch + 1) * 128 * K, :]
                    .rearrange("b (p k) e -> p b k e", k=K),
                    outt[:].rearrange("p (b k) e -> p b k e", k=K))

            for cch in range(NCHUNK):
                s3_max(cch)
                s3_eq(cch)
            for st in (s3_tr, s3_sel, s3_out):
                for cch in range(NCHUNK):
                    st(cch)

    nc.compile()
    return nc


# ---- v2/v1 fallback constants ----
OFF_LOHI = 0                    # [h][ep][j]  2*2*64 = 256
OFF_XM = 256                    # [h][i]      2*9 = 18
OFF_DX = 274                    # [h]         2
OFF_HV = 276                    # halfvol     1
CWP = 280

def _host_prep_v2(corners3d, neck_voxel_sizes):
    f32 = np.float32
    c = np.asarray(corners3d, f32)
    vs = np.asarray(neck_voxel_sizes, f32)[0]
    vox_vol = f32(vs[0]) * f32(vs[1]) * f32(vs[2])
    halfvol = f32(0.5) * vox_vol

    poly = c[:, :, :4, :2]                     # [B,N,4,2]
    nxt = np.roll(poly, -1, axis=2)
    xa, ya = poly[..., 0], poly[..., 1]        # [B,N,4]
    xb, yb = nxt[..., 0], nxt[..., 1]
    dx, dy = xb - xa, yb - ya
    vert = np.abs(dx) < f32(1e-4)
    if bool(vert.any()):
        return None                            # caller falls back to v1
    invdy = (f32(1) / dy).astype(f32)
    w1 = (dy / dx * f32(0.5)).astype(f32)      # [B,N,4]

    zb0 = c[:, :, :, 2].min(axis=2)
    zb1 = c[:, :, :, 2].max(axis=2)
    quad_area = f32(0.5) * np.abs((xa * yb - xb * ya).sum(axis=2, dtype=f32))
    box_vol = (quad_area * (zb1 - zb0)).astype(f32)
    C = (vox_vol + box_vol + f32(1e-9)).astype(f32)
    invC = (f32(1) / C).astype(f32)

    kk = np.arange(K, dtype=f32) + LOW[2]
    z0 = kk * vs[2]
    z1 = (kk + 1) * vs[2]
    zov = np.maximum(np.minimum(z1[None, :, None], zb1[:, None, :])
                     - np.maximum(z0[None, :, None], zb0[:, None, :]),
                     f32(0)).astype(f32)       # [B,K,N]
    eps = (f32(15) - np.arange(N, dtype=f32)) * f32(2.0 ** -20)
    zrho = (zov * invC[:, None, :] * (1 + eps)[None, None, :]).astype(f32)
    C_w4 = (C / (1 + eps)[None, :]).astype(f32)

    d0 = c[:, :, 0, :2] - c[:, :, 3, :2]
    h0 = np.sqrt(d0[..., 0] ** 2 + d0[..., 1] ** 2)
    hs = np.where(h0 == 0, f32(1), h0)
    sin = np.where(h0 > 0, d0[..., 1] / hs, f32(0)).astype(f32)
    cos = np.where(h0 > 0, d0[..., 0] / hs, f32(1)).astype(f32)

    # host j-prep: t-window [lo,hi] of y(t) in [y0_j, y0_j+vs] per (b,n,e,j)
    jj = np.arange(NJ, dtype=f32) + LOW[1]
    y0j = (jj * vs[1]).astype(f32)
    ty0 = ((y0j[None, None, None, :] - ya[..., None]) * invdy[..., None]).astype(f32)
    ty1 = (ty0 + (vs[1] * invdy)[..., None]).astype(f32)
    # note: reference computes ty1 from (y0+vs-ya)*invdy; keep that exact form
    ty1 = ((y0j[None, None, None, :] + vs[1] - ya[..., None])
           * invdy[..., None]).astype(f32)
    u0 = np.clip(ty0, f32(0), f32(1))
    u1 = np.clip(ty1, f32(0), f32(1))
    lo = np.minimum(u0, u1).astype(f32)        # [B,N,4,64]
    hi = np.maximum(u0, u1).astype(f32)

    def colpack(a):                            # [B,N,4] -> [2h][128p]
        return np.ascontiguousarray(a.reshape(2, 2, N, 4).reshape(2, 128))

    # consts per core; dx is folded into lo/hi on the host so the device
    # G op is a plain tensor_tensor (legal on GpSimd)
    lohi = np.stack([lo, hi], axis=3) * dx[..., None, None]   # [B,N,4,2,64]
    lohi_p = (lohi.reshape(2, 2, N, 4, 2, NJ)  # (h, b_lo, n, e, ep, j)
              .transpose(1, 2, 3, 0, 4, 5)     # (b_lo, n, e, h, ep, j)
              .reshape(128, 256))
    xa_p = colpack(xa)
    dx_p = colpack(dx)

    consts = []
    for m in range(NCORES):
        cc = np.zeros((128, CWP), f32)
        cc[:, OFF_LOHI:OFF_LOHI + 256] = lohi_p
        ii = np.arange(NI + 1, dtype=f32) + (m * NI + LOW[0])
        x0i = (ii * vs[0]).astype(f32)         # [9]
        for h in range(2):
            cc[:, OFF_XM + h * 9:OFF_XM + (h + 1) * 9] = \
                x0i[None, :] - xa_p[h][:, None]
            cc[:, OFF_DX + h] = dx_p[h]
        cc[:, OFF_HV] = halfvol
        consts.append(np.ascontiguousarray(cc))

    # rw [128, 2, 256]: zrho (w1 folded) at (b_lo, k, n) per partition
    w1p = colpack(w1)
    rw = np.zeros((128, 2, 2, K, N), f32)
    for h in range(2):
        for p in range(128):
            b_lo, n = p // 64, (p % 64) // 4
            rw[p, h, b_lo, :, n] = zrho[2 * h + b_lo, :, n] * w1p[h, p]
    rw = rw.reshape(128, 512)

    # w4 [128, 96]: selection weights, p = (k, n)
    w4 = np.zeros((128, B, K, 3), f32)
    for p in range(128):
        kq, n = p // N, p % N
        w4[p, :, kq, 0] = C_w4[:, n]
        w4[p, :, kq, 1] = sin[:, n]
        w4[p, :, kq, 2] = cos[:, n]
    w4 = w4.reshape(128, B * K * 3)

    ident = np.eye(128, dtype=f32)
    rww = np.ascontiguousarray(np.concatenate([rw, w4, ident], axis=1))
    return consts, rww


def _build_v2(chain_mode='own', s3_mode='grouped', assign=None):
    import concourse.bass as bass
    import concourse.tile as tile
    from concourse import bacc, mybir

    f32 = mybir.dt.float32
    ALU = mybir.AluOpType
    ACT = mybir.ActivationFunctionType
    X = mybir.AxisListType.X

    nc = bacc.Bacc("TRN2", target_bir_lowering=False, debug=False,
                   num_devices=NCORES)
    d_consts = nc.dram_tensor("consts", [128, CWP], f32, kind="ExternalInput")
    d_rww = nc.dram_tensor("rww", [128, RWW_W], f32, kind="ExternalInput")
    d_out = nc.dram_tensor("out", [B, NCELL * K, 2], f32, kind="ExternalOutput")

    with tile.TileContext(nc) as tc:
        with (
            tc.tile_pool(name="const", bufs=1) as cpool,
            tc.tile_pool(name="const2", bufs=1) as cpool2,
            tc.tile_pool(name="s2", bufs=4) as wpool,
            tc.tile_pool(name="s3", bufs=1) as tpool,
            tc.tile_pool(name="psr", bufs=1, space=bass.MemorySpace.PSUM) as prho,
            tc.tile_pool(name="psoh", bufs=2, space=bass.MemorySpace.PSUM) as poh,
            tc.tile_pool(name="pssel", bufs=1, space=bass.MemorySpace.PSUM) as psel,
        ):
            # --- input DMAs: consts on Pool queue, weights on SP queue ---
            tco = cpool.tile([128, CWP], f32, tag="consts")
            nc.sync.dma_start(tco[:], d_consts[:])
            trww = cpool2.tile([128, RWW_W], wdt, tag="rww")
            nc.sync.dma_start(trww[:], d_rww[:])

            # --- PE warm-up: pin the p-state ramp start near t=0 ---
            zz = cpool.tile([128, 8], f32, tag="zz")
            nc.vector.memset(zz[:], 0.0)

            trw = trww[:, 0:RW_W].rearrange("p (h q) -> p h q", q=256)
            ident = trww[:, ID_OFF:ID_OFF + 128]
            hv_col = tco[:, OFF_HV:OFF_HV + 1]

            rho_ps = []
            for cch in range(NCHUNK):
                rp = prho.tile([128, 2 * K * N * 2], f32, tag=f"rho{cch}",
                               name=f"rho{cch}")
                rho_ps.append(rp)
            # PE warm-up target: scratch corner of rho bank 0, overwritten
            # later by the real start=True matmul
            nc.tensor.matmul(rho_ps[0][:8, 0:8], zz[:], zz[:],
                             start=True, stop=True)

            # per-iter engine assignment: (G, relu, sq, D, ie)
            # engines: V=vector/DVE, P=gpsimd/Pool, A=scalar/Act
            # (G, relu, sq, D, ie); relu=None -> K fused as (G max 0) * G
            ASSIGN = assign or [
                ('V', 'A', 'A', 'V', 'P'),
                ('P', None, 'V', 'V', 'P'),
                ('V', 'A', 'A', 'V', 'P'),
                ('P', None, 'V', 'V', 'P'),
            ]

            def eng(c):
                return {'V': nc.vector, 'P': nc.gpsimd, 'A': nc.scalar}[c]

            def stage2(half, h):
                # G[p, ep, i, j] = dx*lohi[ep,j] - x0mxa[i], i in the half's
                # 5-wide telescoped window
                lohi = tco[:, OFF_LOHI + h * 128:OFF_LOHI + (h + 1) * 128] \
                    .rearrange("p (ep j) -> p ep j", j=NJ)
                xm = tco[:, OFF_XM + h * 9 + half * 4:
                         OFF_XM + h * 9 + half * 4 + 5]
                xm_e = xm[:, :, None].broadcast_to([128, 5, NJ])

                it = half * 2 + h
                eG, eR, eK, eD, eI = ASSIGN[it]
                G = wpool.tile([128, 2, 5, NJ], f32, tag="G", name="G")
                for ep in range(2):
                    lohi_e = lohi[:, ep, :][:, None, :].broadcast_to(
                        [128, 5, NJ])
                    eng(eG).tensor_tensor(G[:, ep], lohi_e, xm_e,
                                          ALU.subtract)
                Ksq = wpool.tile([128, 2, 5, NJ], f32, tag="Ksq", name="Ksq")
                Kf = Ksq[:].rearrange("p a b c -> p (a b c)")
                Gf = G[:].rearrange("p a b c -> p (a b c)")
                if eR is None:
                    # K = relu(G)^2 = max(G,0)*G in one STT op
                    eng(eK).scalar_tensor_tensor(Kf, Gf, zz[:, 0:1],
                                                 Gf, ALU.max, ALU.mult)
                else:
                    # per-endpoint relu+square pipelines the Act chain
                    # behind each G endpoint; relu='T' runs as a DVE
                    # tensor_scalar (2x_2p fast mode), halving Act's chain
                    R = wpool.tile([128, 2, 5, NJ], f32, tag="R", name="R")
                    for ep in range(2):
                        if eR == 'H' and ep == 0:
                            # endpoint 0 fused on DVE, endpoint 1 on Act
                            nc.vector.scalar_tensor_tensor(
                                Ksq[:, ep], G[:, ep], zz[:, 0:1], G[:, ep],
                                ALU.max, ALU.mult)
                            continue
                        if eR == 'T':
                            nc.vector.tensor_scalar(R[:, ep], G[:, ep],
                                                    0.0, None, ALU.max)
                        else:
                            nc.scalar.activation(R[:, ep], G[:, ep], ACT.Relu)
                        if eK == 'A':
                            nc.scalar.activation(Ksq[:, ep], R[:, ep],
                                                 ACT.Square)
                        else:
                            eng(eK).tensor_tensor(Ksq[:, ep], R[:, ep],
                                                  R[:, ep], ALU.mult)
                D = wpool.tile([128, 5, NJ], f32, tag="D", name="D")
                eng(eD).tensor_tensor(D[:], Ksq[:, 1], Ksq[:, 0],
                                      ALU.subtract)
                ie = wpool.tile([128, 4, NJ], f32, tag="ie", name="ie")
                eng(eI).tensor_tensor(ie[:], D[:, 0:4, :], D[:, 1:5, :],
                                      ALU.subtract)
                return G, Ksq, D, ie

            tiles = {}
            for half in range(2):
                for h in range(2):
                    tiles[(half, h)] = stage2(half, h)
            ies = {k: v[3] for k, v in tiles.items()}
            # p-state ramp chain: tiny matmuls dep'd on successively later
            # stage-2 tiles keep PE.SEQ occupied so the real matmuls are
            # dispatched >3us after PE's first barrier and run at full rate.
            # Writing into both rho banks (WAW) keeps the scheduler from
            # hoisting any real matmul ahead of the chain.
            K01 = tiles[(0, 1)][1][:, 0, 0, 0:1]
            D01 = tiles[(0, 1)][2][:, 0, 0:1]
            ie00 = tiles[(0, 0)][3]
            ie01 = tiles[(0, 1)][3]
            ie10 = tiles[(1, 0)][3]
            if chain_mode == 'own':
                chain = [(K01, 0), (D01, 1), (ie00[:, 0, 0:1], 2),
                         (ie01[:, 0, 0:1], 3), (ie00[:, 1, 0:1], 0),
                         (ie01[:, 1, 0:1], 1)]
            elif chain_mode == 'late':
                chain = [(K01, 0), (D01, 1), (ie01[:, 0, 0:1], 2),
                         (ie10[:, 0, 0:1], 3), (ie10[:, 1, 0:1], 0),
                         (ie10[:, 2, 0:1], 1), (ie10[:, 3, 0:1], 2),
                         (ie01[:, 1, 0:1], 3)]
            elif chain_mode == 'rev':
                K10 = tiles[(1, 0)][1][:, 0, 0, 0:1]
                K11 = tiles[(1, 1)][1][:, 0, 0, 0:1]
                ie11 = tiles[(1, 1)][3]
                chain = [(K10, 0), (K11, 1), (ie10[:, 0, 0:1], 2),
                         (ie11[:, 0, 0:1], 3), (ie00[:, 0, 0:1], 0),
                         (ie00[:, 1, 0:1], 1), (ie00[:, 2, 0:1], 2),
                         (ie00[:, 3, 0:1], 3)]
            elif chain_mode == 'mid':
                D11 = tiles[(1, 1)][2]
                chain = [(K01, 0), (D01, 1), (ie01[:, 0, 0:1], 2),
                         (D11[:, 0, 0:1], 3), (D11[:, 1, 0:1], 0),
                         (D11[:, 2, 0:1], 1), (D11[:, 3, 0:1], 2),
                         (ie01[:, 1, 0:1], 3)]
            else:
                chain = []
            for ct, bank in chain:
                nc.tensor.matmul(rho_ps[bank][:1, 0:1], zz[:, 0:1], ct,
                                 start=True, stop=True)
            HORD = (1, 0) if s3_mode == 'pairs-rev' else (0, 1)
            for half in HORD:
                for cc_l in range(2):
                    cch = half * 2 + cc_l
                    for h in range(2):
                        nc.tensor.matmul(
                            rho_ps[cch][:, h * 256:(h + 1) * 256],
                            ies[(half, h)][:, cc_l * 2:cc_l * 2 + 2, :]
                            .rearrange("p i j -> p (i j)"),
                            trw[:, h, :], start=True, stop=True)

            # ---- stage 3 ----
            maxr, oneh, ohss, sels = {}, {}, {}, {}

            def s3_max(cch):
                rho3 = rho_ps[cch][:].rearrange("p (g n) -> p g n", n=N)
                maxrho = tpool.tile([128, B * K], f32, tag=f"maxrho{cch}",
                                    name=f"maxrho{cch}")
                nc.vector.reduce_max(maxrho[:], rho3, axis=X)
                maxr[cch] = maxrho

            def s3_eq(cch):
                rho3 = rho_ps[cch][:].rearrange("p (g n) -> p g n", n=N)
                mx_b = maxr[cch][:][:, :, None].broadcast_to([128, B * K, N])
                onehot = tpool.tile([128, B * K * N], f32, tag=f"onehot{cch}",
                                    name=f"onehot{cch}")
                oh3 = onehot[:].rearrange("p (g n) -> p g n", n=N)
                nc.vector.tensor_tensor(oh3, rho3, mx_b, ALU.is_equal)
                oneh[cch] = onehot

            def s3_tr(cch):
                oh_t = poh.tile([128, 4 * 128], f32, tag="oht", name="oht")
                for b in range(B):
                    nc.tensor.transpose(oh_t[:, b * 128:(b + 1) * 128],
                                        oneh[cch][:, b * 128:(b + 1) * 128],
                                        ident)
                ohs = tpool.tile([128, 4 * 128], f32, tag=f"ohs{cch}",
                                 name=f"ohs{cch}")
                nc.scalar.copy(ohs[:], oh_t[:])
                ohss[cch] = ohs

            def s3_sel(cch):
                sel = psel.tile([128, B * K * 3], f32, tag=f"sel{cch % 2}",
                                name=f"sel{cch}")
                for b in range(B):
                    nc.tensor.matmul(
                        sel[:, b * K * 3:(b + 1) * K * 3],
                        ohss[cch][:, b * 128:(b + 1) * 128],
                        w4ap[:, b * K * 3:(b + 1) * K * 3],
                        start=True, stop=True)
                sels[cch] = sel

            def s3_out(cch):
                selg = sels[cch][:].rearrange("p (g q) -> p g q", q=3)
                inter2 = tpool.tile([128, B * K], f32, tag=f"inter2{cch}",
                                    name=f"inter2{cch}")
                nc.vector.tensor_tensor(inter2[:], maxr[cch][:],
                                        selg[:, :, 0], ALU.mult)
                # outt = (inter2 > halfvol) * sel_sincos, fused in one STT
                outt = tpool.tile([128, B * K, 2], f32, tag=f"outt{cch}",
                                  name=f"outt{cch}")
                i2b = inter2[:][:, :, None].broadcast_to([128, B * K, 2])
                nc.vector.scalar_tensor_tensor(outt[:], i2b, hv_col,
                                               selg[:, :, 1:3], ALU.is_gt,
                                               ALU.mult)
                nc.sync.dma_start(
                    d_out[:, cch * 128 * K:(cch + 1) * 128 * K, :]
                    .rearrange("b (p k) e -> p b k e", k=K),
                    outt[:].rearrange("p (b k) e -> p b k e", k=K))

            stages = [s3_max, s3_eq, s3_tr, s3_sel, s3_out]
            if s3_mode == 'grouped':
                for st in stages:
                    for cch in range(NCHUNK):
                        st(cch)
            elif s3_mode == 'pairs':
                for cch in range(NCHUNK):
                    s3_max(cch)
                    s3_eq(cch)
                for st in (s3_tr, s3_sel, s3_out):
                    for cch in range(NCHUNK):
                        st(cch)
            elif s3_mode == 'pairs-rev':
                for cch in (2, 3, 0, 1):
                    s3_max(cch)
                    s3_eq(cch)
                for st in (s3_tr, s3_sel, s3_out):
                    for cch in (2, 3, 0, 1):
                        st(cch)
            else:
                for cch in range(NCHUNK):
                    for st in stages:
                        st(cch)

    nc.compile()
    return nc



BIG = 1024.0


def _host_prep_v1(corners3d, neck_voxel_sizes):
    c = np.asarray(corners3d, np.float32)
    vs = np.asarray(neck_voxel_sizes, np.float32)[0]
    vox_vol = np.float32(vs[0]) * np.float32(vs[1]) * np.float32(vs[2])

    poly = c[:, :, :4, :2]                     # [B,N,4,2]
    nxt = np.roll(poly, -1, axis=2)
    xa, ya = poly[..., 0], poly[..., 1]        # [B,N,4]
    xb, yb = nxt[..., 0], nxt[..., 1]
    dx, dy = xb - xa, yb - ya
    with np.errstate(divide='ignore'):
        inv_dx = np.where(np.abs(dx) < 1e-12, np.float32(0),
                          np.float32(1) / np.where(dx == 0, np.float32(1), dx))
        inv_dy = np.where(np.abs(dy) < 1e-12, np.float32(0),
                          np.float32(1) / np.where(dy == 0, np.float32(1), dy))

    zb0 = c[:, :, :, 2].min(axis=2)
    zb1 = c[:, :, :, 2].max(axis=2)
    quad_area = 0.5 * np.abs((xa * yb - xb * ya).sum(axis=2))
    box_vol = quad_area * (zb1 - zb0)
    C = (vox_vol + box_vol + np.float32(1e-9)).astype(np.float32)   # [B,N]
    invC = (np.float32(1) / C).astype(np.float32)

    kk = np.arange(K, dtype=np.float32) + LOW[2]
    z0 = kk * vs[2]
    z1 = (kk + 1) * vs[2]
    zov = np.maximum(np.minimum(z1[None, :, None], zb1[:, None, :])
                     - np.maximum(z0[None, :, None], zb0[:, None, :]),
                     np.float32(0))                                  # [B,K,N]
    # tie-break epsilon: rho_n scaled by (1+eps_n), eps decreasing in n, so a
    # single reduce_max + is_equal yields the first-occurrence argmax; the C
    # selection weight is divided by (1+eps_n) to compensate exactly.
    eps = (np.float32(15) - np.arange(N, dtype=np.float32)) * np.float32(2.0 ** -20)
    zrho = (zov * invC[:, None, :] * (1 + eps)[None, None, :]).astype(np.float32)
    C_w4 = (C / (1 + eps)[None, :]).astype(np.float32)

    d = c[:, :, 0, :2] - c[:, :, 3, :2]
    h = np.sqrt(d[..., 0] ** 2 + d[..., 1] ** 2)
    hs = np.where(h == 0, np.float32(1), h)
    sin = np.where(h > 0, d[..., 1] / hs, np.float32(0)).astype(np.float32)
    cos = np.where(h > 0, d[..., 0] / hs, np.float32(1)).astype(np.float32)

    # --- stage-2 per-partition columns, p = b_lo*64 + n*4 + e, per h-iter ---
    def colpack(a):    # [B,N,4] -> [2h][128]
        return a.reshape(2, 2, N, 4).reshape(2, 128)
    DX_EPS = np.float32(1e-4)
    vert = np.abs(dx) < DX_EPS
    w1 = np.where(vert, np.float32(0), dy * inv_dx * np.float32(0.5))
    w2 = np.where(vert, dy, np.float32(0))
    cols = np.zeros((2, 128, 8), np.float32)
    cols[:, :, 0] = colpack(inv_dy)
    cols[:, :, 1] = colpack(-ya * inv_dy)
    cols[:, :, 2] = colpack((vs[1] - ya) * inv_dy)
    cols[:, :, 3] = colpack(dx)
    cols[:, :, 4] = colpack(xa)
    cols[:, :, 5] = colpack(w1)
    cols[:, :, 6] = colpack(w2)
    cols = np.ascontiguousarray(cols.transpose(1, 0, 2))       # [128,2,8]

    # edge-reduction weights with zrho folded in:
    # rw2[p=(b_lo,n,e), h, (b_lo',k,n')] = (b_lo'==b_lo & n'==n) * dy/2 * zrho[b,k,n]
    novert = not bool(vert.any())
    w1p = colpack(w1)                          # [2,128]
    rw = np.zeros((128, 2, 2, K, N), np.float32)
    for h in range(2):
        for p in range(128):
            b_lo, n = p // 64, (p % 64) // 4
            scale = w1p[h, p] if novert else np.float32(1)
            rw[p, h, b_lo, :, n] = zrho[2 * h + b_lo, :, n] * scale
    rw = np.ascontiguousarray(rw.reshape(128, 2, 2 * K * N))   # [128,2,256]

    # selection matmul weights: w4[b][(k,n), (k',q)] = (k==k') * w_q[b,n]
    # q in {C, sin, cos}
    w4 = np.zeros((128, B, K, 3), np.float32)
    for p in range(128):
        kq, n = p // N, p % N
        w4[p, :, kq, 0] = C_w4[:, n]
        w4[p, :, kq, 1] = sin[:, n]
        w4[p, :, kq, 2] = cos[:, n]
    w4 = np.ascontiguousarray(w4.reshape(128, B * K * 3))      # [128,96]

    # --- cells-major broadcast constants (partition-replicated by host) ---
    kbig = BIG - np.arange(N, dtype=np.float32)                # [16]
    jj = np.arange(NJ, dtype=np.float32) + LOW[1]
    y0 = (jj * vs[1]).astype(np.float32)                       # [64]
    halfvol = np.float32(0.5) * vox_vol

    consts = []
    for m in range(NCORES):
        ii = np.arange(NI, dtype=np.float32) + (m * NI + LOW[0])
        x0 = (ii * vs[0]).astype(np.float32)                   # [8]
        row = np.concatenate([kbig, y0, x0,
                              [halfvol, vs[0], -vs[0], 2 * vs[0]]]).astype(np.float32)
        cc = np.concatenate(
            [np.broadcast_to(row, (128, row.size)), cols.reshape(128, 16)],
            axis=1).astype(np.float32)
        consts.append(np.ascontiguousarray(cc))
    ident = np.eye(128, dtype=np.float32)
    return rw, w4, ident, consts, novert


def _build_v1(stages='all', fuse_w1=False):
    import concourse.bass as bass
    import concourse.tile as tile
    from concourse import bacc, mybir

    f32 = mybir.dt.float32
    ALU = mybir.AluOpType
    ACT = mybir.ActivationFunctionType

    CW = 16 + 64 + 8 + 4 + 16
    OFF_KBIG, OFF_Y0, OFF_X0, OFF_MISC, OFF_COLS = 0, 16, 80, 88, 92

    nc = bacc.Bacc("TRN2", target_bir_lowering=False, debug=False,
                   num_devices=NCORES)
    d_consts = nc.dram_tensor("consts", [128, CW], f32, kind="ExternalInput")
    d_rw = nc.dram_tensor("rw", [128, 2, 256], f32, kind="ExternalInput")
    d_w4 = nc.dram_tensor("w4", [128, B * K * 3], f32, kind="ExternalInput")
    d_ident = nc.dram_tensor("ident", [128, 128], f32, kind="ExternalInput")
    d_out = nc.dram_tensor("out", [B, NCELL * K, 2], f32, kind="ExternalOutput")

    with tile.TileContext(nc) as tc:
        with (
            tc.tile_pool(name="const", bufs=1) as cpool,
            tc.tile_pool(name="small", bufs=4) as spool,
            tc.tile_pool(name="work", bufs=6) as wpool,
            tc.tile_pool(name="edge", bufs=4) as epool,
            tc.tile_pool(name="st3", bufs=6) as tpool,
            tc.tile_pool(name="outp", bufs=4) as opool,
            tc.tile_pool(name="psum", bufs=1, space=bass.MemorySpace.PSUM) as ppool,
            tc.tile_pool(name="psum2", bufs=2, space=bass.MemorySpace.PSUM) as ppool2,
        ):
            tco = cpool.tile([128, CW], f32, tag="consts")
            nc.sync.dma_start(tco[:], d_consts[:])
            trw = cpool.tile([128, 2, 256], f32, tag="rw")
            nc.scalar.dma_start(trw[:], d_rw[:])
            tw4 = cpool.tile([128, B * K * 3], f32, tag="w4")
            nc.sync.dma_start(tw4[:], d_w4[:])
            ident = cpool.tile([128, 128], f32, tag="ident")
            nc.sync.dma_start(ident[:], d_ident[:])

            kbig_bc = tco[:, OFF_KBIG:OFF_KBIG + 16]
            y0_bc = tco[:, OFF_Y0:OFF_Y0 + 64]
            x0_bc = tco[:, OFF_X0:OFF_X0 + 8]
            halfvol_col = tco[:, OFF_MISC:OFF_MISC + 1]
            vs0_col = tco[:, OFF_MISC + 1:OFF_MISC + 2]
            nvs0_col = tco[:, OFF_MISC + 2:OFF_MISC + 3]
            vs0x2_col = tco[:, OFF_MISC + 3:OFF_MISC + 4]

            def bj(ap):   # [128,64] j-tile -> broadcast over i: [128,8,64]
                return ap[:, None, :].broadcast_to([128, NI, NJ])

            def bi(ap):   # [128,8] i-tile -> broadcast over j: [128,8,64]
                return ap[:, :, None].broadcast_to([128, NI, NJ])

            rho_ps = []
            for c in range(NCHUNK):
                rp = ppool.tile([128, B * K * N], f32, tag=f"rho{c}")
                rho_ps.append(rp)
            for h in range(2 if stages != 'none' else 0):
                col = lambda q: tco[:, OFF_COLS + h * 8 + q:OFF_COLS + h * 8 + q + 1]
                # j-only quantities [128, 64]
                ty0 = spool.tile([128, NJ], f32, tag="ty0")
                nc.scalar.activation(ty0[:], y0_bc, ACT.Identity,
                                     bias=col(1), scale=col(0))
                ty1 = spool.tile([128, NJ], f32, tag="ty1")
                nc.scalar.activation(ty1[:], y0_bc, ACT.Identity,
                                     bias=col(2), scale=col(0))
                u0 = spool.tile([128, NJ], f32, tag="u0")
                nc.vector.tensor_scalar(u0[:], ty0[:], 0.0, 1.0, ALU.max, ALU.min)
                u1 = spool.tile([128, NJ], f32, tag="u1")
                nc.vector.tensor_scalar(u1[:], ty1[:], 0.0, 1.0, ALU.max, ALU.min)
                lo = spool.tile([128, NJ], f32, tag="lo")
                nc.vector.tensor_tensor(lo[:], u0[:], u1[:], ALU.min)
                hi = spool.tile([128, NJ], f32, tag="hi")
                nc.vector.tensor_tensor(hi[:], u0[:], u1[:], ALU.max)
                hilo = spool.tile([128, NJ], f32, tag="hilo")
                nc.vector.tensor_tensor(hilo[:], hi[:], lo[:], ALU.subtract)
                # i-only quantities [128, 8]
                x0mxa = spool.tile([128, NI], f32, tag="x0mxa")
                nc.vector.tensor_single_scalar(x0mxa[:], x0_bc, col(4),
                                               ALU.subtract)
                # vertical-edge fallback: Fv = clamp(xa - x0, 0, vs0), * w2
                fvw = spool.tile([128, NI], f32, tag="fvw")
                nc.vector.tensor_scalar(fvw[:], x0mxa[:], -1.0, 0.0,
                                        ALU.mult, ALU.max)
                nc.vector.tensor_single_scalar(fvw[:], fvw[:], vs0_col, ALU.min)
                nc.vector.tensor_single_scalar(fvw[:], fvw[:], col(6), ALU.mult)

                NIH = NI // 2
                for half in range(2):
                    isl = slice(half * NIH, (half + 1) * NIH)

                    def full(tag):
                        t = wpool.tile([128, NIH, NJ], f32, tag=tag)
                        return t

                    def bjh(ap):
                        return ap[:, None, :].broadcast_to([128, NIH, NJ])

                    def bih(ap):
                        return ap[:, isl, None].broadcast_to([128, NIH, NJ])

                    # g at t=lo and t=hi:  g = dx*t - (x0 - xa)
                    glo = full("glo")
                    nc.vector.scalar_tensor_tensor(glo[:], bjh(lo[:]), col(3),
                                                   bih(x0mxa[:]), ALU.mult,
                                                   ALU.subtract)
                    ghi = full("ghi")
                    nc.vector.scalar_tensor_tensor(ghi[:], bjh(hi[:]), col(3),
                                                   bih(x0mxa[:]), ALU.mult,
                                                   ALU.subtract)
                    # H(u) = 0.5*clamp(u,0,c)^2 + c*relu(u-c); w1 carries 0.5
                    clo = full("clo")
                    nc.vector.tensor_scalar(clo[:], glo[:], 0.0, vs0_col,
                                            ALU.max, ALU.min)
                    chi = full("chi")
                    nc.gpsimd.tensor_scalar(chi[:], ghi[:], 0.0, vs0_col,
                                            ALU.max, ALU.min)
                    sqlo = full("sqlo")
                    nc.scalar.activation(sqlo[:], clo[:], ACT.Square)
                    sqhi = full("sqhi")
                    nc.scalar.activation(sqhi[:], chi[:], ACT.Square)
                    rlo = full("rlo")
                    nc.scalar.activation(rlo[:], glo[:], ACT.Relu,
                                         bias=nvs0_col)
                    rhi = full("rhi")
                    nc.scalar.activation(rhi[:], ghi[:], ACT.Relu,
                                         bias=nvs0_col)
                    e1 = full("e1")
                    nc.vector.tensor_tensor(e1[:], sqhi[:], sqlo[:],
                                            ALU.subtract)
                    e2 = full("e2")
                    nc.gpsimd.tensor_tensor(e2[:], rhi[:], rlo[:],
                                            ALU.subtract)
                    iedge = epool.tile([128, NCELL // 2], f32, tag="iedge")
                    if fuse_w1:
                        # w1 folded into rw: iedge = 2c*e2 + e1 directly
                        nc.vector.scalar_tensor_tensor(
                            iedge[:].rearrange("p (i j) -> p i j", j=NJ),
                            e2[:], vs0x2_col, e1[:], ALU.mult, ALU.add)
                    else:
                        s = full("s")
                        nc.vector.scalar_tensor_tensor(s[:], e2[:], vs0x2_col,
                                                       e1[:], ALU.mult, ALU.add)
                        t2w = full("t2w")
                        nc.gpsimd.tensor_tensor(t2w[:], bih(fvw[:]), bjh(hilo[:]),
                                                ALU.mult)
                        nc.vector.scalar_tensor_tensor(
                            iedge[:].rearrange("p (i j) -> p i j", j=NJ),
                            s[:], col(5), t2w[:], ALU.mult, ALU.add)

                    for cc in range(2):
                        cch = half * 2 + cc
                        nc.tensor.matmul(
                            rho_ps[cch][:, h * 256:(h + 1) * 256],
                            iedge[:, cc * 128:(cc + 1) * 128],
                            trw[:, h, :], start=True, stop=True)

            # ---- stage 3, cells-major, per 128-cell chunk ----
            for cch in range(NCHUNK if stages == 'all' else 0):
                rho3 = rho_ps[cch][:].rearrange("p (g n) -> p g n", n=N)
                maxrho = tpool.tile([128, B * K], f32, tag="maxrho")
                nc.vector.reduce_max(maxrho[:], rho3, axis=mybir.AxisListType.X)
                mx_bc = maxrho[:][:, :, None].broadcast_to([128, B * K, N])
                onehot = tpool.tile([128, B * K * N], f32, tag="onehot")
                nc.vector.tensor_tensor(
                    onehot[:].rearrange("p (g n) -> p g n", n=N), rho3, mx_bc,
                    ALU.is_equal)

                # selection sums via PE: transpose onehot per b, then matmul
                # against w4 -> SEL[cell, (k, {C,sin,cos})]
                oh_t = ppool2.tile([128, 4 * 128], f32, tag="oht")
                sel_ps = ppool2.tile([128, B * K * 3], f32, tag="selps")
                for b in range(B):
                    nc.tensor.transpose(
                        oh_t[:, b * 128:(b + 1) * 128],
                        onehot[:, b * 128:(b + 1) * 128], ident[:])
                ohs = tpool.tile([128, 4 * 128], f32, tag="ohs")
                nc.scalar.copy(ohs[:], oh_t[:])
                for b in range(B):
                    nc.tensor.matmul(
                        sel_ps[:, b * K * 3:(b + 1) * K * 3],
                        ohs[:, b * 128:(b + 1) * 128],
                        tw4[:, b * K * 3:(b + 1) * K * 3],
                        start=True, stop=True)

                sel3 = sel_ps[:].rearrange("p (b k q) -> p b k q", k=K, q=3)
                mx3 = maxrho[:].rearrange("p (b k) -> p b k", k=K)
                intersel = tpool.tile([128, B * K], f32, tag="intersel")
                nc.vector.tensor_tensor(
                    intersel[:].rearrange("p (b k) -> p b k", k=K),
                    mx3, sel3[:, :, :, 0], ALU.mult)
                mask = tpool.tile([128, B * K], f32, tag="mask")
                nc.vector.tensor_single_scalar(mask[:], intersel[:],
                                               halfvol_col, ALU.is_gt)
                outt = opool.tile([128, B, K, 2], f32, tag="outt")
                nc.vector.tensor_tensor(
                    outt[:, :, :, 0], sel3[:, :, :, 1],
                    mask[:].rearrange("p (b k) -> p b k", k=K), ALU.mult)
                nc.vector.tensor_tensor(
                    outt[:, :, :, 1], sel3[:, :, :, 2],
                    mask[:].rearrange("p (b k) -> p b k", k=K), ALU.mult)
                dma_eng = nc.sync
                dma_eng.dma_start(
                    d_out[:, cch * 128 * K:(cch + 1) * 128 * K, :]
                         .rearrange("b (p k) e -> p b k e", k=K),
                    outt[:])

    if stages != 'all':
        with tile.TileContext(nc) as tc2:
            with tc2.tile_pool(name="fin", bufs=1) as fpool:
                z = fpool.tile([128, 64], f32, tag="z")
                nc.gpsimd.memset(z[:], 0.0)
                nc.gpsimd.dma_start(
                    d_out[:, 0:1024, :].rearrange("b (p k) e -> p b k e", k=K),
                    z[:].rearrange("p (b k e) -> p b k e", k=K, e=2))
    nc.compile()
    return nc




_COMPILED = None


def kernel(corners3d, neck_voxel_sizes):
    global _COMPILED
    from concourse.bass_utils import run_bass_kernel_spmd

    prep3 = _host_prep_v3(corners3d, neck_voxel_sizes)
    if prep3 is not None:
        consts, rww = prep3
        if _COMPILED is None or _COMPILED[0] != 'v3':
            _COMPILED = ('v3', _build_v3(mm='f32'))
        nc = _COMPILED[1]
        in_maps = [{"consts": consts[m], "rww": rww} for m in range(NCORES)]
    else:
        prep = _host_prep_v2(corners3d, neck_voxel_sizes)
        if prep is not None:
            consts, rww = prep
            if _COMPILED is None or _COMPILED[0] != 'v2':
                _COMPILED = ('v2', _build_v2(chain_mode='late', s3_mode='pairs'))
            nc = _COMPILED[1]
            in_maps = [{"consts": consts[m], "rww": rww} for m in range(NCORES)]
        else:
            rw, w4, ident, consts1, novert = _host_prep_v1(corners3d,
                                                           neck_voxel_sizes)
            if _COMPILED is None or _COMPILED[0] != ('v1', novert):
                _COMPILED = (('v1', novert), _build_v1(fuse_w1=novert))
            nc = _COMPILED[1]
            in_maps = [{"consts": consts1[m], "rw": rw, "w4": w4,
                        "ident": ident} for m in range(NCORES)]
    res = run_bass_kernel_spmd(nc, in_maps, list(range(NCORES)))
    out = np.zeros((B, V, 2), np.float32)
    for m in range(NCORES):
        blk = res.results[m]["out"]
        out[:, m * NCELL * K:(m + 1) * NCELL * K, :] = blk
    return out.reshape(B * V, 2)


# revision 12
# speedup vs baseline: 1.1139x; 1.0229x over previous
"""Trainium2 Bass kernel v3 for Box3dEncoder (nn_Box3dEncoder_75453985456565).

v3 redesign vs v2:
  - j-telescoped stage 2: per edge the row-clip boundaries satisfy
    ty1(j) == ty0(j+1), so K(u)=relu(u)^2 is evaluated once per shared row
    boundary (65 of them) instead of per (lo,hi) pair: G/K volume halves.
    Direction sign s=sign(dy) is folded into the rw matmul weights.
  - fp32 matmuls at full p-state via a dep-staggered PE warm-up chain whose
    dummies only reference half-0 tiles, so the real matmuls dispatch as
    soon as their data lands. (float32r at 1 cyc/row was tried and is
    numerically too coarse on HW: argmax/mask flips -> rel err ~1.)
  - gpsimd cannot access PSUM, so stage 3 (reduce/is_equal/select-io) runs
    on DVE with PSUM->SBUF copies on Act; Pool carries most of stage 2 and
    Act the relu+square pairs.
"""
import numpy as np

B, N, K = 4, 16, 8
CUBE = (64, 64, 8)
LOW = (-32, -32, -4)
NCORES = 8
NI = CUBE[0] // NCORES          # 8 i-rows per core
NJ = CUBE[1]                    # 64
NB = NJ + 1                     # 65 row boundaries
NCELL = NI * NJ                 # 512 cells per core
NCHUNK = NCELL // 128           # 4
V = CUBE[0] * CUBE[1] * CUBE[2]

# v3 consts layout [128, CW3]: ub_h0 [65] | ub_h1 [65] | xm_h0 [9] | xm_h1 [9] | hv
OFF_UB = 0
OFF_XM3 = 130
OFF_HV3 = 148
CW3 = 149

RW_W = 512
W4_OFF = 512
ID_OFF = 608
RWW_W = 736


def _colpack128(a):              # [B,N,4] -> [2h][128p], p = b_lo*64+n*4+e
    return np.ascontiguousarray(a.reshape(2, 2, N, 4).reshape(2, 128))


def _host_prep_v3(corners3d, neck_voxel_sizes):
    f32 = np.float32
    c = np.asarray(corners3d, f32)
    vs = np.asarray(neck_voxel_sizes, f32)[0]
    vox_vol = f32(vs[0]) * f32(vs[1]) * f32(vs[2])
    halfvol = f32(0.5) * vox_vol

    poly = c[:, :, :4, :2]
    nxt = np.roll(poly, -1, axis=2)
    xa, ya = poly[..., 0], poly[..., 1]
    xb, yb = nxt[..., 0], nxt[..., 1]
    dx, dy = xb - xa, yb - ya
    if bool((np.abs(dx) < 1e-4).any()) or bool((np.abs(dy) < 1e-6).any()):
        return None
    invdy = (f32(1) / dy).astype(f32)
    s = np.sign(dy).astype(f32)
    w1s = (dy / dx * f32(0.5) * s).astype(f32)

    zb0 = c[:, :, :, 2].min(axis=2)
    zb1 = c[:, :, :, 2].max(axis=2)
    quad_area = f32(0.5) * np.abs((xa * yb - xb * ya).sum(axis=2, dtype=f32))
    box_vol = (quad_area * (zb1 - zb0)).astype(f32)
    C = (vox_vol + box_vol + f32(1e-9)).astype(f32)
    invC = (f32(1) / C).astype(f32)

    kk = np.arange(K, dtype=f32) + LOW[2]
    z0 = kk * vs[2]
    z1 = (kk + 1) * vs[2]
    zov = np.maximum(np.minimum(z1[None, :, None], zb1[:, None, :])
                     - np.maximum(z0[None, :, None], zb0[:, None, :]),
                     f32(0)).astype(f32)       # [B,K,N]
    # first-occurrence-argmax tie-break epsilons (see v2)
    eps = (f32(15) - np.arange(N, dtype=f32)) * f32(2.0 ** -20)
    zrho = (zov * invC[:, None, :] * (1 + eps)[None, None, :]).astype(f32)
    C_w4 = (C / (1 + eps)[None, :]).astype(f32)

    d0 = c[:, :, 0, :2] - c[:, :, 3, :2]
    h0 = np.sqrt(d0[..., 0] ** 2 + d0[..., 1] ** 2)
    hs = np.where(h0 == 0, f32(1), h0)
    sin = np.where(h0 > 0, d0[..., 1] / hs, f32(0)).astype(f32)
    cos = np.where(h0 > 0, d0[..., 0] / hs, f32(1)).astype(f32)

    jb = np.arange(NB, dtype=f32) + LOW[1]
    ybnd = (jb * vs[1]).astype(f32)
    t = ((ybnd[None, None, None, :] - ya[..., None])
         * invdy[..., None]).astype(f32)       # [B,N,4,65]
    u = np.clip(t, f32(0), f32(1))
    ub = (u * dx[..., None]).astype(f32)
    ub_p = (ub.reshape(2, 2, N, 4, NB)
            .transpose(1, 2, 3, 0, 4)
            .reshape(128, 2, NB))
    xa_p = _colpack128(xa)

    consts = []
    for m in range(NCORES):
        cc = np.zeros((128, CW3), f32)
        cc[:, OFF_UB:OFF_UB + NB] = ub_p[:, 0, :]
        cc[:, OFF_UB + NB:OFF_UB + 2 * NB] = ub_p[:, 1, :]
        ii = np.arange(NI + 1, dtype=f32) + (m * NI + LOW[0])
        x0i = (ii * vs[0]).astype(f32)
        for h in range(2):
            cc[:, OFF_XM3 + h * 9:OFF_XM3 + (h + 1) * 9] = \
                x0i[None, :] - xa_p[h][:, None]
        cc[:, OFF_HV3] = halfvol
        consts.append(np.ascontiguousarray(cc))

    w1p = _colpack128(w1s)
    rw = np.zeros((128, 2, 2, K, N), f32)
    for h in range(2):
        for p in range(128):
            b_lo, n = p // 64, (p % 64) // 4
            rw[p, h, b_lo, :, n] = zrho[2 * h + b_lo, :, n] * w1p[h, p]
    rw = rw.reshape(128, 512)

    w4 = np.zeros((128, B, K, 3), f32)
    for p in range(128):
        kq, n = p // N, p % N
        w4[p, :, kq, 0] = C_w4[:, n]
        w4[p, :, kq, 1] = sin[:, n]
        w4[p, :, kq, 2] = cos[:, n]
    w4 = w4.reshape(128, B * K * 3)

    ident = np.eye(128, dtype=f32)
    rww = np.ascontiguousarray(np.concatenate([rw, w4, ident], axis=1))
    return consts, rww


def _build_v3(mm='f32r', cfg=None):
    import concourse.bass as bass
    import concourse.tile as tile
    from concourse import bacc, mybir

    # gpsimd cannot touch PSUM, so stage 3 (reduce/eq/io read PSUM) is
    # DVE-only unless rho is first copied to SBUF (rc='A'), which lets Pool
    # run is_equal; Pool otherwise carries stage 2, Act the K relu+square
    # pairs and the PSUM->SBUF copies.
    cfg = cfg or {}
    S2 = cfg.get('s2') or [('V', 'V', 'V', 'V'), ('P', 'A2', 'P', 'P'),
                           ('P', 'A2', 'P', 'P'), ('P', 'A2', 'P', 'P')]
    RED = cfg.get('red', 'VVVV')
    EQ = cfg.get('eq', 'VVVV')
    IO = cfg.get('io', 'VVVV')
    CPY = cfg.get('cpy', 'AAAA')
    OQ = cfg.get('oq', 'SASS')
    RC = cfg.get('rc', '....')

    f32 = mybir.dt.float32
    f32r = mybir.dt.float32r
    wdt = f32r if mm == 'f32r' else f32
    ALU = mybir.AluOpType
    X = mybir.AxisListType.X

    nc = bacc.Bacc("TRN2", target_bir_lowering=False, debug=False,
                   num_devices=NCORES)
    d_consts = nc.dram_tensor("consts", [128, CW3], f32, kind="ExternalInput")
    d_rww = nc.dram_tensor("rww", [128, RWW_W], wdt, kind="ExternalInput")
    d_out = nc.dram_tensor("out", [B, NCELL * K, 2], f32, kind="ExternalOutput")

    with tile.TileContext(nc) as tc:
        with (
            tc.tile_pool(name="const", bufs=1) as cpool,
            tc.tile_pool(name="const2", bufs=1) as cpool2,
            tc.tile_pool(name="s2", bufs=4) as wpool,
            tc.tile_pool(name="s3", bufs=1) as tpool,
            tc.tile_pool(name="psr", bufs=1, space=bass.MemorySpace.PSUM) as prho,
            tc.tile_pool(name="psoh", bufs=2, space=bass.MemorySpace.PSUM) as poh,
            tc.tile_pool(name="pssel", bufs=1, space=bass.MemorySpace.PSUM) as psel,
        ):
            tco = cpool.tile([128, CW3], f32, tag="consts")
            nc.sync.dma_start(tco[:], d_consts[:])
            trww = cpool2.tile([128, RWW_W], wdt, tag="rww")
            nc.sync.dma_start(trww[:], d_rww[:])

            zz = cpool.tile([128, 8], f32, tag="zz")
            nc.vector.memset(zz[:], 0.0)

            trw = trww[:, 0:RW_W].rearrange("p (h q) -> p h q", q=256)
            ident = trww[:, ID_OFF:ID_OFF + 128].bitcast(f32)
            w4ap = trww[:, W4_OFF:W4_OFF + B * K * 3].bitcast(f32)
            hv_col = tco[:, OFF_HV3:OFF_HV3 + 1]

            rho_ps = []
            for cch in range(NCHUNK):
                rho_ps.append(prho.tile([128, 512], f32, tag=f"rho{cch}",
                                        name=f"rho{cch}"))

            def eng(e):
                return {'V': nc.vector, 'P': nc.gpsimd, 'A': nc.scalar}[e]

            nc.tensor.matmul(rho_ps[0][:8, 0:8], zz[:], zz[:],
                             start=True, stop=True)

            def stage2(half, h):
                ubh = tco[:, OFF_UB + h * NB:OFF_UB + (h + 1) * NB]
                xm5 = tco[:, OFF_XM3 + h * 9 + half * 4:
                          OFF_XM3 + h * 9 + half * 4 + 5]
                eG, eK, eD, eN = S2[half * 2 + h]
                G = wpool.tile([128, 5, NB], f32, tag="G", name="G")
                ub_b = ubh[:, None, :].broadcast_to([128, 5, NB])
                xm_b = xm5[:, :, None].broadcast_to([128, 5, NB])
                eng(eG).tensor_tensor(G[:], ub_b, xm_b, ALU.subtract)
                Ksq = wpool.tile([128, 5, NB], f32, tag="Ksq", name="Ksq")
                if eK == 'A2':
                    ACT = mybir.ActivationFunctionType
                    R = wpool.tile([128, 5, NB], f32, tag="R", name="R")
                    nc.scalar.activation(R[:].rearrange("p a b -> p (a b)"),
                                         G[:].rearrange("p a b -> p (a b)"),
                                         ACT.Relu)
                    nc.scalar.activation(Ksq[:].rearrange("p a b -> p (a b)"),
                                         R[:].rearrange("p a b -> p (a b)"),
                                         ACT.Square)
                else:
                    eng(eK).scalar_tensor_tensor(
                        Ksq[:].rearrange("p a b -> p (a b)"),
                        G[:].rearrange("p a b -> p (a b)"), zz[:, 0:1],
                        G[:].rearrange("p a b -> p (a b)"), ALU.max, ALU.mult)
                if mm == 'f32r':
                    # D' at the high boundaries (jb=1..64) and -D' at the low
                    # boundaries (jb=0..63), both contiguous [4, 64] so the
                    # matmul weight slices stay walrus-legal
                    Dp = wpool.tile([128, 4, NJ], wdt, tag="Dp", name="Dp")
                    eng(eD).tensor_tensor(Dp[:], Ksq[:, 0:4, 1:NB],
                                          Ksq[:, 1:5, 1:NB], ALU.subtract)
                    Dn = wpool.tile([128, 4, NJ], wdt, tag="Dn", name="Dn")
                    eng(eN).tensor_tensor(Dn[:], Ksq[:, 1:5, 0:NJ],
                                          Ksq[:, 0:4, 0:NJ], ALU.subtract)
                    return G, Ksq, Dp, Dn
                Dp = wpool.tile([128, 4, NB], f32, tag="Dp", name="Dp")
                eng(eD).tensor_tensor(Dp[:], Ksq[:, 0:4, :], Ksq[:, 1:5, :],
                                      ALU.subtract)
                ie = wpool.tile([128, 4, NJ], f32, tag="ie", name="ie")
                eng(eN).tensor_tensor(ie[:], Dp[:, :, 1:NB], Dp[:, :, 0:NJ],
                                      ALU.subtract)
                return G, Ksq, Dp, ie

            tiles = {}
            for half in range(2):
                for h in range(2):
                    tiles[(half, h)] = stage2(half, h)

            t00, t01 = tiles[(0, 0)], tiles[(0, 1)]
            # dummies dep'd only on half-0 tiles (ready 3.5-6.1us) so the
            # real matmuls dispatch as soon as their data exists, at full
            # p-state
            chain = [(t00[0][:, 0, 0:1], 0), (t01[0][:, 0, 0:1], 1),
                     (t00[1][:, 0, 0:1], 2), (t00[2][:, 0, 0:1], 3),
                     (t00[3][:, 0, 0:1], 0), (t01[1][:, 0, 0:1], 1),
                     (t01[2][:, 0, 0:1], 2), (t01[3][:, 0, 0:1], 3)]
            for ct, bank in chain:
                nc.tensor.matmul(rho_ps[bank][:1, 0:1], zz[:, 0:1], ct,
                                 start=True, stop=True)

            for half in range(2):
                for cc_l in range(2):
                    cch = half * 2 + cc_l
                    for h in range(2):
                        dst = rho_ps[cch][:, h * 256:(h + 1) * 256]
                        if mm == 'f32r':
                            _, _, Dp, Dn = tiles[(half, h)]
                            mva = Dp[:, cc_l * 2:cc_l * 2 + 2, :]
                            mvb = Dn[:, cc_l * 2:cc_l * 2 + 2, :]
                            nc.tensor.matmul(dst, mva, trw[:, h, :],
                                             start=True, stop=False)
                            nc.tensor.matmul(dst, mvb, trw[:, h, :],
                                             start=False, stop=True)
                        else:
                            ie = tiles[(half, h)][3]
                            nc.tensor.matmul(
                                dst,
                                ie[:, cc_l * 2:cc_l * 2 + 2, :]
                                .rearrange("p i j -> p (i j)"),
                                trw[:, h, :], start=True, stop=True)

            # ---- stage 3 ----
            maxr, oneh, ohss, sels = {}, {}, {}, {}

            rho_sb = {}

            def s3_max(cch):
                if RC[cch] == 'A':
                    rs = tpool.tile([128, 512], f32, tag=f"rhosb{cch}",
                                    name=f"rhosb{cch}")
                    nc.scalar.copy(rs[:], rho_ps[cch][:])
                    rho_sb[cch] = rs
                src = rho_sb.get(cch, rho_ps[cch])
                rho3 = src[:].rearrange("p (g n) -> p g n", n=N)
                maxrho = tpool.tile([128, B * K], f32, tag=f"maxrho{cch}",
                                    name=f"maxrho{cch}")
                eng(RED[cch]).reduce_max(maxrho[:], rho3, axis=X)
                maxr[cch] = maxrho

            def s3_eq(cch):
                src = rho_sb.get(cch, rho_ps[cch])
                rho3 = src[:].rearrange("p (g n) -> p g n", n=N)
                mx_b = maxr[cch][:][:, :, None].broadcast_to([128, B * K, N])
                onehot = tpool.tile([128, B * K * N], f32, tag=f"onehot{cch}",
                                    name=f"onehot{cch}")
                oh3 = onehot[:].rearrange("p (g n) -> p g n", n=N)
                eng(EQ[cch]).tensor_tensor(oh3, rho3, mx_b, ALU.is_equal)
                oneh[cch] = onehot

            def s3_tr(cch):
                oh_t = poh.tile([128, 4 * 128], f32, tag="oht", name="oht")
                for b in range(B):
                    nc.tensor.transpose(oh_t[:, b * 128:(b + 1) * 128],
                                        oneh[cch][:, b * 128:(b + 1) * 128],
                                        ident)
                ohs = tpool.tile([128, 4 * 128], f32, tag=f"ohs{cch}",
                                 name=f"ohs{cch}")
                if CPY[cch] == 'V':
                    nc.vector.tensor_scalar(ohs[:], oh_t[:], 0.0, None,
                                            ALU.add)
                else:
                    nc.scalar.copy(ohs[:], oh_t[:])
                ohss[cch] = ohs

            def s3_sel(cch):
                sel = psel.tile([128, B * K * 3], f32, tag=f"sel{cch % 2}",
                                name=f"sel{cch}")
                for b in range(B):
                    nc.tensor.matmul(
                        sel[:, b * K * 3:(b + 1) * K * 3],
                        ohss[cch][:, b * 128:(b + 1) * 128],
                        w4ap[:, b * K * 3:(b + 1) * K * 3],
                        start=True, stop=True)
                sels[cch] = sel

            def s3_out(cch):
                selg = sels[cch][:].rearrange("p (g q) -> p g q", q=3)
                inter2 = tpool.tile([128, B * K], f32, tag=f"inter2{cch}",
                                    name=f"inter2{cch}")
                e = eng(IO[cch])
                e.tensor_tensor(inter2[:], maxr[cch][:], selg[:, :, 0],
                                ALU.mult)
                outt = tpool.tile([128, B * K, 2], f32, tag=f"outt{cch}",
                                  name=f"outt{cch}")
                i2b = inter2[:][:, :, None].broadcast_to([128, B * K, 2])
                e.scalar_tensor_tensor(outt[:], i2b, hv_col,
                                       selg[:, :, 1:3], ALU.is_gt, ALU.mult)
                q = {'S': nc.sync, 'A': nc.scalar}[OQ[cch]]
                q.dma_start(
                    d_out[:, cch * 128 * K:(cch + 1) * 128 * K, :]
                    .rearrange("b (p k) e -> p b k e", k=K),
                    outt[:].rearrange("p (b k) e -> p b k e", k=K))

            for cch in range(NCHUNK):
                s3_max(cch)
                s3_eq(cch)
            for st in (s3_tr, s3_sel, s3_out):
                for cch in range(NCHUNK):
                    st(cch)

    nc.compile()
    return nc


# ---- v2/v1 fallback constants ----
OFF_LOHI = 0                    # [h][ep][j]  2*2*64 = 256
OFF_XM = 256                    # [h][i]      2*9 = 18
OFF_DX = 274                    # [h]         2
OFF_HV = 276                    # halfvol     1
CWP = 280

def _host_prep_v2(corners3d, neck_voxel_sizes):
    f32 = np.float32
    c = np.asarray(corners3d, f32)
    vs = np.asarray(neck_voxel_sizes, f32)[0]
    vox_vol = f32(vs[0]) * f32(vs[1]) * f32(vs[2])
    halfvol = f32(0.5) * vox_vol

    poly = c[:, :, :4, :2]                     # [B,N,4,2]
    nxt = np.roll(poly, -1, axis=2)
    xa, ya = poly[..., 0], poly[..., 1]        # [B,N,4]
    xb, yb = nxt[..., 0], nxt[..., 1]
    dx, dy = xb - xa, yb - ya
    vert = np.abs(dx) < f32(1e-4)
    if bool(vert.any()):
        return None                            # caller falls back to v1
    invdy = (f32(1) / dy).astype(f32)
    w1 = (dy / dx * f32(0.5)).astype(f32)      # [B,N,4]

    zb0 = c[:, :, :, 2].min(axis=2)
    zb1 = c[:, :, :, 2].max(axis=2)
    quad_area = f32(0.5) * np.abs((xa * yb - xb * ya).sum(axis=2, dtype=f32))
    box_vol = (quad_area * (zb1 - zb0)).astype(f32)
    C = (vox_vol + box_vol + f32(1e-9)).astype(f32)
    invC = (f32(1) / C).astype(f32)

    kk = np.arange(K, dtype=f32) + LOW[2]
    z0 = kk * vs[2]
    z1 = (kk + 1) * vs[2]
    zov = np.maximum(np.minimum(z1[None, :, None], zb1[:, None, :])
                     - np.maximum(z0[None, :, None], zb0[:, None, :]),
                     f32(0)).astype(f32)       # [B,K,N]
    eps = (f32(15) - np.arange(N, dtype=f32)) * f32(2.0 ** -20)
    zrho = (zov * invC[:, None, :] * (1 + eps)[None, None, :]).astype(f32)
    C_w4 = (C / (1 + eps)[None, :]).astype(f32)

    d0 = c[:, :, 0, :2] - c[:, :, 3, :2]
    h0 = np.sqrt(d0[..., 0] ** 2 + d0[..., 1] ** 2)
    hs = np.where(h0 == 0, f32(1), h0)
    sin = np.where(h0 > 0, d0[..., 1] / hs, f32(0)).astype(f32)
    cos = np.where(h0 > 0, d0[..., 0] / hs, f32(1)).astype(f32)

    # host j-prep: t-window [lo,hi] of y(t) in [y0_j, y0_j+vs] per (b,n,e,j)
    jj = np.arange(NJ, dtype=f32) + LOW[1]
    y0j = (jj * vs[1]).astype(f32)
    ty0 = ((y0j[None, None, None, :] - ya[..., None]) * invdy[..., None]).astype(f32)
    ty1 = (ty0 + (vs[1] * invdy)[..., None]).astype(f32)
    # note: reference computes ty1 from (y0+vs-ya)*invdy; keep that exact form
    ty1 = ((y0j[None, None, None, :] + vs[1] - ya[..., None])
           * invdy[..., None]).astype(f32)
    u0 = np.clip(ty0, f32(0), f32(1))
    u1 = np.clip(ty1, f32(0), f32(1))
    lo = np.minimum(u0, u1).astype(f32)        # [B,N,4,64]
    hi = np.maximum(u0, u1).astype(f32)

    def colpack(a):                            # [B,N,4] -> [2h][128p]
        return np.ascontiguousarray(a.reshape(2, 2, N, 4).reshape(2, 128))

    # consts per core; dx is folded into lo/hi on the host so the device
    # G op is a plain tensor_tensor (legal on GpSimd)
    lohi = np.stack([lo, hi], axis=3) * dx[..., None, None]   # [B,N,4,2,64]
    lohi_p = (lohi.reshape(2, 2, N, 4, 2, NJ)  # (h, b_lo, n, e, ep, j)
              .transpose(1, 2, 3, 0, 4, 5)     # (b_lo, n, e, h, ep, j)
              .reshape(128, 256))
    xa_p = colpack(xa)
    dx_p = colpack(dx)

    consts = []
    for m in range(NCORES):
        cc = np.zeros((128, CWP), f32)
        cc[:, OFF_LOHI:OFF_LOHI + 256] = lohi_p
        ii = np.arange(NI + 1, dtype=f32) + (m * NI + LOW[0])
        x0i = (ii * vs[0]).astype(f32)         # [9]
        for h in range(2):
            cc[:, OFF_XM + h * 9:OFF_XM + (h + 1) * 9] = \
                x0i[None, :] - xa_p[h][:, None]
            cc[:, OFF_DX + h] = dx_p[h]
        cc[:, OFF_HV] = halfvol
        consts.append(np.ascontiguousarray(cc))

    # rw [128, 2, 256]: zrho (w1 folded) at (b_lo, k, n) per partition
    w1p = colpack(w1)
    rw = np.zeros((128, 2, 2, K, N), f32)
    for h in range(2):
        for p in range(128):
            b_lo, n = p // 64, (p % 64) // 4
            rw[p, h, b_lo, :, n] = zrho[2 * h + b_lo, :, n] * w1p[h, p]
    rw = rw.reshape(128, 512)

    # w4 [128, 96]: selection weights, p = (k, n)
    w4 = np.zeros((128, B, K, 3), f32)
    for p in range(128):
        kq, n = p // N, p % N
        w4[p, :, kq, 0] = C_w4[:, n]
        w4[p, :, kq, 1] = sin[:, n]
        w4[p, :, kq, 2] = cos[:, n]
    w4 = w4.reshape(128, B * K * 3)

    ident = np.eye(128, dtype=f32)
    rww = np.ascontiguousarray(np.concatenate([rw, w4, ident], axis=1))
    return consts, rww


def _build_v2(chain_mode='own', s3_mode='grouped', assign=None):
    import concourse.bass as bass
    import concourse.tile as tile
    from concourse import bacc, mybir

    f32 = mybir.dt.float32
    ALU = mybir.AluOpType
    ACT = mybir.ActivationFunctionType
    X = mybir.AxisListType.X

    nc = bacc.Bacc("TRN2", target_bir_lowering=False, debug=False,
                   num_devices=NCORES)
    d_consts = nc.dram_tensor("consts", [128, CWP], f32, kind="ExternalInput")
    d_rww = nc.dram_tensor("rww", [128, RWW_W], f32, kind="ExternalInput")
    d_out = nc.dram_tensor("out", [B, NCELL * K, 2], f32, kind="ExternalOutput")

    with tile.TileContext(nc) as tc:
        with (
            tc.tile_pool(name="const", bufs=1) as cpool,
            tc.tile_pool(name="const2", bufs=1) as cpool2,
            tc.tile_pool(name="s2", bufs=4) as wpool,
            tc.tile_pool(name="s3", bufs=1) as tpool,
            tc.tile_pool(name="psr", bufs=1, space=bass.MemorySpace.PSUM) as prho,
            tc.tile_pool(name="psoh", bufs=2, space=bass.MemorySpace.PSUM) as poh,
            tc.tile_pool(name="pssel", bufs=1, space=bass.MemorySpace.PSUM) as psel,
        ):
            # --- input DMAs: consts on Pool queue, weights on SP queue ---
            tco = cpool.tile([128, CWP], f32, tag="consts")
            nc.sync.dma_start(tco[:], d_consts[:])
            trww = cpool2.tile([128, RWW_W], wdt, tag="rww")
            nc.sync.dma_start(trww[:], d_rww[:])

            # --- PE warm-up: pin the p-state ramp start near t=0 ---
            zz = cpool.tile([128, 8], f32, tag="zz")
            nc.vector.memset(zz[:], 0.0)

            trw = trww[:, 0:RW_W].rearrange("p (h q) -> p h q", q=256)
            ident = trww[:, ID_OFF:ID_OFF + 128]
            hv_col = tco[:, OFF_HV:OFF_HV + 1]

            rho_ps = []
            for cch in range(NCHUNK):
                rp = prho.tile([128, 2 * K * N * 2], f32, tag=f"rho{cch}",
                               name=f"rho{cch}")
                rho_ps.append(rp)
            # PE warm-up target: scratch corner of rho bank 0, overwritten
            # later by the real start=True matmul
            nc.tensor.matmul(rho_ps[0][:8, 0:8], zz[:], zz[:],
                             start=True, stop=True)

            # per-iter engine assignment: (G, relu, sq, D, ie)
            # engines: V=vector/DVE, P=gpsimd/Pool, A=scalar/Act
            # (G, relu, sq, D, ie); relu=None -> K fused as (G max 0) * G
            ASSIGN = assign or [
                ('V', 'A', 'A', 'V', 'P'),
                ('P', None, 'V', 'V', 'P'),
                ('V', 'A', 'A', 'V', 'P'),
                ('P', None, 'V', 'V', 'P'),
            ]

            def eng(c):
                return {'V': nc.vector, 'P': nc.gpsimd, 'A': nc.scalar}[c]

            def stage2(half, h):
                # G[p, ep, i, j] = dx*lohi[ep,j] - x0mxa[i], i in the half's
                # 5-wide telescoped window
                lohi = tco[:, OFF_LOHI + h * 128:OFF_LOHI + (h + 1) * 128] \
                    .rearrange("p (ep j) -> p ep j", j=NJ)
                xm = tco[:, OFF_XM + h * 9 + half * 4:
                         OFF_XM + h * 9 + half * 4 + 5]
                xm_e = xm[:, :, None].broadcast_to([128, 5, NJ])

                it = half * 2 + h
                eG, eR, eK, eD, eI = ASSIGN[it]
                G = wpool.tile([128, 2, 5, NJ], f32, tag="G", name="G")
                for ep in range(2):
                    lohi_e = lohi[:, ep, :][:, None, :].broadcast_to(
                        [128, 5, NJ])
                    eng(eG).tensor_tensor(G[:, ep], lohi_e, xm_e,
                                          ALU.subtract)
                Ksq = wpool.tile([128, 2, 5, NJ], f32, tag="Ksq", name="Ksq")
                Kf = Ksq[:].rearrange("p a b c -> p (a b c)")
                Gf = G[:].rearrange("p a b c -> p (a b c)")
                if eR is None:
                    # K = relu(G)^2 = max(G,0)*G in one STT op
                    eng(eK).scalar_tensor_tensor(Kf, Gf, zz[:, 0:1],
                                                 Gf, ALU.max, ALU.mult)
                else:
                    # per-endpoint relu+square pipelines the Act chain
                    # behind each G endpoint; relu='T' runs as a DVE
                    # tensor_scalar (2x_2p fast mode), halving Act's chain
                    R = wpool.tile([128, 2, 5, NJ], f32, tag="R", name="R")
                    for ep in range(2):
                        if eR == 'H' and ep == 0:
                            # endpoint 0 fused on DVE, endpoint 1 on Act
                            nc.vector.scalar_tensor_tensor(
                                Ksq[:, ep], G[:, ep], zz[:, 0:1], G[:, ep],
                                ALU.max, ALU.mult)
                            continue
                        if eR == 'T':
                            nc.vector.tensor_scalar(R[:, ep], G[:, ep],
                                                    0.0, None, ALU.max)
                        else:
                            nc.scalar.activation(R[:, ep], G[:, ep], ACT.Relu)
                        if eK == 'A':
                            nc.scalar.activation(Ksq[:, ep], R[:, ep],
                                                 ACT.Square)
                        else:
                            eng(eK).tensor_tensor(Ksq[:, ep], R[:, ep],
                                                  R[:, ep], ALU.mult)
                D = wpool.tile([128, 5, NJ], f32, tag="D", name="D")
                eng(eD).tensor_tensor(D[:], Ksq[:, 1], Ksq[:, 0],
                                      ALU.subtract)
                ie = wpool.tile([128, 4, NJ], f32, tag="ie", name="ie")
                eng(eI).tensor_tensor(ie[:], D[:, 0:4, :], D[:, 1:5, :],
                                      ALU.subtract)
                return G, Ksq, D, ie

            tiles = {}
            for half in range(2):
                for h in range(2):
                    tiles[(half, h)] = stage2(half, h)
            ies = {k: v[3] for k, v in tiles.items()}
            # p-state ramp chain: tiny matmuls dep'd on successively later
            # stage-2 tiles keep PE.SEQ occupied so the real matmuls are
            # dispatched >3us after PE's first barrier and run at full rate.
            # Writing into both rho banks (WAW) keeps the scheduler from
            # hoisting any real matmul ahead of the chain.
            K01 = tiles[(0, 1)][1][:, 0, 0, 0:1]
            D01 = tiles[(0, 1)][2][:, 0, 0:1]
            ie00 = tiles[(0, 0)][3]
            ie01 = tiles[(0, 1)][3]
            ie10 = tiles[(1, 0)][3]
            if chain_mode == 'own':
                chain = [(K01, 0), (D01, 1), (ie00[:, 0, 0:1], 2),
                         (ie01[:, 0, 0:1], 3), (ie00[:, 1, 0:1], 0),
                         (ie01[:, 1, 0:1], 1)]
            elif chain_mode == 'late':
                chain = [(K01, 0), (D01, 1), (ie01[:, 0, 0:1], 2),
                         (ie10[:, 0, 0:1], 3), (ie10[:, 1, 0:1], 0),
                         (ie10[:, 2, 0:1], 1), (ie10[:, 3, 0:1], 2),
                         (ie01[:, 1, 0:1], 3)]
            elif chain_mode == 'rev':
                K10 = tiles[(1, 0)][1][:, 0, 0, 0:1]
                K11 = tiles[(1, 1)][1][:, 0, 0, 0:1]
                ie11 = tiles[(1, 1)][3]
                chain = [(K10, 0), (K11, 1), (ie10[:, 0, 0:1], 2),
                         (ie11[:, 0, 0:1], 3), (ie00[:, 0, 0:1], 0),
                         (ie00[:, 1, 0:1], 1), (ie00[:, 2, 0:1], 2),
                         (ie00[:, 3, 0:1], 3)]
            elif chain_mode == 'mid':
                D11 = tiles[(1, 1)][2]
                chain = [(K01, 0), (D01, 1), (ie01[:, 0, 0:1], 2),
                         (D11[:, 0, 0:1], 3), (D11[:, 1, 0:1], 0),
                         (D11[:, 2, 0:1], 1), (D11[:, 3, 0:1], 2),
                         (ie01[:, 1, 0:1], 3)]
            else:
                chain = []
            for ct, bank in chain:
                nc.tensor.matmul(rho_ps[bank][:1, 0:1], zz[:, 0:1], ct,
                                 start=True, stop=True)
            HORD = (1, 0) if s3_mode == 'pairs-rev' else (0, 1)
            for half in HORD:
                for cc_l in range(2):
                    cch = half * 2 + cc_l
                    for h in range(2):
                        nc.tensor.matmul(
                            rho_ps[cch][:, h * 256:(h + 1) * 256],
                            ies[(half, h)][:, cc_l * 2:cc_l * 2 + 2, :]
                            .rearrange("p i j -> p (i j)"),
                            trw[:, h, :], start=True, stop=True)

            # ---- stage 3 ----
            maxr, oneh, ohss, sels = {}, {}, {}, {}

            def s3_max(cch):
                rho3 = rho_ps[cch][:].rearrange("p (g n) -> p g n", n=N)
                maxrho = tpool.tile([128, B * K], f32, tag=f"maxrho{cch}",
                                    name=f"maxrho{cch}")
                nc.vector.reduce_max(maxrho[:], rho3, axis=X)
                maxr[cch] = maxrho

            def s3_eq(cch):
                rho3 = rho_ps[cch][:].rearrange("p (g n) -> p g n", n=N)
                mx_b = maxr[cch][:][:, :, None].broadcast_to([128, B * K, N])
                onehot = tpool.tile([128, B * K * N], f32, tag=f"onehot{cch}",
                                    name=f"onehot{cch}")
                oh3 = onehot[:].rearrange("p (g n) -> p g n", n=N)
                nc.vector.tensor_tensor(oh3, rho3, mx_b, ALU.is_equal)
                oneh[cch] = onehot

            def s3_tr(cch):
                oh_t = poh.tile([128, 4 * 128], f32, tag="oht", name="oht")
                for b in range(B):
                    nc.tensor.transpose(oh_t[:, b * 128:(b + 1) * 128],
                                        oneh[cch][:, b * 128:(b + 1) * 128],
                                        ident)
                ohs = tpool.tile([128, 4 * 128], f32, tag=f"ohs{cch}",
                                 name=f"ohs{cch}")
                nc.scalar.copy(ohs[:], oh_t[:])
                ohss[cch] = ohs

            def s3_sel(cch):
                sel = psel.tile([128, B * K * 3], f32, tag=f"sel{cch % 2}",
                                name=f"sel{cch}")
                for b in range(B):
                    nc.tensor.matmul(
                        sel[:, b * K * 3:(b + 1) * K * 3],
                        ohss[cch][:, b * 128:(b + 1) * 128],
                        w4ap[:, b * K * 3:(b + 1) * K * 3],
                        start=True, stop=True)
                sels[cch] = sel

            def s3_out(cch):
                selg = sels[cch][:].rearrange("p (g q) -> p g q", q=3)
                inter2 = tpool.tile([128, B * K], f32, tag=f"inter2{cch}",
                                    name=f"inter2{cch}")
                nc.vector.tensor_tensor(inter2[:], maxr[cch][:],
                                        selg[:, :, 0], ALU.mult)
                # outt = (inter2 > halfvol) * sel_sincos, fused in one STT
                outt = tpool.tile([128, B * K, 2], f32, tag=f"outt{cch}",
                                  name=f"outt{cch}")
                i2b = inter2[:][:, :, None].broadcast_to([128, B * K, 2])
                nc.vector.scalar_tensor_tensor(outt[:], i2b, hv_col,
                                               selg[:, :, 1:3], ALU.is_gt,
                                               ALU.mult)
                nc.sync.dma_start(
                    d_out[:, cch * 128 * K:(cch + 1) * 128 * K, :]
                    .rearrange("b (p k) e -> p b k e", k=K),
                    outt[:].rearrange("p (b k) e -> p b k e", k=K))

            stages = [s3_max, s3_eq, s3_tr, s3_sel, s3_out]
            if s3_mode == 'grouped':
                for st in stages:
                    for cch in range(NCHUNK):
                        st(cch)
            elif s3_mode == 'pairs':
                for cch in range(NCHUNK):
                    s3_max(cch)
                    s3_eq(cch)
                for st in (s3_tr, s3_sel, s3_out):
                    for cch in range(NCHUNK):
                        st(cch)
            elif s3_mode == 'pairs-rev':
                for cch in (2, 3, 0, 1):
                    s3_max(cch)
                    s3_eq(cch)
                for st in (s3_tr, s3_sel, s3_out):
                    for cch in (2, 3, 0, 1):
                        st(cch)
            else:
                for cch in range(NCHUNK):
                    for st in stages:
                        st(cch)

    nc.compile()
    return nc



BIG = 1024.0


def _host_prep_v1(corners3d, neck_voxel_sizes):
    c = np.asarray(corners3d, np.float32)
    vs = np.asarray(neck_voxel_sizes, np.float32)[0]
    vox_vol = np.float32(vs[0]) * np.float32(vs[1]) * np.float32(vs[2])

    poly = c[:, :, :4, :2]                     # [B,N,4,2]
    nxt = np.roll(poly, -1, axis=2)
    xa, ya = poly[..., 0], poly[..., 1]        # [B,N,4]
    xb, yb = nxt[..., 0], nxt[..., 1]
    dx, dy = xb - xa, yb - ya
    with np.errstate(divide='ignore'):
        inv_dx = np.where(np.abs(dx) < 1e-12, np.float32(0),
                          np.float32(1) / np.where(dx == 0, np.float32(1), dx))
        inv_dy = np.where(np.abs(dy) < 1e-12, np.float32(0),
                          np.float32(1) / np.where(dy == 0, np.float32(1), dy))

    zb0 = c[:, :, :, 2].min(axis=2)
    zb1 = c[:, :, :, 2].max(axis=2)
    quad_area = 0.5 * np.abs((xa * yb - xb * ya).sum(axis=2))
    box_vol = quad_area * (zb1 - zb0)
    C = (vox_vol + box_vol + np.float32(1e-9)).astype(np.float32)   # [B,N]
    invC = (np.float32(1) / C).astype(np.float32)

    kk = np.arange(K, dtype=np.float32) + LOW[2]
    z0 = kk * vs[2]
    z1 = (kk + 1) * vs[2]
    zov = np.maximum(np.minimum(z1[None, :, None], zb1[:, None, :])
                     - np.maximum(z0[None, :, None], zb0[:, None, :]),
                     np.float32(0))                                  # [B,K,N]
    # tie-break epsilon: rho_n scaled by (1+eps_n), eps decreasing in n, so a
    # single reduce_max + is_equal yields the first-occurrence argmax; the C
    # selection weight is divided by (1+eps_n) to compensate exactly.
    eps = (np.float32(15) - np.arange(N, dtype=np.float32)) * np.float32(2.0 ** -20)
    zrho = (zov * invC[:, None, :] * (1 + eps)[None, None, :]).astype(np.float32)
    C_w4 = (C / (1 + eps)[None, :]).astype(np.float32)

    d = c[:, :, 0, :2] - c[:, :, 3, :2]
    h = np.sqrt(d[..., 0] ** 2 + d[..., 1] ** 2)
    hs = np.where(h == 0, np.float32(1), h)
    sin = np.where(h > 0, d[..., 1] / hs, np.float32(0)).astype(np.float32)
    cos = np.where(h > 0, d[..., 0] / hs, np.float32(1)).astype(np.float32)

    # --- stage-2 per-partition columns, p = b_lo*64 + n*4 + e, per h-iter ---
    def colpack(a):    # [B,N,4] -> [2h][128]
        return a.reshape(2, 2, N, 4).reshape(2, 128)
    DX_EPS = np.float32(1e-4)
    vert = np.abs(dx) < DX_EPS
    w1 = np.where(vert, np.float32(0), dy * inv_dx * np.float32(0.5))
    w2 = np.where(vert, dy, np.float32(0))
    cols = np.zeros((2, 128, 8), np.float32)
    cols[:, :, 0] = colpack(inv_dy)
    cols[:, :, 1] = colpack(-ya * inv_dy)
    cols[:, :, 2] = colpack((vs[1] - ya) * inv_dy)
    cols[:, :, 3] = colpack(dx)
    cols[:, :, 4] = colpack(xa)
    cols[:, :, 5] = colpack(w1)
    cols[:, :, 6] = colpack(w2)
    cols = np.ascontiguousarray(cols.transpose(1, 0, 2))       # [128,2,8]

    # edge-reduction weights with zrho folded in:
    # rw2[p=(b_lo,n,e), h, (b_lo',k,n')] = (b_lo'==b_lo & n'==n) * dy/2 * zrho[b,k,n]
    novert = not bool(vert.any())
    w1p = colpack(w1)                          # [2,128]
    rw = np.zeros((128, 2, 2, K, N), np.float32)
    for h in range(2):
        for p in range(128):
            b_lo, n = p // 64, (p % 64) // 4
            scale = w1p[h, p] if novert else np.float32(1)
            rw[p, h, b_lo, :, n] = zrho[2 * h + b_lo, :, n] * scale
    rw = np.ascontiguousarray(rw.reshape(128, 2, 2 * K * N))   # [128,2,256]

    # selection matmul weights: w4[b][(k,n), (k',q)] = (k==k') * w_q[b,n]
    # q in {C, sin, cos}
    w4 = np.zeros((128, B, K, 3), np.float32)
    for p in range(128):
        kq, n = p // N, p % N
        w4[p, :, kq, 0] = C_w4[:, n]
        w4[p, :, kq, 1] = sin[:, n]
        w4[p, :, kq, 2] = cos[:, n]
    w4 = np.ascontiguousarray(w4.reshape(128, B * K * 3))      # [128,96]

    # --- cells-major broadcast constants (partition-replicated by host) ---
    kbig = BIG - np.arange(N, dtype=np.float32)                # [16]
    jj = np.arange(NJ, dtype=np.float32) + LOW[1]
    y0 = (jj * vs[1]).astype(np.float32)                       # [64]
    halfvol = np.float32(0.5) * vox_vol

    consts = []
    for m in range(NCORES):
        ii = np.arange(NI, dtype=np.float32) + (m * NI + LOW[0])
        x0 = (ii * vs[0]).astype(np.float32)                   # [8]
        row = np.concatenate([kbig, y0, x0,
                              [halfvol, vs[0], -vs[0], 2 * vs[0]]]).astype(np.float32)
        cc = np.concatenate(
            [np.broadcast_to(row, (128, row.size)), cols.reshape(128, 16)],
            axis=1).astype(np.float32)
        consts.append(np.ascontiguousarray(cc))
    ident = np.eye(128, dtype=np.float32)
    return rw, w4, ident, consts, novert


def _build_v1(stages='all', fuse_w1=False):
    import concourse.bass as bass
    import concourse.tile as tile
    from concourse import bacc, mybir

    f32 = mybir.dt.float32
    ALU = mybir.AluOpType
    ACT = mybir.ActivationFunctionType

    CW = 16 + 64 + 8 + 4 + 16
    OFF_KBIG, OFF_Y0, OFF_X0, OFF_MISC, OFF_COLS = 0, 16, 80, 88, 92

    nc = bacc.Bacc("TRN2", target_bir_lowering=False, debug=False,
                   num_devices=NCORES)
    d_consts = nc.dram_tensor("consts", [128, CW], f32, kind="ExternalInput")
    d_rw = nc.dram_tensor("rw", [128, 2, 256], f32, kind="ExternalInput")
    d_w4 = nc.dram_tensor("w4", [128, B * K * 3], f32, kind="ExternalInput")
    d_ident = nc.dram_tensor("ident", [128, 128], f32, kind="ExternalInput")
    d_out = nc.dram_tensor("out", [B, NCELL * K, 2], f32, kind="ExternalOutput")

    with tile.TileContext(nc) as tc:
        with (
            tc.tile_pool(name="const", bufs=1) as cpool,
            tc.tile_pool(name="small", bufs=4) as spool,
            tc.tile_pool(name="work", bufs=6) as wpool,
            tc.tile_pool(name="edge", bufs=4) as epool,
            tc.tile_pool(name="st3", bufs=6) as tpool,
            tc.tile_pool(name="outp", bufs=4) as opool,
            tc.tile_pool(name="psum", bufs=1, space=bass.MemorySpace.PSUM) as ppool,
            tc.tile_pool(name="psum2", bufs=2, space=bass.MemorySpace.PSUM) as ppool2,
        ):
            tco = cpool.tile([128, CW], f32, tag="consts")
            nc.sync.dma_start(tco[:], d_consts[:])
            trw = cpool.tile([128, 2, 256], f32, tag="rw")
            nc.scalar.dma_start(trw[:], d_rw[:])
            tw4 = cpool.tile([128, B * K * 3], f32, tag="w4")
            nc.sync.dma_start(tw4[:], d_w4[:])
            ident = cpool.tile([128, 128], f32, tag="ident")
            nc.sync.dma_start(ident[:], d_ident[:])

            kbig_bc = tco[:, OFF_KBIG:OFF_KBIG + 16]
            y0_bc = tco[:, OFF_Y0:OFF_Y0 + 64]
            x0_bc = tco[:, OFF_X0:OFF_X0 + 8]
            halfvol_col = tco[:, OFF_MISC:OFF_MISC + 1]
            vs0_col = tco[:, OFF_MISC + 1:OFF_MISC + 2]
            nvs0_col = tco[:, OFF_MISC + 2:OFF_MISC + 3]
            vs0x2_col = tco[:, OFF_MISC + 3:OFF_MISC + 4]

            def bj(ap):   # [128,64] j-tile -> broadcast over i: [128,8,64]
                return ap[:, None, :].broadcast_to([128, NI, NJ])

            def bi(ap):   # [128,8] i-tile -> broadcast over j: [128,8,64]
                return ap[:, :, None].broadcast_to([128, NI, NJ])

            rho_ps = []
            for c in range(NCHUNK):
                rp = ppool.tile([128, B * K * N], f32, tag=f"rho{c}")
                rho_ps.append(rp)
            for h in range(2 if stages != 'none' else 0):
                col = lambda q: tco[:, OFF_COLS + h * 8 + q:OFF_COLS + h * 8 + q + 1]
                # j-only quantities [128, 64]
                ty0 = spool.tile([128, NJ], f32, tag="ty0")
                nc.scalar.activation(ty0[:], y0_bc, ACT.Identity,
                                     bias=col(1), scale=col(0))
                ty1 = spool.tile([128, NJ], f32, tag="ty1")
                nc.scalar.activation(ty1[:], y0_bc, ACT.Identity,
                                     bias=col(2), scale=col(0))
                u0 = spool.tile([128, NJ], f32, tag="u0")
                nc.vector.tensor_scalar(u0[:], ty0[:], 0.0, 1.0, ALU.max, ALU.min)
                u1 = spool.tile([128, NJ], f32, tag="u1")
                nc.vector.tensor_scalar(u1[:], ty1[:], 0.0, 1.0, ALU.max, ALU.min)
                lo = spool.tile([128, NJ], f32, tag="lo")
                nc.vector.tensor_tensor(lo[:], u0[:], u1[:], ALU.min)
                hi = spool.tile([128, NJ], f32, tag="hi")
                nc.vector.tensor_tensor(hi[:], u0[:], u1[:], ALU.max)
                hilo = spool.tile([128, NJ], f32, tag="hilo")
                nc.vector.tensor_tensor(hilo[:], hi[:], lo[:], ALU.subtract)
                # i-only quantities [128, 8]
                x0mxa = spool.tile([128, NI], f32, tag="x0mxa")
                nc.vector.tensor_single_scalar(x0mxa[:], x0_bc, col(4),
                                               ALU.subtract)
                # vertical-edge fallback: Fv = clamp(xa - x0, 0, vs0), * w2
                fvw = spool.tile([128, NI], f32, tag="fvw")
                nc.vector.tensor_scalar(fvw[:], x0mxa[:], -1.0, 0.0,
                                        ALU.mult, ALU.max)
                nc.vector.tensor_single_scalar(fvw[:], fvw[:], vs0_col, ALU.min)
                nc.vector.tensor_single_scalar(fvw[:], fvw[:], col(6), ALU.mult)

                NIH = NI // 2
                for half in range(2):
                    isl = slice(half * NIH, (half + 1) * NIH)

                    def full(tag):
                        t = wpool.tile([128, NIH, NJ], f32, tag=tag)
                        return t

                    def bjh(ap):
                        return ap[:, None, :].broadcast_to([128, NIH, NJ])

                    def bih(ap):
                        return ap[:, isl, None].broadcast_to([128, NIH, NJ])

                    # g at t=lo and t=hi:  g = dx*t - (x0 - xa)
                    glo = full("glo")
                    nc.vector.scalar_tensor_tensor(glo[:], bjh(lo[:]), col(3),
                                                   bih(x0mxa[:]), ALU.mult,
                                                   ALU.subtract)
                    ghi = full("ghi")
                    nc.vector.scalar_tensor_tensor(ghi[:], bjh(hi[:]), col(3),
                                                   bih(x0mxa[:]), ALU.mult,
                                                   ALU.subtract)
                    # H(u) = 0.5*clamp(u,0,c)^2 + c*relu(u-c); w1 carries 0.5
                    clo = full("clo")
                    nc.vector.tensor_scalar(clo[:], glo[:], 0.0, vs0_col,
                                            ALU.max, ALU.min)
                    chi = full("chi")
                    nc.gpsimd.tensor_scalar(chi[:], ghi[:], 0.0, vs0_col,
                                            ALU.max, ALU.min)
                    sqlo = full("sqlo")
                    nc.scalar.activation(sqlo[:], clo[:], ACT.Square)
                    sqhi = full("sqhi")
                    nc.scalar.activation(sqhi[:], chi[:], ACT.Square)
                    rlo = full("rlo")
                    nc.scalar.activation(rlo[:], glo[:], ACT.Relu,
                                         bias=nvs0_col)
                    rhi = full("rhi")
                    nc.scalar.activation(rhi[:], ghi[:], ACT.Relu,
                                         bias=nvs0_col)
                    e1 = full("e1")
                    nc.vector.tensor_tensor(e1[:], sqhi[:], sqlo[:],
                                            ALU.subtract)
                    e2 = full("e2")
                    nc.gpsimd.tensor_tensor(e2[:], rhi[:], rlo[:],
                                            ALU.subtract)
                    iedge = epool.tile([128, NCELL // 2], f32, tag="iedge")
                    if fuse_w1:
                        # w1 folded into rw: iedge = 2c*e2 + e1 directly
                        nc.vector.scalar_tensor_tensor(
                            iedge[:].rearrange("p (i j) -> p i j", j=NJ),
                            e2[:], vs0x2_col, e1[:], ALU.mult, ALU.add)
                    else:
                        s = full("s")
                        nc.vector.scalar_tensor_tensor(s[:], e2[:], vs0x2_col,
                                                       e1[:], ALU.mult, ALU.add)
                        t2w = full("t2w")
                        nc.gpsimd.tensor_tensor(t2w[:], bih(fvw[:]), bjh(hilo[:]),
                                                ALU.mult)
                        nc.vector.scalar_tensor_tensor(
                            iedge[:].rearrange("p (i j) -> p i j", j=NJ),
                            s[:], col(5), t2w[:], ALU.mult, ALU.add)

                    for cc in range(2):
                        cch = half * 2 + cc
                        nc.tensor.matmul(
                            rho_ps[cch][:, h * 256:(h + 1) * 256],
                            iedge[:, cc * 128:(cc + 1) * 128],
                            trw[:, h, :], start=True, stop=True)

            # ---- stage 3, cells-major, per 128-cell chunk ----
            for cch in range(NCHUNK if stages == 'all' else 0):
                rho3 = rho_ps[cch][:].rearrange("p (g n) -> p g n", n=N)
                maxrho = tpool.tile([128, B * K], f32, tag="maxrho")
                nc.vector.reduce_max(maxrho[:], rho3, axis=mybir.AxisListType.X)
                mx_bc = maxrho[:][:, :, None].broadcast_to([128, B * K, N])
                onehot = tpool.tile([128, B * K * N], f32, tag="onehot")
                nc.vector.tensor_tensor(
                    onehot[:].rearrange("p (g n) -> p g n", n=N), rho3, mx_bc,
                    ALU.is_equal)

                # selection sums via PE: transpose onehot per b, then matmul
                # against w4 -> SEL[cell, (k, {C,sin,cos})]
                oh_t = ppool2.tile([128, 4 * 128], f32, tag="oht")
                sel_ps = ppool2.tile([128, B * K * 3], f32, tag="selps")
                for b in range(B):
                    nc.tensor.transpose(
                        oh_t[:, b * 128:(b + 1) * 128],
                        onehot[:, b * 128:(b + 1) * 128], ident[:])
                ohs = tpool.tile([128, 4 * 128], f32, tag="ohs")
                nc.scalar.copy(ohs[:], oh_t[:])
                for b in range(B):
                    nc.tensor.matmul(
                        sel_ps[:, b * K * 3:(b + 1) * K * 3],
                        ohs[:, b * 128:(b + 1) * 128],
                        tw4[:, b * K * 3:(b + 1) * K * 3],
                        start=True, stop=True)

                sel3 = sel_ps[:].rearrange("p (b k q) -> p b k q", k=K, q=3)
                mx3 = maxrho[:].rearrange("p (b k) -> p b k", k=K)
                intersel = tpool.tile([128, B * K], f32, tag="intersel")
                nc.vector.tensor_tensor(
                    intersel[:].rearrange("p (b k) -> p b k", k=K),
                    mx3, sel3[:, :, :, 0], ALU.mult)
                mask = tpool.tile([128, B * K], f32, tag="mask")
                nc.vector.tensor_single_scalar(mask[:], intersel[:],
                                               halfvol_col, ALU.is_gt)
                outt = opool.tile([128, B, K, 2], f32, tag="outt")
                nc.vector.tensor_tensor(
                    outt[:, :, :, 0], sel3[:, :, :, 1],
                    mask[:].rearrange("p (b k) -> p b k", k=K), ALU.mult)
                nc.vector.tensor_tensor(
                    outt[:, :, :, 1], sel3[:, :, :, 2],
                    mask[:].rearrange("p (b k) -> p b k", k=K), ALU.mult)
                dma_eng = nc.sync
                dma_eng.dma_start(
                    d_out[:, cch * 128 * K:(cch + 1) * 128 * K, :]
                         .rearrange("b (p k) e -> p b k e", k=K),
                    outt[:])

    if stages != 'all':
        with tile.TileContext(nc) as tc2:
            with tc2.tile_pool(name="fin", bufs=1) as fpool:
                z = fpool.tile([128, 64], f32, tag="z")
                nc.gpsimd.memset(z[:], 0.0)
                nc.gpsimd.dma_start(
                    d_out[:, 0:1024, :].rearrange("b (p k) e -> p b k e", k=K),
                    z[:].rearrange("p (b k e) -> p b k e", k=K, e=2))
    nc.compile()
    return nc




_COMPILED = None


def kernel(corners3d, neck_voxel_sizes):
    global _COMPILED
    from concourse.bass_utils import run_bass_kernel_spmd

    prep3 = _host_prep_v3(corners3d, neck_voxel_sizes)
    if prep3 is not None:
        consts, rww = prep3
        if _COMPILED is None or _COMPILED[0] != 'v3':
            _COMPILED = ('v3', _build_v3(mm='f32'))
        nc = _COMPILED[1]
        in_maps = [{"consts": consts[m], "rww": rww} for m in range(NCORES)]
    else:
        prep = _host_prep_v2(corners3d, neck_voxel_sizes)
        if prep is not None:
            consts, rww = prep
            if _COMPILED is None or _COMPILED[0] != 'v2':
                _COMPILED = ('v2', _build_v2(chain_mode='late', s3_mode='pairs'))
            nc = _COMPILED[1]
            in_maps = [{"consts": consts[m], "rww": rww} for m in range(NCORES)]
        else:
            rw, w4, ident, consts1, novert = _host_prep_v1(corners3d,
                                                           neck_voxel_sizes)
            if _COMPILED is None or _COMPILED[0] != ('v1', novert):
                _COMPILED = (('v1', novert), _build_v1(fuse_w1=novert))
            nc = _COMPILED[1]
            in_maps = [{"consts": consts1[m], "rw": rw, "w4": w4,
                        "ident": ident} for m in range(NCORES)]
    res = run_bass_kernel_spmd(nc, in_maps, list(range(NCORES)))
    out = np.zeros((B, V, 2), np.float32)
    for m in range(NCORES):
        blk = res.results[m]["out"]
        out[:, m * NCELL * K:(m + 1) * NCELL * K, :] = blk
    return out.reshape(B * V, 2)


# revision 13
# speedup vs baseline: 1.1596x; 1.0410x over previous
"""Trainium2 Bass kernel v3 for Box3dEncoder (nn_Box3dEncoder_75453985456565).

v3 redesign vs v2:
  - j-telescoped stage 2: per edge the row-clip boundaries satisfy
    ty1(j) == ty0(j+1), so K(u)=relu(u)^2 is evaluated once per shared row
    boundary (65 of them) instead of per (lo,hi) pair: G/K volume halves.
    Direction sign s=sign(dy) is folded into the rw matmul weights.
  - fp32 matmuls at full p-state via a dep-staggered PE warm-up chain whose
    dummies only reference half-0 tiles, so the real matmuls dispatch as
    soon as their data lands. (float32r at 1 cyc/row was tried and is
    numerically too coarse on HW: argmax/mask flips -> rel err ~1.)
  - gpsimd cannot access PSUM, so stage 3 (reduce/is_equal/select-io) runs
    on DVE with PSUM->SBUF copies on Act; Pool carries most of stage 2 and
    Act the relu+square pairs.
"""
import numpy as np

B, N, K = 4, 16, 8
CUBE = (64, 64, 8)
LOW = (-32, -32, -4)
NCORES = 8
NI = CUBE[0] // NCORES          # 8 i-rows per core
NJ = CUBE[1]                    # 64
NB = NJ + 1                     # 65 row boundaries
NCELL = NI * NJ                 # 512 cells per core
NCHUNK = NCELL // 128           # 4
V = CUBE[0] * CUBE[1] * CUBE[2]

# v3 consts layout [128, CW3]: ub_h0 [65] | ub_h1 [65] | xm_h0 [9] | xm_h1 [9] | hv
OFF_UB = 0
OFF_XM3 = 130
OFF_HV3 = 148
CW3 = 149

RW_W = 512
W4_OFF = 512
ID_OFF = 608
RWW_W = 736


def _colpack128(a):              # [B,N,4] -> [2h][128p], p = b_lo*64+n*4+e
    return np.ascontiguousarray(a.reshape(2, 2, N, 4).reshape(2, 128))


def _host_prep_v3(corners3d, neck_voxel_sizes):
    f32 = np.float32
    c = np.asarray(corners3d, f32)
    vs = np.asarray(neck_voxel_sizes, f32)[0]
    vox_vol = f32(vs[0]) * f32(vs[1]) * f32(vs[2])
    halfvol = f32(0.5) * vox_vol

    poly = c[:, :, :4, :2]
    nxt = np.roll(poly, -1, axis=2)
    xa, ya = poly[..., 0], poly[..., 1]
    xb, yb = nxt[..., 0], nxt[..., 1]
    dx, dy = xb - xa, yb - ya
    if bool((np.abs(dx) < 1e-4).any()) or bool((np.abs(dy) < 1e-6).any()):
        return None
    invdy = (f32(1) / dy).astype(f32)
    s = np.sign(dy).astype(f32)
    w1s = (dy / dx * f32(0.5) * s).astype(f32)

    zb0 = c[:, :, :, 2].min(axis=2)
    zb1 = c[:, :, :, 2].max(axis=2)
    quad_area = f32(0.5) * np.abs((xa * yb - xb * ya).sum(axis=2, dtype=f32))
    box_vol = (quad_area * (zb1 - zb0)).astype(f32)
    C = (vox_vol + box_vol + f32(1e-9)).astype(f32)
    invC = (f32(1) / C).astype(f32)

    kk = np.arange(K, dtype=f32) + LOW[2]
    z0 = kk * vs[2]
    z1 = (kk + 1) * vs[2]
    zov = np.maximum(np.minimum(z1[None, :, None], zb1[:, None, :])
                     - np.maximum(z0[None, :, None], zb0[:, None, :]),
                     f32(0)).astype(f32)       # [B,K,N]
    # first-occurrence-argmax tie-break epsilons (see v2)
    eps = (f32(15) - np.arange(N, dtype=f32)) * f32(2.0 ** -20)
    zrho = (zov * invC[:, None, :] * (1 + eps)[None, None, :]).astype(f32)
    C_w4 = (C / (1 + eps)[None, :]).astype(f32)

    d0 = c[:, :, 0, :2] - c[:, :, 3, :2]
    h0 = np.sqrt(d0[..., 0] ** 2 + d0[..., 1] ** 2)
    hs = np.where(h0 == 0, f32(1), h0)
    sin = np.where(h0 > 0, d0[..., 1] / hs, f32(0)).astype(f32)
    cos = np.where(h0 > 0, d0[..., 0] / hs, f32(1)).astype(f32)

    jb = np.arange(NB, dtype=f32) + LOW[1]
    ybnd = (jb * vs[1]).astype(f32)
    t = ((ybnd[None, None, None, :] - ya[..., None])
         * invdy[..., None]).astype(f32)       # [B,N,4,65]
    u = np.clip(t, f32(0), f32(1))
    ub = (u * dx[..., None]).astype(f32)
    ub_p = (ub.reshape(2, 2, N, 4, NB)
            .transpose(1, 2, 3, 0, 4)
            .reshape(128, 2, NB))
    xa_p = _colpack128(xa)

    consts = []
    for m in range(NCORES):
        cc = np.zeros((128, CW3), f32)
        cc[:, OFF_UB:OFF_UB + NB] = ub_p[:, 0, :]
        cc[:, OFF_UB + NB:OFF_UB + 2 * NB] = ub_p[:, 1, :]
        ii = np.arange(NI + 1, dtype=f32) + (m * NI + LOW[0])
        x0i = (ii * vs[0]).astype(f32)
        for h in range(2):
            cc[:, OFF_XM3 + h * 9:OFF_XM3 + (h + 1) * 9] = \
                x0i[None, :] - xa_p[h][:, None]
        cc[:, OFF_HV3] = halfvol
        consts.append(np.ascontiguousarray(cc))

    w1p = _colpack128(w1s)
    rw = np.zeros((128, 2, 2, K, N), f32)
    for h in range(2):
        for p in range(128):
            b_lo, n = p // 64, (p % 64) // 4
            rw[p, h, b_lo, :, n] = zrho[2 * h + b_lo, :, n] * w1p[h, p]
    rw = rw.reshape(128, 512)

    w4 = np.zeros((128, B, K, 3), f32)
    for p in range(128):
        kq, n = p // N, p % N
        w4[p, :, kq, 0] = C_w4[:, n]
        w4[p, :, kq, 1] = sin[:, n]
        w4[p, :, kq, 2] = cos[:, n]
    w4 = w4.reshape(128, B * K * 3)

    ident = np.eye(128, dtype=f32)
    rww = np.ascontiguousarray(np.concatenate([rw, w4, ident], axis=1))
    return consts, rww


def _build_v3(mm='f32r', cfg=None):
    import concourse.bass as bass
    import concourse.tile as tile
    from concourse import bacc, mybir

    # gpsimd cannot touch PSUM, so stage 3 (reduce/eq/io read PSUM) is
    # DVE-only unless rho is first copied to SBUF (rc='A'), which lets Pool
    # run is_equal; Pool otherwise carries stage 2, Act the K relu+square
    # pairs and the PSUM->SBUF copies.
    cfg = cfg or {}
    S2 = cfg.get('s2') or [('V', 'V', 'V', 'V'), ('P', 'V', 'V', 'V'),
                           ('P', 'A2', 'P', 'P'), ('P', 'V', 'P', 'P')]
    RED = cfg.get('red', 'VVVV')
    EQ = cfg.get('eq', 'VVVV')
    IO = cfg.get('io', 'VVVV')
    CPY = cfg.get('cpy', 'AAAA')
    OQ = cfg.get('oq', 'SASS')
    RC = cfg.get('rc', '....')

    f32 = mybir.dt.float32
    f32r = mybir.dt.float32r
    wdt = f32r if mm == 'f32r' else f32
    ALU = mybir.AluOpType
    X = mybir.AxisListType.X

    nc = bacc.Bacc("TRN2", target_bir_lowering=False, debug=False,
                   num_devices=NCORES)
    d_consts = nc.dram_tensor("consts", [128, CW3], f32, kind="ExternalInput")
    d_rww = nc.dram_tensor("rww", [128, RWW_W], wdt, kind="ExternalInput")
    d_out = nc.dram_tensor("out", [B, NCELL * K, 2], f32, kind="ExternalOutput")

    with tile.TileContext(nc) as tc:
        with (
            tc.tile_pool(name="const", bufs=1) as cpool,
            tc.tile_pool(name="const2", bufs=1) as cpool2,
            tc.tile_pool(name="s2", bufs=4) as wpool,
            tc.tile_pool(name="s3", bufs=1) as tpool,
            tc.tile_pool(name="psr", bufs=1, space=bass.MemorySpace.PSUM) as prho,
            tc.tile_pool(name="psoh", bufs=2, space=bass.MemorySpace.PSUM) as poh,
            tc.tile_pool(name="pssel", bufs=1, space=bass.MemorySpace.PSUM) as psel,
        ):
            tco = cpool.tile([128, CW3], f32, tag="consts")
            nc.sync.dma_start(tco[:], d_consts[:])
            trww = cpool2.tile([128, RWW_W], wdt, tag="rww")
            nc.sync.dma_start(trww[:], d_rww[:])

            zz = cpool.tile([128, 8], f32, tag="zz")
            nc.vector.memset(zz[:], 0.0)

            trw = trww[:, 0:RW_W].rearrange("p (h q) -> p h q", q=256)
            ident = trww[:, ID_OFF:ID_OFF + 128].bitcast(f32)
            w4ap = trww[:, W4_OFF:W4_OFF + B * K * 3].bitcast(f32)
            hv_col = tco[:, OFF_HV3:OFF_HV3 + 1]

            rho_ps = []
            for cch in range(NCHUNK):
                rho_ps.append(prho.tile([128, 512], f32, tag=f"rho{cch}",
                                        name=f"rho{cch}"))

            def eng(e):
                return {'V': nc.vector, 'P': nc.gpsimd, 'A': nc.scalar}[e]

            nc.tensor.matmul(rho_ps[0][:8, 0:8], zz[:], zz[:],
                             start=True, stop=True)

            def stage2(half, h):
                ubh = tco[:, OFF_UB + h * NB:OFF_UB + (h + 1) * NB]
                xm5 = tco[:, OFF_XM3 + h * 9 + half * 4:
                          OFF_XM3 + h * 9 + half * 4 + 5]
                eG, eK, eD, eN = S2[half * 2 + h]
                G = wpool.tile([128, 5, NB], f32, tag="G", name="G")
                ub_b = ubh[:, None, :].broadcast_to([128, 5, NB])
                xm_b = xm5[:, :, None].broadcast_to([128, 5, NB])
                eng(eG).tensor_tensor(G[:], ub_b, xm_b, ALU.subtract)
                Ksq = wpool.tile([128, 5, NB], f32, tag="Ksq", name="Ksq")
                if eK == 'A2':
                    ACT = mybir.ActivationFunctionType
                    R = wpool.tile([128, 5, NB], f32, tag="R", name="R")
                    nc.scalar.activation(R[:].rearrange("p a b -> p (a b)"),
                                         G[:].rearrange("p a b -> p (a b)"),
                                         ACT.Relu)
                    nc.scalar.activation(Ksq[:].rearrange("p a b -> p (a b)"),
                                         R[:].rearrange("p a b -> p (a b)"),
                                         ACT.Square)
                else:
                    eng(eK).scalar_tensor_tensor(
                        Ksq[:].rearrange("p a b -> p (a b)"),
                        G[:].rearrange("p a b -> p (a b)"), zz[:, 0:1],
                        G[:].rearrange("p a b -> p (a b)"), ALU.max, ALU.mult)
                if mm == 'f32r':
                    # D' at the high boundaries (jb=1..64) and -D' at the low
                    # boundaries (jb=0..63), both contiguous [4, 64] so the
                    # matmul weight slices stay walrus-legal
                    Dp = wpool.tile([128, 4, NJ], wdt, tag="Dp", name="Dp")
                    eng(eD).tensor_tensor(Dp[:], Ksq[:, 0:4, 1:NB],
                                          Ksq[:, 1:5, 1:NB], ALU.subtract)
                    Dn = wpool.tile([128, 4, NJ], wdt, tag="Dn", name="Dn")
                    eng(eN).tensor_tensor(Dn[:], Ksq[:, 1:5, 0:NJ],
                                          Ksq[:, 0:4, 0:NJ], ALU.subtract)
                    return G, Ksq, Dp, Dn
                Dp = wpool.tile([128, 4, NB], f32, tag="Dp", name="Dp")
                eng(eD).tensor_tensor(Dp[:], Ksq[:, 0:4, :], Ksq[:, 1:5, :],
                                      ALU.subtract)
                ie = wpool.tile([128, 4, NJ], f32, tag="ie", name="ie")
                eng(eN).tensor_tensor(ie[:], Dp[:, :, 1:NB], Dp[:, :, 0:NJ],
                                      ALU.subtract)
                return G, Ksq, Dp, ie

            tiles = {}
            for half in range(2):
                for h in range(2):
                    tiles[(half, h)] = stage2(half, h)

            t00, t01 = tiles[(0, 0)], tiles[(0, 1)]
            # dummies dep'd only on half-0 tiles (ready 3.5-6.1us) so the
            # real matmuls dispatch as soon as their data exists, at full
            # p-state
            chain = [(t00[0][:, 0, 0:1], 0), (t01[0][:, 0, 0:1], 1),
                     (t00[1][:, 0, 0:1], 2), (t00[2][:, 0, 0:1], 3),
                     (t00[3][:, 0, 0:1], 0), (t01[1][:, 0, 0:1], 1),
                     (t01[2][:, 0, 0:1], 2), (t01[3][:, 0, 0:1], 3)]
            for ct, bank in chain:
                nc.tensor.matmul(rho_ps[bank][:1, 0:1], zz[:, 0:1], ct,
                                 start=True, stop=True)

            for half in range(2):
                for cc_l in range(2):
                    cch = half * 2 + cc_l
                    for h in range(2):
                        dst = rho_ps[cch][:, h * 256:(h + 1) * 256]
                        if mm == 'f32r':
                            _, _, Dp, Dn = tiles[(half, h)]
                            mva = Dp[:, cc_l * 2:cc_l * 2 + 2, :]
                            mvb = Dn[:, cc_l * 2:cc_l * 2 + 2, :]
                            nc.tensor.matmul(dst, mva, trw[:, h, :],
                                             start=True, stop=False)
                            nc.tensor.matmul(dst, mvb, trw[:, h, :],
                                             start=False, stop=True)
                        else:
                            ie = tiles[(half, h)][3]
                            nc.tensor.matmul(
                                dst,
                                ie[:, cc_l * 2:cc_l * 2 + 2, :]
                                .rearrange("p i j -> p (i j)"),
                                trw[:, h, :], start=True, stop=True)

            # ---- stage 3 ----
            maxr, oneh, ohss, sels = {}, {}, {}, {}

            rho_sb = {}

            def s3_max(cch):
                if RC[cch] == 'A':
                    rs = tpool.tile([128, 512], f32, tag=f"rhosb{cch}",
                                    name=f"rhosb{cch}")
                    nc.scalar.copy(rs[:], rho_ps[cch][:])
                    rho_sb[cch] = rs
                src = rho_sb.get(cch, rho_ps[cch])
                rho3 = src[:].rearrange("p (g n) -> p g n", n=N)
                maxrho = tpool.tile([128, B * K], f32, tag=f"maxrho{cch}",
                                    name=f"maxrho{cch}")
                eng(RED[cch]).reduce_max(maxrho[:], rho3, axis=X)
                maxr[cch] = maxrho

            def s3_eq(cch):
                src = rho_sb.get(cch, rho_ps[cch])
                rho3 = src[:].rearrange("p (g n) -> p g n", n=N)
                mx_b = maxr[cch][:][:, :, None].broadcast_to([128, B * K, N])
                onehot = tpool.tile([128, B * K * N], f32, tag=f"onehot{cch}",
                                    name=f"onehot{cch}")
                oh3 = onehot[:].rearrange("p (g n) -> p g n", n=N)
                eng(EQ[cch]).tensor_tensor(oh3, rho3, mx_b, ALU.is_equal)
                oneh[cch] = onehot

            def s3_tr(cch):
                oh_t = poh.tile([128, 4 * 128], f32, tag="oht", name="oht")
                for b in range(B):
                    nc.tensor.transpose(oh_t[:, b * 128:(b + 1) * 128],
                                        oneh[cch][:, b * 128:(b + 1) * 128],
                                        ident)
                ohs = tpool.tile([128, 4 * 128], f32, tag=f"ohs{cch}",
                                 name=f"ohs{cch}")
                if CPY[cch] == 'V':
                    nc.vector.tensor_scalar(ohs[:], oh_t[:], 0.0, None,
                                            ALU.add)
                else:
                    nc.scalar.copy(ohs[:], oh_t[:])
                ohss[cch] = ohs

            def s3_sel(cch):
                sel = psel.tile([128, B * K * 3], f32, tag=f"sel{cch % 2}",
                                name=f"sel{cch}")
                for b in range(B):
                    nc.tensor.matmul(
                        sel[:, b * K * 3:(b + 1) * K * 3],
                        ohss[cch][:, b * 128:(b + 1) * 128],
                        w4ap[:, b * K * 3:(b + 1) * K * 3],
                        start=True, stop=True)
                sels[cch] = sel

            def s3_out(cch):
                selg = sels[cch][:].rearrange("p (g q) -> p g q", q=3)
                inter2 = tpool.tile([128, B * K], f32, tag=f"inter2{cch}",
                                    name=f"inter2{cch}")
                e = eng(IO[cch])
                e.tensor_tensor(inter2[:], maxr[cch][:], selg[:, :, 0],
                                ALU.mult)
                outt = tpool.tile([128, B * K, 2], f32, tag=f"outt{cch}",
                                  name=f"outt{cch}")
                i2b = inter2[:][:, :, None].broadcast_to([128, B * K, 2])
                e.scalar_tensor_tensor(outt[:], i2b, hv_col,
                                       selg[:, :, 1:3], ALU.is_gt, ALU.mult)
                q = {'S': nc.sync, 'A': nc.scalar}[OQ[cch]]
                q.dma_start(
                    d_out[:, cch * 128 * K:(cch + 1) * 128 * K, :]
                    .rearrange("b (p k) e -> p b k e", k=K),
                    outt[:].rearrange("p (b k) e -> p b k e", k=K))

            for cch in range(NCHUNK):
                s3_max(cch)
                s3_eq(cch)
            for st in (s3_tr, s3_sel, s3_out):
                for cch in range(NCHUNK):
                    st(cch)

    nc.compile()
    return nc


# ---- v2/v1 fallback constants ----
OFF_LOHI = 0                    # [h][ep][j]  2*2*64 = 256
OFF_XM = 256                    # [h][i]      2*9 = 18
OFF_DX = 274                    # [h]         2
OFF_HV = 276                    # halfvol     1
CWP = 280

def _host_prep_v2(corners3d, neck_voxel_sizes):
    f32 = np.float32
    c = np.asarray(corners3d, f32)
    vs = np.asarray(neck_voxel_sizes, f32)[0]
    vox_vol = f32(vs[0]) * f32(vs[1]) * f32(vs[2])
    halfvol = f32(0.5) * vox_vol

    poly = c[:, :, :4, :2]                     # [B,N,4,2]
    nxt = np.roll(poly, -1, axis=2)
    xa, ya = poly[..., 0], poly[..., 1]        # [B,N,4]
    xb, yb = nxt[..., 0], nxt[..., 1]
    dx, dy = xb - xa, yb - ya
    vert = np.abs(dx) < f32(1e-4)
    if bool(vert.any()):
        return None                            # caller falls back to v1
    invdy = (f32(1) / dy).astype(f32)
    w1 = (dy / dx * f32(0.5)).astype(f32)      # [B,N,4]

    zb0 = c[:, :, :, 2].min(axis=2)
    zb1 = c[:, :, :, 2].max(axis=2)
    quad_area = f32(0.5) * np.abs((xa * yb - xb * ya).sum(axis=2, dtype=f32))
    box_vol = (quad_area * (zb1 - zb0)).astype(f32)
    C = (vox_vol + box_vol + f32(1e-9)).astype(f32)
    invC = (f32(1) / C).astype(f32)

    kk = np.arange(K, dtype=f32) + LOW[2]
    z0 = kk * vs[2]
    z1 = (kk + 1) * vs[2]
    zov = np.maximum(np.minimum(z1[None, :, None], zb1[:, None, :])
                     - np.maximum(z0[None, :, None], zb0[:, None, :]),
                     f32(0)).astype(f32)       # [B,K,N]
    eps = (f32(15) - np.arange(N, dtype=f32)) * f32(2.0 ** -20)
    zrho = (zov * invC[:, None, :] * (1 + eps)[None, None, :]).astype(f32)
    C_w4 = (C / (1 + eps)[None, :]).astype(f32)

    d0 = c[:, :, 0, :2] - c[:, :, 3, :2]
    h0 = np.sqrt(d0[..., 0] ** 2 + d0[..., 1] ** 2)
    hs = np.where(h0 == 0, f32(1), h0)
    sin = np.where(h0 > 0, d0[..., 1] / hs, f32(0)).astype(f32)
    cos = np.where(h0 > 0, d0[..., 0] / hs, f32(1)).astype(f32)

    # host j-prep: t-window [lo,hi] of y(t) in [y0_j, y0_j+vs] per (b,n,e,j)
    jj = np.arange(NJ, dtype=f32) + LOW[1]
    y0j = (jj * vs[1]).astype(f32)
    ty0 = ((y0j[None, None, None, :] - ya[..., None]) * invdy[..., None]).astype(f32)
    ty1 = (ty0 + (vs[1] * invdy)[..., None]).astype(f32)
    # note: reference computes ty1 from (y0+vs-ya)*invdy; keep that exact form
    ty1 = ((y0j[None, None, None, :] + vs[1] - ya[..., None])
           * invdy[..., None]).astype(f32)
    u0 = np.clip(ty0, f32(0), f32(1))
    u1 = np.clip(ty1, f32(0), f32(1))
    lo = np.minimum(u0, u1).astype(f32)        # [B,N,4,64]
    hi = np.maximum(u0, u1).astype(f32)

    def colpack(a):                            # [B,N,4] -> [2h][128p]
        return np.ascontiguousarray(a.reshape(2, 2, N, 4).reshape(2, 128))

    # consts per core; dx is folded into lo/hi on the host so the device
    # G op is a plain tensor_tensor (legal on GpSimd)
    lohi = np.stack([lo, hi], axis=3) * dx[..., None, None]   # [B,N,4,2,64]
    lohi_p = (lohi.reshape(2, 2, N, 4, 2, NJ)  # (h, b_lo, n, e, ep, j)
              .transpose(1, 2, 3, 0, 4, 5)     # (b_lo, n, e, h, ep, j)
              .reshape(128, 256))
    xa_p = colpack(xa)
    dx_p = colpack(dx)

    consts = []
    for m in range(NCORES):
        cc = np.zeros((128, CWP), f32)
        cc[:, OFF_LOHI:OFF_LOHI + 256] = lohi_p
        ii = np.arange(NI + 1, dtype=f32) + (m * NI + LOW[0])
        x0i = (ii * vs[0]).astype(f32)         # [9]
        for h in range(2):
            cc[:, OFF_XM + h * 9:OFF_XM + (h + 1) * 9] = \
                x0i[None, :] - xa_p[h][:, None]
            cc[:, OFF_DX + h] = dx_p[h]
        cc[:, OFF_HV] = halfvol
        consts.append(np.ascontiguousarray(cc))

    # rw [128, 2, 256]: zrho (w1 folded) at (b_lo, k, n) per partition
    w1p = colpack(w1)
    rw = np.zeros((128, 2, 2, K, N), f32)
    for h in range(2):
        for p in range(128):
            b_lo, n = p // 64, (p % 64) // 4
            rw[p, h, b_lo, :, n] = zrho[2 * h + b_lo, :, n] * w1p[h, p]
    rw = rw.reshape(128, 512)

    # w4 [128, 96]: selection weights, p = (k, n)
    w4 = np.zeros((128, B, K, 3), f32)
    for p in range(128):
        kq, n = p // N, p % N
        w4[p, :, kq, 0] = C_w4[:, n]
        w4[p, :, kq, 1] = sin[:, n]
        w4[p, :, kq, 2] = cos[:, n]
    w4 = w4.reshape(128, B * K * 3)

    ident = np.eye(128, dtype=f32)
    rww = np.ascontiguousarray(np.concatenate([rw, w4, ident], axis=1))
    return consts, rww


def _build_v2(chain_mode='own', s3_mode='grouped', assign=None):
    import concourse.bass as bass
    import concourse.tile as tile
    from concourse import bacc, mybir

    f32 = mybir.dt.float32
    ALU = mybir.AluOpType
    ACT = mybir.ActivationFunctionType
    X = mybir.AxisListType.X

    nc = bacc.Bacc("TRN2", target_bir_lowering=False, debug=False,
                   num_devices=NCORES)
    d_consts = nc.dram_tensor("consts", [128, CWP], f32, kind="ExternalInput")
    d_rww = nc.dram_tensor("rww", [128, RWW_W], f32, kind="ExternalInput")
    d_out = nc.dram_tensor("out", [B, NCELL * K, 2], f32, kind="ExternalOutput")

    with tile.TileContext(nc) as tc:
        with (
            tc.tile_pool(name="const", bufs=1) as cpool,
            tc.tile_pool(name="const2", bufs=1) as cpool2,
            tc.tile_pool(name="s2", bufs=4) as wpool,
            tc.tile_pool(name="s3", bufs=1) as tpool,
            tc.tile_pool(name="psr", bufs=1, space=bass.MemorySpace.PSUM) as prho,
            tc.tile_pool(name="psoh", bufs=2, space=bass.MemorySpace.PSUM) as poh,
            tc.tile_pool(name="pssel", bufs=1, space=bass.MemorySpace.PSUM) as psel,
        ):
            # --- input DMAs: consts on Pool queue, weights on SP queue ---
            tco = cpool.tile([128, CWP], f32, tag="consts")
            nc.sync.dma_start(tco[:], d_consts[:])
            trww = cpool2.tile([128, RWW_W], wdt, tag="rww")
            nc.sync.dma_start(trww[:], d_rww[:])

            # --- PE warm-up: pin the p-state ramp start near t=0 ---
            zz = cpool.tile([128, 8], f32, tag="zz")
            nc.vector.memset(zz[:], 0.0)

            trw = trww[:, 0:RW_W].rearrange("p (h q) -> p h q", q=256)
            ident = trww[:, ID_OFF:ID_OFF + 128]
            hv_col = tco[:, OFF_HV:OFF_HV + 1]

            rho_ps = []
            for cch in range(NCHUNK):
                rp = prho.tile([128, 2 * K * N * 2], f32, tag=f"rho{cch}",
                               name=f"rho{cch}")
                rho_ps.append(rp)
            # PE warm-up target: scratch corner of rho bank 0, overwritten
            # later by the real start=True matmul
            nc.tensor.matmul(rho_ps[0][:8, 0:8], zz[:], zz[:],
                             start=True, stop=True)

            # per-iter engine assignment: (G, relu, sq, D, ie)
            # engines: V=vector/DVE, P=gpsimd/Pool, A=scalar/Act
            # (G, relu, sq, D, ie); relu=None -> K fused as (G max 0) * G
            ASSIGN = assign or [
                ('V', 'A', 'A', 'V', 'P'),
                ('P', None, 'V', 'V', 'P'),
                ('V', 'A', 'A', 'V', 'P'),
                ('P', None, 'V', 'V', 'P'),
            ]

            def eng(c):
                return {'V': nc.vector, 'P': nc.gpsimd, 'A': nc.scalar}[c]

            def stage2(half, h):
                # G[p, ep, i, j] = dx*lohi[ep,j] - x0mxa[i], i in the half's
                # 5-wide telescoped window
                lohi = tco[:, OFF_LOHI + h * 128:OFF_LOHI + (h + 1) * 128] \
                    .rearrange("p (ep j) -> p ep j", j=NJ)
                xm = tco[:, OFF_XM + h * 9 + half * 4:
                         OFF_XM + h * 9 + half * 4 + 5]
                xm_e = xm[:, :, None].broadcast_to([128, 5, NJ])

                it = half * 2 + h
                eG, eR, eK, eD, eI = ASSIGN[it]
                G = wpool.tile([128, 2, 5, NJ], f32, tag="G", name="G")
                for ep in range(2):
                    lohi_e = lohi[:, ep, :][:, None, :].broadcast_to(
                        [128, 5, NJ])
                    eng(eG).tensor_tensor(G[:, ep], lohi_e, xm_e,
                                          ALU.subtract)
                Ksq = wpool.tile([128, 2, 5, NJ], f32, tag="Ksq", name="Ksq")
                Kf = Ksq[:].rearrange("p a b c -> p (a b c)")
                Gf = G[:].rearrange("p a b c -> p (a b c)")
                if eR is None:
                    # K = relu(G)^2 = max(G,0)*G in one STT op
                    eng(eK).scalar_tensor_tensor(Kf, Gf, zz[:, 0:1],
                                                 Gf, ALU.max, ALU.mult)
                else:
                    # per-endpoint relu+square pipelines the Act chain
                    # behind each G endpoint; relu='T' runs as a DVE
                    # tensor_scalar (2x_2p fast mode), halving Act's chain
                    R = wpool.tile([128, 2, 5, NJ], f32, tag="R", name="R")
                    for ep in range(2):
                        if eR == 'H' and ep == 0:
                            # endpoint 0 fused on DVE, endpoint 1 on Act
                            nc.vector.scalar_tensor_tensor(
                                Ksq[:, ep], G[:, ep], zz[:, 0:1], G[:, ep],
                                ALU.max, ALU.mult)
                            continue
                        if eR == 'T':
                            nc.vector.tensor_scalar(R[:, ep], G[:, ep],
                                                    0.0, None, ALU.max)
                        else:
                            nc.scalar.activation(R[:, ep], G[:, ep], ACT.Relu)
                        if eK == 'A':
                            nc.scalar.activation(Ksq[:, ep], R[:, ep],
                                                 ACT.Square)
                        else:
                            eng(eK).tensor_tensor(Ksq[:, ep], R[:, ep],
                                                  R[:, ep], ALU.mult)
                D = wpool.tile([128, 5, NJ], f32, tag="D", name="D")
                eng(eD).tensor_tensor(D[:], Ksq[:, 1], Ksq[:, 0],
                                      ALU.subtract)
                ie = wpool.tile([128, 4, NJ], f32, tag="ie", name="ie")
                eng(eI).tensor_tensor(ie[:], D[:, 0:4, :], D[:, 1:5, :],
                                      ALU.subtract)
                return G, Ksq, D, ie

            tiles = {}
            for half in range(2):
                for h in range(2):
                    tiles[(half, h)] = stage2(half, h)
            ies = {k: v[3] for k, v in tiles.items()}
            # p-state ramp chain: tiny matmuls dep'd on successively later
            # stage-2 tiles keep PE.SEQ occupied so the real matmuls are
            # dispatched >3us after PE's first barrier and run at full rate.
            # Writing into both rho banks (WAW) keeps the scheduler from
            # hoisting any real matmul ahead of the chain.
            K01 = tiles[(0, 1)][1][:, 0, 0, 0:1]
            D01 = tiles[(0, 1)][2][:, 0, 0:1]
            ie00 = tiles[(0, 0)][3]
            ie01 = tiles[(0, 1)][3]
            ie10 = tiles[(1, 0)][3]
            if chain_mode == 'own':
                chain = [(K01, 0), (D01, 1), (ie00[:, 0, 0:1], 2),
                         (ie01[:, 0, 0:1], 3), (ie00[:, 1, 0:1], 0),
                         (ie01[:, 1, 0:1], 1)]
            elif chain_mode == 'late':
                chain = [(K01, 0), (D01, 1), (ie01[:, 0, 0:1], 2),
                         (ie10[:, 0, 0:1], 3), (ie10[:, 1, 0:1], 0),
                         (ie10[:, 2, 0:1], 1), (ie10[:, 3, 0:1], 2),
                         (ie01[:, 1, 0:1], 3)]
            elif chain_mode == 'rev':
                K10 = tiles[(1, 0)][1][:, 0, 0, 0:1]
                K11 = tiles[(1, 1)][1][:, 0, 0, 0:1]
                ie11 = tiles[(1, 1)][3]
                chain = [(K10, 0), (K11, 1), (ie10[:, 0, 0:1], 2),
                         (ie11[:, 0, 0:1], 3), (ie00[:, 0, 0:1], 0),
                         (ie00[:, 1, 0:1], 1), (ie00[:, 2, 0:1], 2),
                         (ie00[:, 3, 0:1], 3)]
            elif chain_mode == 'mid':
                D11 = tiles[(1, 1)][2]
                chain = [(K01, 0), (D01, 1), (ie01[:, 0, 0:1], 2),
                         (D11[:, 0, 0:1], 3), (D11[:, 1, 0:1], 0),
                         (D11[:, 2, 0:1], 1), (D11[:, 3, 0:1], 2),
                         (ie01[:, 1, 0:1], 3)]
            else:
                chain = []
            for ct, bank in chain:
                nc.tensor.matmul(rho_ps[bank][:1, 0:1], zz[:, 0:1], ct,
                                 start=True, stop=True)
            HORD = (1, 0) if s3_mode == 'pairs-rev' else (0, 1)
            for half in HORD:
                for cc_l in range(2):
                    cch = half * 2 + cc_l
                    for h in range(2):
                        nc.tensor.matmul(
                            rho_ps[cch][:, h * 256:(h + 1) * 256],
                            ies[(half, h)][:, cc_l * 2:cc_l * 2 + 2, :]
                            .rearrange("p i j -> p (i j)"),
                            trw[:, h, :], start=True, stop=True)

            # ---- stage 3 ----
            maxr, oneh, ohss, sels = {}, {}, {}, {}

            def s3_max(cch):
                rho3 = rho_ps[cch][:].rearrange("p (g n) -> p g n", n=N)
                maxrho = tpool.tile([128, B * K], f32, tag=f"maxrho{cch}",
                                    name=f"maxrho{cch}")
                nc.vector.reduce_max(maxrho[:], rho3, axis=X)
                maxr[cch] = maxrho

            def s3_eq(cch):
                rho3 = rho_ps[cch][:].rearrange("p (g n) -> p g n", n=N)
                mx_b = maxr[cch][:][:, :, None].broadcast_to([128, B * K, N])
                onehot = tpool.tile([128, B * K * N], f32, tag=f"onehot{cch}",
                                    name=f"onehot{cch}")
                oh3 = onehot[:].rearrange("p (g n) -> p g n", n=N)
                nc.vector.tensor_tensor(oh3, rho3, mx_b, ALU.is_equal)
                oneh[cch] = onehot

            def s3_tr(cch):
                oh_t = poh.tile([128, 4 * 128], f32, tag="oht", name="oht")
                for b in range(B):
                    nc.tensor.transpose(oh_t[:, b * 128:(b + 1) * 128],
                                        oneh[cch][:, b * 128:(b + 1) * 128],
                                        ident)
                ohs = tpool.tile([128, 4 * 128], f32, tag=f"ohs{cch}",
                                 name=f"ohs{cch}")
                nc.scalar.copy(ohs[:], oh_t[:])
                ohss[cch] = ohs

            def s3_sel(cch):
                sel = psel.tile([128, B * K * 3], f32, tag=f"sel{cch % 2}",
                                name=f"sel{cch}")
                for b in range(B):
                    nc.tensor.matmul(
                        sel[:, b * K * 3:(b + 1) * K * 3],
                        ohss[cch][:, b * 128:(b + 1) * 128],
                        w4ap[:, b * K * 3:(b + 1) * K * 3],
                        start=True, stop=True)
                sels[cch] = sel

            def s3_out(cch):
                selg = sels[cch][:].rearrange("p (g q) -> p g q", q=3)
                inter2 = tpool.tile([128, B * K], f32, tag=f"inter2{cch}",
                                    name=f"inter2{cch}")
                nc.vector.tensor_tensor(inter2[:], maxr[cch][:],
                                        selg[:, :, 0], ALU.mult)
                # outt = (inter2 > halfvol) * sel_sincos, fused in one STT
                outt = tpool.tile([128, B * K, 2], f32, tag=f"outt{cch}",
                                  name=f"outt{cch}")
                i2b = inter2[:][:, :, None].broadcast_to([128, B * K, 2])
                nc.vector.scalar_tensor_tensor(outt[:], i2b, hv_col,
                                               selg[:, :, 1:3], ALU.is_gt,
                                               ALU.mult)
                nc.sync.dma_start(
                    d_out[:, cch * 128 * K:(cch + 1) * 128 * K, :]
                    .rearrange("b (p k) e -> p b k e", k=K),
                    outt[:].rearrange("p (b k) e -> p b k e", k=K))

            stages = [s3_max, s3_eq, s3_tr, s3_sel, s3_out]
            if s3_mode == 'grouped':
                for st in stages:
                    for cch in range(NCHUNK):
                        st(cch)
            elif s3_mode == 'pairs':
                for cch in range(NCHUNK):
                    s3_max(cch)
                    s3_eq(cch)
                for st in (s3_tr, s3_sel, s3_out):
                    for cch in range(NCHUNK):
                        st(cch)
            elif s3_mode == 'pairs-rev':
                for cch in (2, 3, 0, 1):
                    s3_max(cch)
                    s3_eq(cch)
                for st in (s3_tr, s3_sel, s3_out):
                    for cch in (2, 3, 0, 1):
                        st(cch)
            else:
                for cch in range(NCHUNK):
                    for st in stages:
                        st(cch)

    nc.compile()
    return nc



BIG = 1024.0


def _host_prep_v1(corners3d, neck_voxel_sizes):
    c = np.asarray(corners3d, np.float32)
    vs = np.asarray(neck_voxel_sizes, np.float32)[0]
    vox_vol = np.float32(vs[0]) * np.float32(vs[1]) * np.float32(vs[2])

    poly = c[:, :, :4, :2]                     # [B,N,4,2]
    nxt = np.roll(poly, -1, axis=2)
    xa, ya = poly[..., 0], poly[..., 1]        # [B,N,4]
    xb, yb = nxt[..., 0], nxt[..., 1]
    dx, dy = xb - xa, yb - ya
    with np.errstate(divide='ignore'):
        inv_dx = np.where(np.abs(dx) < 1e-12, np.float32(0),
                          np.float32(1) / np.where(dx == 0, np.float32(1), dx))
        inv_dy = np.where(np.abs(dy) < 1e-12, np.float32(0),
                          np.float32(1) / np.where(dy == 0, np.float32(1), dy))

    zb0 = c[:, :, :, 2].min(axis=2)
    zb1 = c[:, :, :, 2].max(axis=2)
    quad_area = 0.5 * np.abs((xa * yb - xb * ya).sum(axis=2))
    box_vol = quad_area * (zb1 - zb0)
    C = (vox_vol + box_vol + np.float32(1e-9)).astype(np.float32)   # [B,N]
    invC = (np.float32(1) / C).astype(np.float32)

    kk = np.arange(K, dtype=np.float32) + LOW[2]
    z0 = kk * vs[2]
    z1 = (kk + 1) * vs[2]
    zov = np.maximum(np.minimum(z1[None, :, None], zb1[:, None, :])
                     - np.maximum(z0[None, :, None], zb0[:, None, :]),
                     np.float32(0))                                  # [B,K,N]
    # tie-break epsilon: rho_n scaled by (1+eps_n), eps decreasing in n, so a
    # single reduce_max + is_equal yields the first-occurrence argmax; the C
    # selection weight is divided by (1+eps_n) to compensate exactly.
    eps = (np.float32(15) - np.arange(N, dtype=np.float32)) * np.float32(2.0 ** -20)
    zrho = (zov * invC[:, None, :] * (1 + eps)[None, None, :]).astype(np.float32)
    C_w4 = (C / (1 + eps)[None, :]).astype(np.float32)

    d = c[:, :, 0, :2] - c[:, :, 3, :2]
    h = np.sqrt(d[..., 0] ** 2 + d[..., 1] ** 2)
    hs = np.where(h == 0, np.float32(1), h)
    sin = np.where(h > 0, d[..., 1] / hs, np.float32(0)).astype(np.float32)
    cos = np.where(h > 0, d[..., 0] / hs, np.float32(1)).astype(np.float32)

    # --- stage-2 per-partition columns, p = b_lo*64 + n*4 + e, per h-iter ---
    def colpack(a):    # [B,N,4] -> [2h][128]
        return a.reshape(2, 2, N, 4).reshape(2, 128)
    DX_EPS = np.float32(1e-4)
    vert = np.abs(dx) < DX_EPS
    w1 = np.where(vert, np.float32(0), dy * inv_dx * np.float32(0.5))
    w2 = np.where(vert, dy, np.float32(0))
    cols = np.zeros((2, 128, 8), np.float32)
    cols[:, :, 0] = colpack(inv_dy)
    cols[:, :, 1] = colpack(-ya * inv_dy)
    cols[:, :, 2] = colpack((vs[1] - ya) * inv_dy)
    cols[:, :, 3] = colpack(dx)
    cols[:, :, 4] = colpack(xa)
    cols[:, :, 5] = colpack(w1)
    cols[:, :, 6] = colpack(w2)
    cols = np.ascontiguousarray(cols.transpose(1, 0, 2))       # [128,2,8]

    # edge-reduction weights with zrho folded in:
    # rw2[p=(b_lo,n,e), h, (b_lo',k,n')] = (b_lo'==b_lo & n'==n) * dy/2 * zrho[b,k,n]
    novert = not bool(vert.any())
    w1p = colpack(w1)                          # [2,128]
    rw = np.zeros((128, 2, 2, K, N), np.float32)
    for h in range(2):
        for p in range(128):
            b_lo, n = p // 64, (p % 64) // 4
            scale = w1p[h, p] if novert else np.float32(1)
            rw[p, h, b_lo, :, n] = zrho[2 * h + b_lo, :, n] * scale
    rw = np.ascontiguousarray(rw.reshape(128, 2, 2 * K * N))   # [128,2,256]

    # selection matmul weights: w4[b][(k,n), (k',q)] = (k==k') * w_q[b,n]
    # q in {C, sin, cos}
    w4 = np.zeros((128, B, K, 3), np.float32)
    for p in range(128):
        kq, n = p // N, p % N
        w4[p, :, kq, 0] = C_w4[:, n]
        w4[p, :, kq, 1] = sin[:, n]
        w4[p, :, kq, 2] = cos[:, n]
    w4 = np.ascontiguousarray(w4.reshape(128, B * K * 3))      # [128,96]

    # --- cells-major broadcast constants (partition-replicated by host) ---
    kbig = BIG - np.arange(N, dtype=np.float32)                # [16]
    jj = np.arange(NJ, dtype=np.float32) + LOW[1]
    y0 = (jj * vs[1]).astype(np.float32)                       # [64]
    halfvol = np.float32(0.5) * vox_vol

    consts = []
    for m in range(NCORES):
        ii = np.arange(NI, dtype=np.float32) + (m * NI + LOW[0])
        x0 = (ii * vs[0]).astype(np.float32)                   # [8]
        row = np.concatenate([kbig, y0, x0,
                              [halfvol, vs[0], -vs[0], 2 * vs[0]]]).astype(np.float32)
        cc = np.concatenate(
            [np.broadcast_to(row, (128, row.size)), cols.reshape(128, 16)],
            axis=1).astype(np.float32)
        consts.append(np.ascontiguousarray(cc))
    ident = np.eye(128, dtype=np.float32)
    return rw, w4, ident, consts, novert


def _build_v1(stages='all', fuse_w1=False):
    import concourse.bass as bass
    import concourse.tile as tile
    from concourse import bacc, mybir

    f32 = mybir.dt.float32
    ALU = mybir.AluOpType
    ACT = mybir.ActivationFunctionType

    CW = 16 + 64 + 8 + 4 + 16
    OFF_KBIG, OFF_Y0, OFF_X0, OFF_MISC, OFF_COLS = 0, 16, 80, 88, 92

    nc = bacc.Bacc("TRN2", target_bir_lowering=False, debug=False,
                   num_devices=NCORES)
    d_consts = nc.dram_tensor("consts", [128, CW], f32, kind="ExternalInput")
    d_rw = nc.dram_tensor("rw", [128, 2, 256], f32, kind="ExternalInput")
    d_w4 = nc.dram_tensor("w4", [128, B * K * 3], f32, kind="ExternalInput")
    d_ident = nc.dram_tensor("ident", [128, 128], f32, kind="ExternalInput")
    d_out = nc.dram_tensor("out", [B, NCELL * K, 2], f32, kind="ExternalOutput")

    with tile.TileContext(nc) as tc:
        with (
            tc.tile_pool(name="const", bufs=1) as cpool,
            tc.tile_pool(name="small", bufs=4) as spool,
            tc.tile_pool(name="work", bufs=6) as wpool,
            tc.tile_pool(name="edge", bufs=4) as epool,
            tc.tile_pool(name="st3", bufs=6) as tpool,
            tc.tile_pool(name="outp", bufs=4) as opool,
            tc.tile_pool(name="psum", bufs=1, space=bass.MemorySpace.PSUM) as ppool,
            tc.tile_pool(name="psum2", bufs=2, space=bass.MemorySpace.PSUM) as ppool2,
        ):
            tco = cpool.tile([128, CW], f32, tag="consts")
            nc.sync.dma_start(tco[:], d_consts[:])
            trw = cpool.tile([128, 2, 256], f32, tag="rw")
            nc.scalar.dma_start(trw[:], d_rw[:])
            tw4 = cpool.tile([128, B * K * 3], f32, tag="w4")
            nc.sync.dma_start(tw4[:], d_w4[:])
            ident = cpool.tile([128, 128], f32, tag="ident")
            nc.sync.dma_start(ident[:], d_ident[:])

            kbig_bc = tco[:, OFF_KBIG:OFF_KBIG + 16]
            y0_bc = tco[:, OFF_Y0:OFF_Y0 + 64]
            x0_bc = tco[:, OFF_X0:OFF_X0 + 8]
            halfvol_col = tco[:, OFF_MISC:OFF_MISC + 1]
            vs0_col = tco[:, OFF_MISC + 1:OFF_MISC + 2]
            nvs0_col = tco[:, OFF_MISC + 2:OFF_MISC + 3]
            vs0x2_col = tco[:, OFF_MISC + 3:OFF_MISC + 4]

            def bj(ap):   # [128,64] j-tile -> broadcast over i: [128,8,64]
                return ap[:, None, :].broadcast_to([128, NI, NJ])

            def bi(ap):   # [128,8] i-tile -> broadcast over j: [128,8,64]
                return ap[:, :, None].broadcast_to([128, NI, NJ])

            rho_ps = []
            for c in range(NCHUNK):
                rp = ppool.tile([128, B * K * N], f32, tag=f"rho{c}")
                rho_ps.append(rp)
            for h in range(2 if stages != 'none' else 0):
                col = lambda q: tco[:, OFF_COLS + h * 8 + q:OFF_COLS + h * 8 + q + 1]
                # j-only quantities [128, 64]
                ty0 = spool.tile([128, NJ], f32, tag="ty0")
                nc.scalar.activation(ty0[:], y0_bc, ACT.Identity,
                                     bias=col(1), scale=col(0))
                ty1 = spool.tile([128, NJ], f32, tag="ty1")
                nc.scalar.activation(ty1[:], y0_bc, ACT.Identity,
                                     bias=col(2), scale=col(0))
                u0 = spool.tile([128, NJ], f32, tag="u0")
                nc.vector.tensor_scalar(u0[:], ty0[:], 0.0, 1.0, ALU.max, ALU.min)
                u1 = spool.tile([128, NJ], f32, tag="u1")
                nc.vector.tensor_scalar(u1[:], ty1[:], 0.0, 1.0, ALU.max, ALU.min)
                lo = spool.tile([128, NJ], f32, tag="lo")
                nc.vector.tensor_tensor(lo[:], u0[:], u1[:], ALU.min)
                hi = spool.tile([128, NJ], f32, tag="hi")
                nc.vector.tensor_tensor(hi[:], u0[:], u1[:], ALU.max)
                hilo = spool.tile([128, NJ], f32, tag="hilo")
                nc.vector.tensor_tensor(hilo[:], hi[:], lo[:], ALU.subtract)
                # i-only quantities [128, 8]
                x0mxa = spool.tile([128, NI], f32, tag="x0mxa")
                nc.vector.tensor_single_scalar(x0mxa[:], x0_bc, col(4),
                                               ALU.subtract)
                # vertical-edge fallback: Fv = clamp(xa - x0, 0, vs0), * w2
                fvw = spool.tile([128, NI], f32, tag="fvw")
                nc.vector.tensor_scalar(fvw[:], x0mxa[:], -1.0, 0.0,
                                        ALU.mult, ALU.max)
                nc.vector.tensor_single_scalar(fvw[:], fvw[:], vs0_col, ALU.min)
                nc.vector.tensor_single_scalar(fvw[:], fvw[:], col(6), ALU.mult)

                NIH = NI // 2
                for half in range(2):
                    isl = slice(half * NIH, (half + 1) * NIH)

                    def full(tag):
                        t = wpool.tile([128, NIH, NJ], f32, tag=tag)
                        return t

                    def bjh(ap):
                        return ap[:, None, :].broadcast_to([128, NIH, NJ])

                    def bih(ap):
                        return ap[:, isl, None].broadcast_to([128, NIH, NJ])

                    # g at t=lo and t=hi:  g = dx*t - (x0 - xa)
                    glo = full("glo")
                    nc.vector.scalar_tensor_tensor(glo[:], bjh(lo[:]), col(3),
                                                   bih(x0mxa[:]), ALU.mult,
                                                   ALU.subtract)
                    ghi = full("ghi")
                    nc.vector.scalar_tensor_tensor(ghi[:], bjh(hi[:]), col(3),
                                                   bih(x0mxa[:]), ALU.mult,
                                                   ALU.subtract)
                    # H(u) = 0.5*clamp(u,0,c)^2 + c*relu(u-c); w1 carries 0.5
                    clo = full("clo")
                    nc.vector.tensor_scalar(clo[:], glo[:], 0.0, vs0_col,
                                            ALU.max, ALU.min)
                    chi = full("chi")
                    nc.gpsimd.tensor_scalar(chi[:], ghi[:], 0.0, vs0_col,
                                            ALU.max, ALU.min)
                    sqlo = full("sqlo")
                    nc.scalar.activation(sqlo[:], clo[:], ACT.Square)
                    sqhi = full("sqhi")
                    nc.scalar.activation(sqhi[:], chi[:], ACT.Square)
                    rlo = full("rlo")
                    nc.scalar.activation(rlo[:], glo[:], ACT.Relu,
                                         bias=nvs0_col)
                    rhi = full("rhi")
                    nc.scalar.activation(rhi[:], ghi[:], ACT.Relu,
                                         bias=nvs0_col)
                    e1 = full("e1")
                    nc.vector.tensor_tensor(e1[:], sqhi[:], sqlo[:],
                                            ALU.subtract)
                    e2 = full("e2")
                    nc.gpsimd.tensor_tensor(e2[:], rhi[:], rlo[:],
                                            ALU.subtract)
                    iedge = epool.tile([128, NCELL // 2], f32, tag="iedge")
                    if fuse_w1:
                        # w1 folded into rw: iedge = 2c*e2 + e1 directly
                        nc.vector.scalar_tensor_tensor(
                            iedge[:].rearrange("p (i j) -> p i j", j=NJ),
                            e2[:], vs0x2_col, e1[:], ALU.mult, ALU.add)
                    else:
                        s = full("s")
                        nc.vector.scalar_tensor_tensor(s[:], e2[:], vs0x2_col,
                                                       e1[:], ALU.mult, ALU.add)
                        t2w = full("t2w")
                        nc.gpsimd.tensor_tensor(t2w[:], bih(fvw[:]), bjh(hilo[:]),
                                                ALU.mult)
                        nc.vector.scalar_tensor_tensor(
                            iedge[:].rearrange("p (i j) -> p i j", j=NJ),
                            s[:], col(5), t2w[:], ALU.mult, ALU.add)

                    for cc in range(2):
                        cch = half * 2 + cc
                        nc.tensor.matmul(
                            rho_ps[cch][:, h * 256:(h + 1) * 256],
                            iedge[:, cc * 128:(cc + 1) * 128],
                            trw[:, h, :], start=True, stop=True)

            # ---- stage 3, cells-major, per 128-cell chunk ----
            for cch in range(NCHUNK if stages == 'all' else 0):
                rho3 = rho_ps[cch][:].rearrange("p (g n) -> p g n", n=N)
                maxrho = tpool.tile([128, B * K], f32, tag="maxrho")
                nc.vector.reduce_max(maxrho[:], rho3, axis=mybir.AxisListType.X)
                mx_bc = maxrho[:][:, :, None].broadcast_to([128, B * K, N])
                onehot = tpool.tile([128, B * K * N], f32, tag="onehot")
                nc.vector.tensor_tensor(
                    onehot[:].rearrange("p (g n) -> p g n", n=N), rho3, mx_bc,
                    ALU.is_equal)

                # selection sums via PE: transpose onehot per b, then matmul
                # against w4 -> SEL[cell, (k, {C,sin,cos})]
                oh_t = ppool2.tile([128, 4 * 128], f32, tag="oht")
                sel_ps = ppool2.tile([128, B * K * 3], f32, tag="selps")
                for b in range(B):
                    nc.tensor.transpose(
                        oh_t[:, b * 128:(b + 1) * 128],
                        onehot[:, b * 128:(b + 1) * 128], ident[:])
                ohs = tpool.tile([128, 4 * 128], f32, tag="ohs")
                nc.scalar.copy(ohs[:], oh_t[:])
                for b in range(B):
                    nc.tensor.matmul(
                        sel_ps[:, b * K * 3:(b + 1) * K * 3],
                        ohs[:, b * 128:(b + 1) * 128],
                        tw4[:, b * K * 3:(b + 1) * K * 3],
                        start=True, stop=True)

                sel3 = sel_ps[:].rearrange("p (b k q) -> p b k q", k=K, q=3)
                mx3 = maxrho[:].rearrange("p (b k) -> p b k", k=K)
                intersel = tpool.tile([128, B * K], f32, tag="intersel")
                nc.vector.tensor_tensor(
                    intersel[:].rearrange("p (b k) -> p b k", k=K),
                    mx3, sel3[:, :, :, 0], ALU.mult)
                mask = tpool.tile([128, B * K], f32, tag="mask")
                nc.vector.tensor_single_scalar(mask[:], intersel[:],
                                               halfvol_col, ALU.is_gt)
                outt = opool.tile([128, B, K, 2], f32, tag="outt")
                nc.vector.tensor_tensor(
                    outt[:, :, :, 0], sel3[:, :, :, 1],
                    mask[:].rearrange("p (b k) -> p b k", k=K), ALU.mult)
                nc.vector.tensor_tensor(
                    outt[:, :, :, 1], sel3[:, :, :, 2],
                    mask[:].rearrange("p (b k) -> p b k", k=K), ALU.mult)
                dma_eng = nc.sync
                dma_eng.dma_start(
                    d_out[:, cch * 128 * K:(cch + 1) * 128 * K, :]
                         .rearrange("b (p k) e -> p b k e", k=K),
                    outt[:])

    if stages != 'all':
        with tile.TileContext(nc) as tc2:
            with tc2.tile_pool(name="fin", bufs=1) as fpool:
                z = fpool.tile([128, 64], f32, tag="z")
                nc.gpsimd.memset(z[:], 0.0)
                nc.gpsimd.dma_start(
                    d_out[:, 0:1024, :].rearrange("b (p k) e -> p b k e", k=K),
                    z[:].rearrange("p (b k e) -> p b k e", k=K, e=2))
    nc.compile()
    return nc




_COMPILED = None


def kernel(corners3d, neck_voxel_sizes):
    global _COMPILED
    from concourse.bass_utils import run_bass_kernel_spmd

    prep3 = _host_prep_v3(corners3d, neck_voxel_sizes)
    if prep3 is not None:
        consts, rww = prep3
        if _COMPILED is None or _COMPILED[0] != 'v3':
            _COMPILED = ('v3', _build_v3(mm='f32'))
        nc = _COMPILED[1]
        in_maps = [{"consts": consts[m], "rww": rww} for m in range(NCORES)]
    else:
        prep = _host_prep_v2(corners3d, neck_voxel_sizes)
        if prep is not None:
            consts, rww = prep
            if _COMPILED is None or _COMPILED[0] != 'v2':
                _COMPILED = ('v2', _build_v2(chain_mode='late', s3_mode='pairs'))
            nc = _COMPILED[1]
            in_maps = [{"consts": consts[m], "rww": rww} for m in range(NCORES)]
        else:
            rw, w4, ident, consts1, novert = _host_prep_v1(corners3d,
                                                           neck_voxel_sizes)
            if _COMPILED is None or _COMPILED[0] != ('v1', novert):
                _COMPILED = (('v1', novert), _build_v1(fuse_w1=novert))
            nc = _COMPILED[1]
            in_maps = [{"consts": consts1[m], "rw": rw, "w4": w4,
                        "ident": ident} for m in range(NCORES)]
    res = run_bass_kernel_spmd(nc, in_maps, list(range(NCORES)))
    out = np.zeros((B, V, 2), np.float32)
    for m in range(NCORES):
        blk = res.results[m]["out"]
        out[:, m * NCELL * K:(m + 1) * NCELL * K, :] = blk
    return out.reshape(B * V, 2)
